# revision 14
# baseline (speedup 1.0000x reference)
"""Trainium2 Bass kernel for 3-layer per-task LoRA MLP.

Full-input contract: kernel(**inputs) takes the unsharded tensors and returns
the full [8, 1024, 1024] output. Internally the task axis (t=8) is sharded
across 8 NeuronCores (one task per core).

Strategy:
  - LoRA is folded on the host into per-task effective weights
    W_eff = k + (alpha/r) * d @ u  (standard LoRA weight merging), so the
    device kernel is a plain 3-layer MLP — no rank-8 matmuls on the PE.
  - weights and activations are bf16 on device (1 cycle/row on the PE, same
    as f32r, but half the DMA traffic and SBUF footprint); PSUM accumulation
    stays f32. Measured pipeline error ~4e-3 relative.
  - x is pre-transposed on the host so activations live as h^T
    [feat(part), batch(free)] with zero on-device transposes; the final
    layer uses h2^T as the *stationary* operand and w2 as the moving
    operand, producing natural-layout [batch, feat] output directly.
  - weights are pre-packed on the host into the exact SBUF tile layout so
    every DMA is >=2KB-contiguous per partition.
  - layer-2 bias arrives pre-broadcast [128, H3] and is added by the DVE
    while draining PSUM; layer-0/1 biases ride the activation instruction.
"""

import sys

if "/opt/trn_rl_repo" not in sys.path:
    sys.path.insert(0, "/opt/trn_rl_repo")

import numpy as np

T, B, D = 8, 1024, 1024
H1, H2, H3 = 2048, 2048, 1024
SCALING = 2.0  # alpha/rank = 16/8
P = 128
NT = 512  # PSUM free-dim tile (fp32 one-bank limit)

_CACHE = {}


def _build():
    import concourse.mybir as mybir
    from concourse import bacc
    from concourse.tile import TileContext
    from concourse.bass import ts

    f32 = mybir.dt.float32
    bf16 = mybir.dt.bfloat16
    AF = mybir.ActivationFunctionType

    nc = bacc.Bacc(None, target_bir_lowering=False, name="lora_mlp")

    KT0 = D // P      # 8  k-tiles, layer 0
    KT1 = H1 // P     # 16 k-tiles, layer 1
    KT2 = H2 // P     # 16 k-tiles, layer 2
    MT0 = H1 // P     # 16 m-tiles, layer 0
    MT1 = H2 // P     # 16 m-tiles, layer 1
    BT = B // P       # 8  batch 128-tiles
    NB = B // NT      # 2  batch 512-halves (free dim, layers 0/1)
    N2 = H3 // NT     # 2  feature 512-halves (free dim, layer 2)

    xt = nc.dram_tensor("xt", (D, B), bf16, kind="ExternalInput")
    w0 = nc.dram_tensor("w0", (MT0, P, KT0 * P), bf16, kind="ExternalInput")
    b0 = nc.dram_tensor("b0", (P, MT0), f32, kind="ExternalInput")
    w1 = nc.dram_tensor("w1", (MT1, P, KT1 * P), bf16, kind="ExternalInput")
    b1 = nc.dram_tensor("b1", (P, MT1), f32, kind="ExternalInput")
    w2 = nc.dram_tensor("w2", (H2, H3), bf16, kind="ExternalInput")
    b2 = nc.dram_tensor("b2", (P, H3), f32, kind="ExternalInput")
    out = nc.dram_tensor("out", (B, H3), f32, kind="ExternalOutput")

    with TileContext(nc) as tc:
        with (
            tc.tile_pool(name="main", bufs=1) as pool,
            tc.tile_pool(name="psum", bufs=1, space="PSUM") as pp,
        ):
            # PE warmup: the tensor engine clock ramps with sustained use
            # (0.65 -> 1.2 -> 2.4 GHz over ~3us). Run throwaway matmuls on a
            # memset tile while the first input DMAs land, so the real
            # matmuls start at full clock.
            wu = pool.tile([P, 2 * P], bf16, tag="wu", bufs=1)
            nc.vector.memset(wu, 0.125)
            wps = pp.tile([P, 2 * P], f32, tag="wps", bufs=1)
            for _ in range(10):
                nc.tensor.matmul(wps, wu[:, :P], wu, start=True, stop=True)

            # x^T lives as 16 half-tiles [128, 512], one DMA each (consumers
            # gate on whole-tile writes, so one-DMA-per-tile keeps the first
            # PSUM groups from waiting on later chunks). Loads are spread
            # over the three DMA channels (SP HWDGE, Act HWDGE, Pool SWDGE);
            # the Act queue starts ~1.5us late behind LoadActFuncSet.
            xh = [
                [
                    pool.tile([P, NT], bf16, tag="X", bufs=2 * KT0, name=f"xh{k}_{n}")
                    for n in range(NB)
                ]
                for k in range(KT0)
            ]
            w0t = [None]  # m=0 uses the split w0ta/w0tb tiles below
            for m in range(1, MT0):
                w0t.append(pool.tile([P, KT0 * P], bf16, tag="W0", bufs=6, name=f"w0t{m}"))
            # first m-tile of w0 split in two so k=0..3 don't wait on k=4..7
            w0ta = pool.tile([P, 4 * P], bf16, tag="W0a", bufs=1)
            w0tb = pool.tile([P, 4 * P], bf16, tag="W0b", bufs=1)

            def xdma(eng, k, n):
                eng.dma_start(out=xh[k][n], in_=xt[ts(k, P), ts(n, NT)])

            with tc.high_priority():
                # first halves (n=0): gate the very first PSUM group.
                # Only SP + Pool here — the Act HWDGE queue starts ~1.5us
                # late (LoadActFuncSet) and the scheduler hoists other
                # loads onto it.
                nc.sync.dma_start(out=w0ta, in_=w0[0, :, 0 : 4 * P])
                nc.gpsimd.dma_start(out=w0tb, in_=w0[0, :, 4 * P : 8 * P])
                xdma(nc.sync, 0, 0)
                xdma(nc.gpsimd, 3, 0)
                xdma(nc.sync, 1, 0)
                xdma(nc.gpsimd, 4, 0)
                xdma(nc.sync, 2, 0)
                xdma(nc.gpsimd, 5, 0)
                xdma(nc.gpsimd, 6, 0)
                xdma(nc.gpsimd, 7, 0)
                # second halves (n=1)
                xdma(nc.sync, 0, 1)
                xdma(nc.sync, 1, 1)
                xdma(nc.sync, 2, 1)
                xdma(nc.sync, 3, 1)
                xdma(nc.gpsimd, 4, 1)
                xdma(nc.gpsimd, 5, 1)
                xdma(nc.gpsimd, 6, 1)
                xdma(nc.gpsimd, 7, 1)
                nc.sync.dma_start(out=w0t[1], in_=w0[1])
            b0_sb = pool.tile([P, MT0], f32, tag="b0", bufs=1)
            nc.gpsimd.dma_start(out=b0_sb, in_=b0[:, :])
            b1_sb = pool.tile([P, MT1], f32, tag="b1", bufs=1)
            nc.gpsimd.dma_start(out=b1_sb, in_=b1[:, :])
            b2_sb = pool.tile([P, H3], f32, tag="b2", bufs=1)

            # =================== layer 0 ===================
            # k-accumulation order of the first two groups follows DMA
            # arrival order; later groups have everything resident.
            korder0 = [0, 3, 1, 4, 2, 5, 6, 7]
            korder1 = [0, 1, 4, 2, 5, 3, 6, 7]
            h0T = []
            for m in range(MT0):
                wt = w0t[m]
                if m >= 2:
                    nc.sync.dma_start(out=wt, in_=w0[m])
                ht = pool.tile([P, B], bf16, tag="H0", bufs=MT0)
                h0T.append(ht)
                for n in range(NB):
                    ps = pp.tile([P, NT], f32, tag="pm", bufs=6)
                    if m == 0:
                        ks = korder0 if n == 0 else korder1
                    else:
                        ks = range(KT0)
                    for i, k in enumerate(ks):
                        if m == 0:
                            stat = (w0ta if k < 4 else w0tb)[:, ts(k % 4, P)]
                        else:
                            stat = wt[:, ts(k, P)]
                        nc.tensor.matmul(
                            ps,
                            stat,
                            xh[k][n],
                            start=(i == 0),
                            stop=(i == KT0 - 1),
                        )
                    nc.scalar.activation(
                        ht[:, ts(n, NT)], ps, AF.Relu, bias=b0_sb[:, ts(m, 1)]
                    )

            # =================== layer 1 ===================
            h1T = []
            for m in range(MT1):
                wt = pool.tile([P, KT1 * P], bf16, tag="W1", bufs=6)
                nc.sync.dma_start(out=wt, in_=w1[m])
                ht = pool.tile([P, B], bf16, tag="H1", bufs=MT1)
                h1T.append(ht)
                for n in range(NB):
                    ps = pp.tile([P, NT], f32, tag="pm", bufs=6)
                    for k in range(KT1):
                        nc.tensor.matmul(
                            ps,
                            wt[:, ts(k, P)],
                            h0T[k][:, ts(n, NT)],
                            start=(k == 0),
                            stop=(k == KT1 - 1),
                        )
                    nc.scalar.activation(
                        ht[:, ts(n, NT)], ps, AF.Relu, bias=b1_sb[:, ts(m, 1)]
                    )

            # =================== layer 2 (natural output) ===================
            # w2 streams on the Activation-engine HWDGE queue so it never
            # queues behind the slot-paced w0/w1 stream on the sync queue.
            k2t = []
            for k in range(KT2):
                kt_ = pool.tile([P, H3], bf16, tag="K2", bufs=KT2, name=f"k2t{k}")
                k2t.append(kt_)
                nc.scalar.dma_start(out=kt_, in_=w2[ts(k, P), :])
            nc.scalar.dma_start(out=b2_sb, in_=b2[:, :])
            for m in range(BT):
                ot = pool.tile([P, H3], f32, tag="O", bufs=4)
                # last m-tile runs in 128-wide chunks so the final
                # DVE-add + store chain after the last matmul is short
                nchunks, cw = (N2, NT) if m < BT - 1 else (8, H3 // 8)
                for n in range(nchunks):
                    ps = pp.tile([P, cw], f32, tag="pm", bufs=6)
                    for k in range(KT2):
                        nc.tensor.matmul(
                            ps,
                            h1T[k][:, ts(m, P)],
                            k2t[k][:, ts(n, cw)],
                            start=(k == 0),
                            stop=(k == KT2 - 1),
                        )
                    nc.vector.tensor_add(ot[:, ts(n, cw)], ps, b2_sb[:, ts(n, cw)])
                    nc.scalar.dma_start(
                        out=out[ts(m, P), ts(n, cw)], in_=ot[:, ts(n, cw)]
                    )

    if not nc.is_finalized():
        nc.finalize()
    return nc


def _get_nc():
    if "nc" not in _CACHE:
        _CACHE["nc"] = _build()
    return _CACHE["nc"]


def _task_in_map(inputs, t, bf16, b0c, b1c, b2c):
    W0 = inputs["k0"] + SCALING * (inputs["d0"][:, :, t] @ inputs["u0"][:, :, t])
    W1 = inputs["k1"] + SCALING * (inputs["d1"][:, :, t] @ inputs["u1"][:, :, t])
    W2 = inputs["k2"] + SCALING * (inputs["d2"][:, :, t] @ inputs["u2"][:, :, t])
    # pack [K, M] -> [m, p, k*128+c] with element (m,p,kc) = W[k*128+p, m*128+c]
    w0r = np.ascontiguousarray(
        W0.reshape(8, 128, 16, 128).transpose(2, 1, 0, 3).reshape(16, 128, 1024),
        dtype=bf16,
    )
    w1r = np.ascontiguousarray(
        W1.reshape(16, 128, 16, 128).transpose(2, 1, 0, 3).reshape(16, 128, 2048),
        dtype=bf16,
    )
    w2r = np.ascontiguousarray(W2, dtype=bf16)
    xtr = np.ascontiguousarray(inputs["x"][t].T, dtype=bf16)
    return {
        "xt": xtr,
        "w0": w0r,
        "b0": b0c,
        "w1": w1r,
        "b1": b1c,
        "w2": w2r,
        "b2": b2c,
    }


def build_in_maps(inputs):
    import concurrent.futures

    import ml_dtypes

    bf16 = ml_dtypes.bfloat16
    b0c = np.ascontiguousarray(inputs["b0"].reshape(16, 128).T, dtype=np.float32)
    b1c = np.ascontiguousarray(inputs["b1"].reshape(16, 128).T, dtype=np.float32)
    b2c = np.ascontiguousarray(
        np.broadcast_to(inputs["b2"], (P, H3)), dtype=np.float32
    )
    with concurrent.futures.ThreadPoolExecutor(max_workers=T) as ex:
        in_maps = list(
            ex.map(lambda t: _task_in_map(inputs, t, bf16, b0c, b1c, b2c), range(T))
        )
    return in_maps


def kernel(**inputs):
    from concourse import bass_utils

    nc = _get_nc()
    in_maps = build_in_maps(inputs)
    res = bass_utils.run_bass_kernel_spmd(nc, in_maps, core_ids=list(range(T)))
    return np.stack([r["out"] for r in res.results], axis=0)


# revision 17
# speedup vs baseline: 1.1193x; 1.1193x over previous
"""Trainium2 Bass kernel for 3-layer per-task LoRA MLP.

Full-input contract: kernel(**inputs) takes the unsharded tensors and returns
the full [8, 1024, 1024] output. Internally the task axis (t=8) is sharded
across 8 NeuronCores (one task per core).

Strategy:
  - LoRA is folded on the host into per-task effective weights
    W_eff = k + (alpha/r) * d @ u  (standard LoRA weight merging), so the
    device kernel is a plain 3-layer MLP — no rank-8 matmuls on the PE.
  - weights and activations are bf16 on device (1 cycle/row on the PE, same
    as f32r, but half the DMA traffic and SBUF footprint); PSUM accumulation
    stays f32.
  - every layer runs one level of Strassen over 2x2 quadrant blocks: 7
    half-size products instead of 8 cuts PE row-streaming by 12.5% per
    layer. Weight-side operand combinations are free on the host; the
    activation-side combinations and C-quadrant accumulations are spread
    across the Pool/DVE/Act engines (Pool cannot read PSUM; DVE PSUM reads
    cost ~0.76us so each PSUM group gets at most one), overlapped so the
    PE stream stays gap-free. Measured pipeline error ~1e-2 relative
    (gate is 2e-2).
  - activations live as h^T [feat(part), batch(free)] with zero on-device
    transposes; the final layer uses h1^T slices as the *stationary*
    operand, producing natural-layout [batch, feat] output directly.
  - all host-side packs match the exact SBUF tile layout so every DMA is a
    contiguous >=1KB-per-partition stream, one DMA per consumed tile
    (consumers gate on whole-tile writes), spread over the three DMA
    channels (SP/Act HWDGE + Pool SWDGE) in consumption order.
  - a short PE warmup ramps the tensor-engine clock to 2.4 GHz while the
    first DMAs land; the final output quadrant drains in 128-wide chunks
    to shorten the post-PE tail.
"""

import sys

if "/opt/trn_rl_repo" not in sys.path:
    sys.path.insert(0, "/opt/trn_rl_repo")

import numpy as np

T, B, D = 8, 1024, 1024
H1, H2, H3 = 2048, 2048, 1024
SCALING = 2.0  # alpha/rank = 16/8
P = 128
NT = 512  # PSUM free-dim tile (fp32 one-bank limit)

_CACHE = {}


def _build():
    import concourse.mybir as mybir
    from concourse import bacc
    from concourse.tile import TileContext
    from concourse.bass import ts

    f32 = mybir.dt.float32
    bf16 = mybir.dt.bfloat16
    AF = mybir.ActivationFunctionType

    nc = bacc.Bacc(None, target_bir_lowering=False, name="lora_mlp")

    KT0 = D // P      # 8  k-tiles, layer 0
    KT1 = H1 // P     # 16 k-tiles, layer 1
    KT2 = H2 // P     # 16 k-tiles, layer 2
    MT0 = H1 // P     # 16 m-tiles, layer 0
    MT1 = H2 // P     # 16 m-tiles, layer 1
    BT = B // P       # 8  batch 128-tiles
    NB = B // NT      # 2  batch 512-halves (free dim, layers 0/1)
    N2 = H3 // NT     # 2  feature 512-halves (free dim, layer 2)

    # layer-0 inputs arrive Strassen-ready: xs = the 7 moving-operand
    # combos of x^T quadrants [K/2, B/2]; w0 = the 7 stationary operands
    xs = nc.dram_tensor("xt", (7, D // 2, B // 2), bf16, kind="ExternalInput")
    w0 = nc.dram_tensor("w0", (7, 8, P, 4 * P), bf16, kind="ExternalInput")
    b0 = nc.dram_tensor("b0", (P, MT0), f32, kind="ExternalInput")
    # layer-1 weights arrive as the 7 Strassen stationary operands
    # S_i [K/2, M/2], host-combined and packed [i, j(m-tile), p, kk*128+c]
    w1 = nc.dram_tensor("w1", (7, 8, P, 8 * P), bf16, kind="ExternalInput")
    b1 = nc.dram_tensor("b1", (P, MT1), f32, kind="ExternalInput")
    w2 = nc.dram_tensor("w2", (7, 8, P, H3 // 2), bf16, kind="ExternalInput")
    b2 = nc.dram_tensor("b2", (P, H3), f32, kind="ExternalInput")
    out = nc.dram_tensor("out", (B, H3), f32, kind="ExternalOutput")

    with TileContext(nc) as tc:
        with (
            tc.tile_pool(name="main", bufs=1) as pool,
            tc.tile_pool(name="psum", bufs=1, space="PSUM") as pp,
        ):
            # PE warmup: the tensor engine clock ramps with sustained use
            # (0.65 -> 1.2 -> 2.4 GHz over ~3us). Run throwaway matmuls on a
            # memset tile while the first input DMAs land, so the real
            # matmuls start at full clock.
            wu = pool.tile([P, 2 * P], bf16, tag="wu", bufs=1)
            nc.vector.memset(wu, 0.125)
            wps = pp.tile([P, 2 * P], f32, tag="wps", bufs=1)
            for _ in range(10):
                nc.tensor.matmul(wps, wu[:, :P], wu, start=True, stop=True)

            b0_sb = pool.tile([P, MT0], f32, tag="b0", bufs=1)
            b1_sb = pool.tile([P, MT1], f32, tag="b1", bufs=1)
            b2_sb = pool.tile([P, H3], f32, tag="b2", bufs=1)

            # =================== layer 0 (one-level Strassen) ===================
            # C = W0^T @ x over (512 x 1024) quadrant blocks; both operand
            # combos come pre-built on the host, so the device only runs the
            # 7 products and the C-quadrant accumulation. The product loop is
            # OUTER so each product's 4 moving tiles (TX ring, prefetched one
            # product ahead on Pool/Act queues) serve 8 consecutive groups —
            # the DMA stream never races the PE. First-product copies into
            # each C quadrant run on the Act engine, accumulating adds on
            # DVE, bias+relu on Pool (tensor_scalar add+max).
            h0T = [
                pool.tile([P, B], bf16, tag="H0", bufs=MT0, name=f"h0T{m}")
                for m in range(MT0)
            ]
            lo, hi = ts(0, NT), ts(1, NT)
            PORD0 = (2, 1, 3, 4, 5, 6, 7)
            alu = mybir.AluOpType

            def prelu(dst, src, bcol):
                nc.gpsimd.tensor_scalar(dst, src, bcol, 0.0, alu.add, alu.max)

            TX = {}

            def tx_load(i):
                TX[i] = [
                    pool.tile([P, NT], bf16, tag="TX", bufs=8, name=f"TX{i}_{kk}")
                    for kk in range(4)
                ]
                for kk in range(4):
                    eng = nc.gpsimd if kk < 2 else nc.scalar
                    eng.dma_start(out=TX[i][kk], in_=xs[i - 1, ts(kk, P), :])

            C = {q: [None] * 8 for q in (11, 12, 21, 22)}
            KH = KT1 // 2  # 8 k-tiles per K-half of layer 1
            Tc = {
                i: [
                    pool.tile([P, NT], bf16, tag="T", bufs=40, name=f"T{i}_{kk}")
                    for kk in range(KH)
                ]
                for i in (1, 3, 4, 6, 7)
            }
            with tc.high_priority():
                tx_load(PORD0[0])
                tx_load(PORD0[1])
                nc.gpsimd.dma_start(out=b0_sb, in_=b0[:, :])
                nc.gpsimd.dma_start(out=b1_sb, in_=b1[:, :])
            for idx, i in enumerate(PORD0):
                if idx + 2 < len(PORD0):
                    tx_load(PORD0[idx + 2])
                if i == 7:
                    # layer-1's T7 combos (Q21+Q22) only need L0's C21/C22
                    # quadrants — emit them ahead of i7's Pool relus so the
                    # Pool stream has them ready before L1's second product
                    for kk in range(KH):
                        nc.gpsimd.tensor_add(
                            Tc[7][kk], h0T[8 + kk][:, lo], h0T[8 + kk][:, hi]
                        )
                for j in range(8):
                    st = pool.tile([P, 4 * P], bf16, tag="W0s", bufs=6, name=f"s0_{i}_{j}")
                    nc.sync.dma_start(out=st, in_=w0[i - 1, j])
                    ps = pp.tile([P, NT], f32, tag="pm", bufs=6, name=f"q{i}_{j}")
                    for kk in range(4):
                        nc.tensor.matmul(
                            ps,
                            st[:, ts(kk, P)],
                            TX[i][kk],
                            start=(kk == 0),
                            stop=(kk == 3),
                        )
                    # Pool cannot read PSUM, and one DVE PSUM-read op costs
                    # ~0.76us vs the 0.85us group cadence — so each group
                    # carries exactly one DVE op; ACT copies shared products
                    # (M4, M5) to SBUF where Pool runs the second add.
                    if i == 2:
                        C[21][j] = pool.tile([P, NT], bf16, tag="X", bufs=32, name=f"d21_{j}")
                        C[22][j] = pool.tile([P, NT], bf16, tag="X", bufs=32, name=f"d22_{j}")
                        nc.scalar.copy(C[21][j], ps)
                        nc.vector.tensor_scalar_mul(C[22][j], ps, -1.0)
                    elif i == 1:
                        C[11][j] = pool.tile([P, NT], bf16, tag="X", bufs=32, name=f"d11_{j}")
                        nc.scalar.copy(C[11][j], ps)
                        nc.vector.tensor_add(C[22][j], C[22][j], ps)
                    elif i == 3:
                        C[12][j] = pool.tile([P, NT], bf16, tag="X", bufs=32, name=f"d12_{j}")
                        nc.scalar.copy(C[12][j], ps)
                        nc.vector.tensor_add(C[22][j], C[22][j], ps)
                    elif i == 4:
                        e4 = pool.tile([P, NT], bf16, tag="E", bufs=3, name=f"e4_{j}")
                        nc.scalar.copy(e4, ps)
                        nc.vector.tensor_add(C[11][j], C[11][j], ps)
                        nc.gpsimd.tensor_add(C[21][j], C[21][j], e4)
                        prelu(h0T[8 + j][:, lo], C[21][j], b0_sb[:, ts(8 + j, 1)])
                    elif i == 5:
                        e5 = pool.tile([P, NT], bf16, tag="E", bufs=3, name=f"e5_{j}")
                        nc.scalar.copy(e5, ps)
                        nc.vector.tensor_sub(C[11][j], C[11][j], ps)
                        nc.gpsimd.tensor_add(C[12][j], C[12][j], e5)
                        prelu(h0T[j][:, hi], C[12][j], b0_sb[:, ts(j, 1)])
                    elif i == 6:
                        nc.vector.tensor_add(C[22][j], C[22][j], ps)
                        prelu(h0T[8 + j][:, hi], C[22][j], b0_sb[:, ts(8 + j, 1)])
                    elif i == 7:
                        nc.vector.tensor_add(C[11][j], C[11][j], ps)
                        prelu(h0T[j][:, lo], C[11][j], b0_sb[:, ts(j, 1)])

            # =================== layer 1 (one-level Strassen) ===================
            # C = W1^T @ h0 over 1024x1024 quadrants: 7 half-size products
            # instead of 8 (PE rows 229376 vs 262144). Weight-side combos
            # S1..S7 are free on the host; activation-side combos T run on
            # the otherwise-idle Pool engine; products accumulate into the
            # four C quadrants via DVE reads of each product's PSUM bank.
            #   quadrant views of h0: Q11/Q12 = h0T[kk] cols lo/hi,
            #                         Q21/Q22 = h0T[8+kk] cols lo/hi
            # Product order is chosen by operand availability: L0 finishes
            # quadrants in the order C21(i4), C12(i5), C22(i6), C11(i7), so
            # L1 opens with M5 (pure Q22) and M7 (Q21+Q22) whose inputs are
            # ready a product-width before L0's PE stream even ends (T7's
            # combos were emitted inside the L0 loop for the same reason);
            # remaining combos are product-major so the Pool engine always
            # has a full product window of slack.
            for i, fn, sel in (
                (1, nc.gpsimd.tensor_add, lambda a, b_: (a[:, lo], b_[:, hi])),  # Q11+Q22
                (3, nc.gpsimd.tensor_sub, lambda a, b_: (a[:, hi], b_[:, hi])),  # Q12-Q22
                (4, nc.gpsimd.tensor_sub, lambda a, b_: (b_[:, lo], a[:, lo])),  # Q21-Q11
                (6, nc.gpsimd.tensor_add, lambda a, b_: (a[:, lo], a[:, hi])),   # Q11+Q12
            ):
                for kk in range(KH):
                    x0, x1 = sel(h0T[kk], h0T[8 + kk])
                    fn(Tc[i][kk], x0, x1)

            def mov1(i, kk):
                if i == 2:
                    return h0T[kk][:, lo]       # Q11
                if i == 5:
                    return h0T[8 + kk][:, hi]   # Q22
                return Tc[i][kk]

            h1T = [
                pool.tile([P, B], bf16, tag="H1", bufs=MT1, name=f"h1T{m}")
                for m in range(MT1)
            ]
            # product order by L0-output availability (see combo comment):
            #   C11 = M1 + M4 - M5 + M7   (built as -M5, +M7, +M1, +M4)
            #   C12 = M3 + M5             (copy M5, +M3)
            #   C21 = M2 + M4             (copy M2, +M4)
            #   C22 = M1 - M2 + M3 + M6   (built as -M2, +M1, +M3, +M6)
            PORDER = (5, 7, 2, 1, 3, 4, 6)
            for j in range(8):
                for i in PORDER:
                    st = pool.tile([P, KH * P], bf16, tag="W1", bufs=6, name=f"s{i}_{j}")
                    nc.sync.dma_start(out=st, in_=w1[i - 1, j])
                    ps = pp.tile([P, NT], f32, tag="pm", bufs=6, name=f"p{i}_{j}")
                    for kk in range(KH):
                        nc.tensor.matmul(
                            ps,
                            st[:, ts(kk, P)],
                            mov1(i, kk),
                            start=(kk == 0),
                            stop=(kk == KH - 1),
                        )
                    if i == 5:
                        c11 = pool.tile([P, NT], bf16, tag="X", bufs=32, name=f"c11_{j}")
                        c12 = pool.tile([P, NT], bf16, tag="X", bufs=32, name=f"c12_{j}")
                        nc.vector.tensor_scalar_mul(c11, ps, -1.0)
                        nc.scalar.copy(c12, ps)
                    elif i == 7:
                        nc.vector.tensor_add(c11, c11, ps)
                    elif i == 2:
                        c21 = pool.tile([P, NT], bf16, tag="X", bufs=32, name=f"c21_{j}")
                        c22 = pool.tile([P, NT], bf16, tag="X", bufs=32, name=f"c22_{j}")
                        nc.scalar.copy(c21, ps)
                        nc.vector.tensor_scalar_mul(c22, ps, -1.0)
                    elif i == 1:
                        nc.vector.tensor_add(c11, c11, ps)
                        nc.vector.tensor_add(c22, c22, ps)
                    elif i == 3:
                        nc.vector.tensor_add(c12, c12, ps)
                        nc.vector.tensor_add(c22, c22, ps)
                        nc.scalar.activation(
                            h1T[j][:, hi], c12, AF.Relu, bias=b1_sb[:, ts(j, 1)]
                        )
                    elif i == 4:
                        nc.vector.tensor_add(c11, c11, ps)
                        nc.vector.tensor_add(c21, c21, ps)
                        nc.scalar.activation(
                            h1T[8 + j][:, lo], c21, AF.Relu, bias=b1_sb[:, ts(8 + j, 1)]
                        )
                        nc.scalar.activation(
                            h1T[j][:, lo], c11, AF.Relu, bias=b1_sb[:, ts(j, 1)]
                        )
                    elif i == 6:
                        nc.vector.tensor_add(c22, c22, ps)
                        nc.scalar.activation(
                            h1T[8 + j][:, hi], c22, AF.Relu, bias=b1_sb[:, ts(8 + j, 1)]
                        )

            # =================== layer 2 (one-level Strassen) ===================
            # out = h1 @ W2 over quadrants: the W2-side combos arrive from the
            # host as the 7 moving operands (TW ring, product-outer like L0);
            # the h1-side stationary combos SC run on Pool, emitted in the
            # order products consume them. Finished C quadrants get their
            # (free-dim) bias added on Pool and store immediately.
            nc.gpsimd.dma_start(out=b2_sb, in_=b2[:, :])
            TW = {}

            def tw_load(i):
                TW[i] = [
                    pool.tile([P, NT], bf16, tag="TW", bufs=16, name=f"TW{i}_{kk}")
                    for kk in range(KH)
                ]
                for kk in range(KH):
                    nc.sync.dma_start(out=TW[i][kk], in_=w2[i - 1, kk])

            SC = {
                i: [
                    pool.tile([P, NT], bf16, tag="T", bufs=40, name=f"SC{i}_{kk}")
                    for kk in range(KH)
                ]
                for i in (1, 2, 5, 6, 7)
            }
            for i, fn, sel in (
                (5, nc.gpsimd.tensor_add, lambda a, b_: (a[:, lo], b_[:, lo])),  # P11+P12
                (6, nc.gpsimd.tensor_sub, lambda a, b_: (a[:, hi], a[:, lo])),   # P21-P11
                (2, nc.gpsimd.tensor_add, lambda a, b_: (a[:, hi], b_[:, hi])),  # P21+P22
                (1, nc.gpsimd.tensor_add, lambda a, b_: (a[:, lo], b_[:, hi])),  # P11+P22
                (7, nc.gpsimd.tensor_sub, lambda a, b_: (b_[:, lo], b_[:, hi])),  # P12-P22
            ):
                for kk in range(KH):
                    x0, x1 = sel(h1T[kk], h1T[8 + kk])
                    fn(SC[i][kk], x0, x1)

            def stat2(i, j, kk):
                if i == 3:
                    return h1T[kk][:, ts(j, P)]                    # P11
                if i == 4:
                    return h1T[8 + kk][:, NT + j * P : NT + (j + 1) * P]  # P22
                return SC[i][kk][:, ts(j, P)]

            # quadrant -> (out row block base, out col half)
            QOUT = {11: (0, 0), 12: (0, 1), 21: (1, 0), 22: (1, 1)}

            def store_quad(q, j, ctile, chunks=1):
                rbase, chalf = QOUT[q]
                cw2 = NT // chunks
                for c in range(chunks):
                    o5 = pool.tile([P, cw2], f32, tag="O5", bufs=8, name=f"o{q}_{j}_{c}")
                    nc.gpsimd.tensor_add(
                        o5, ctile[:, ts(c, cw2)],
                        b2_sb[:, chalf * NT + c * cw2 : chalf * NT + (c + 1) * cw2],
                    )
                    nc.scalar.dma_start(
                        out=out[
                            rbase * NT + j * P : rbase * NT + (j + 1) * P,
                            chalf * NT + c * cw2 : chalf * NT + (c + 1) * cw2,
                        ],
                        in_=o5,
                    )

            # C11 = M1+M4-M5+M7, C12 = M3+M5, C21 = M2+M4, C22 = M1-M2+M3+M6
            PORD2 = (5, 3, 2, 6, 1, 4, 7)
            D2 = {q: [None] * 4 for q in (11, 12, 21, 22)}
            tw_load(PORD2[0])
            tw_load(PORD2[1])
            for idx, i in enumerate(PORD2):
                if idx + 2 < len(PORD2):
                    tw_load(PORD2[idx + 2])
                for j in range(4):
                    ps = pp.tile([P, NT], f32, tag="pm", bufs=6, name=f"r{i}_{j}")
                    for kk in range(KH):
                        nc.tensor.matmul(
                            ps,
                            stat2(i, j, kk),
                            TW[i][kk],
                            start=(kk == 0),
                            stop=(kk == KH - 1),
                        )
                    if i == 5:
                        D2[11][j] = pool.tile([P, NT], bf16, tag="X", bufs=32, name=f"g11_{j}")
                        D2[12][j] = pool.tile([P, NT], bf16, tag="X", bufs=32, name=f"g12_{j}")
                        nc.vector.tensor_scalar_mul(D2[11][j], ps, -1.0)
                        nc.scalar.copy(D2[12][j], ps)
                    elif i == 3:
                        D2[22][j] = pool.tile([P, NT], bf16, tag="X", bufs=32, name=f"g22_{j}")
                        nc.vector.tensor_add(D2[12][j], D2[12][j], ps)
                        nc.scalar.copy(D2[22][j], ps)
                        store_quad(12, j, D2[12][j])
                    elif i == 2:
                        D2[21][j] = pool.tile([P, NT], bf16, tag="X", bufs=32, name=f"g21_{j}")
                        nc.vector.tensor_sub(D2[22][j], D2[22][j], ps)
                        nc.scalar.copy(D2[21][j], ps)
                    elif i == 6:
                        nc.vector.tensor_add(D2[22][j], D2[22][j], ps)
                    elif i == 1:
                        nc.vector.tensor_add(D2[22][j], D2[22][j], ps)
                        nc.vector.tensor_add(D2[11][j], D2[11][j], ps)
                        store_quad(22, j, D2[22][j])
                    elif i == 4:
                        nc.vector.tensor_add(D2[21][j], D2[21][j], ps)
                        nc.vector.tensor_add(D2[11][j], D2[11][j], ps)
                        store_quad(21, j, D2[21][j])
                    elif i == 7:
                        if j < 3:
                            nc.vector.tensor_add(D2[11][j], D2[11][j], ps)
                            store_quad(11, j, D2[11][j])
                        else:
                            # final quadrant: chunk the whole accumulate ->
                            # bias -> store chain so the post-PE tail is short
                            cw2 = NT // 4
                            for c in range(4):
                                sl = ts(c, cw2)
                                nc.vector.tensor_add(
                                    D2[11][j][:, sl], D2[11][j][:, sl], ps[:, sl]
                                )
                                o5 = pool.tile([P, cw2], f32, tag="O5", bufs=8, name=f"of_{c}")
                                nc.gpsimd.tensor_add(
                                    o5, D2[11][j][:, sl], b2_sb[:, c * cw2 : (c + 1) * cw2]
                                )
                                nc.scalar.dma_start(
                                    out=out[j * P : (j + 1) * P, c * cw2 : (c + 1) * cw2],
                                    in_=o5,
                                )

    if not nc.is_finalized():
        nc.finalize()
    return nc


def _get_nc():
    if "nc" not in _CACHE:
        _CACHE["nc"] = _build()
    return _CACHE["nc"]


def _task_in_map(inputs, t, bf16, b0c, b1c, b2c):
    W0 = inputs["k0"] + SCALING * (inputs["d0"][:, :, t] @ inputs["u0"][:, :, t])
    W1 = inputs["k1"] + SCALING * (inputs["d1"][:, :, t] @ inputs["u1"][:, :, t])
    W2 = inputs["k2"] + SCALING * (inputs["d2"][:, :, t] @ inputs["u2"][:, :, t])
    # layer-0 Strassen: both operand sets host-combined.
    # stationary S_i from W0 (512 x 1024) blocks
    blk0 = W0.reshape(2, 512, 2, 1024)
    S0 = np.stack(
        [
            blk0[0, :, 0] + blk0[1, :, 1],
            blk0[0, :, 1] + blk0[1, :, 1],
            blk0[0, :, 0],
            blk0[1, :, 1],
            blk0[0, :, 0] + blk0[1, :, 0],
            blk0[0, :, 1] - blk0[0, :, 0],
            blk0[1, :, 0] - blk0[1, :, 1],
        ]
    )  # [7, K/2, M/2]
    w0r = np.ascontiguousarray(
        S0.reshape(7, 4, 128, 8, 128).transpose(0, 3, 2, 1, 4).reshape(7, 8, 128, 512),
        dtype=bf16,
    )
    # moving combos T_i from x^T quadrants
    xT = inputs["x"][t].T
    Q11, Q12 = xT[:512, :512], xT[:512, 512:]
    Q21, Q22 = xT[512:, :512], xT[512:, 512:]
    xsr = np.ascontiguousarray(
        np.stack(
            [Q11 + Q22, Q11, Q12 - Q22, Q21 - Q11, Q22, Q11 + Q12, Q21 + Q22]
        ),
        dtype=bf16,
    )
    # layer-1 Strassen stationary operands from W1 quadrants blk[r, c]
    blk = W1.reshape(2, 1024, 2, 1024)
    S = np.stack(
        [
            blk[0, :, 0] + blk[1, :, 1],  # (P11+P22)^T
            blk[0, :, 1] + blk[1, :, 1],  # (P21+P22)^T
            blk[0, :, 0],                 # P11^T
            blk[1, :, 1],                 # P22^T
            blk[0, :, 0] + blk[1, :, 0],  # (P11+P12)^T
            blk[0, :, 1] - blk[0, :, 0],  # (P21-P11)^T
            blk[1, :, 0] - blk[1, :, 1],  # (P12-P22)^T
        ]
    )  # [7, K/2, M/2]
    w1r = np.ascontiguousarray(
        S.reshape(7, 8, 128, 8, 128).transpose(0, 3, 2, 1, 4).reshape(7, 8, 128, 1024),
        dtype=bf16,
    )
    # layer-2 Strassen moving operands from W2 (1024 x 512) blocks
    blk2 = W2.reshape(2, 1024, 2, 512)
    S2_ = np.stack(
        [
            blk2[0, :, 0] + blk2[1, :, 1],
            blk2[0, :, 0],
            blk2[0, :, 1] - blk2[1, :, 1],
            blk2[1, :, 0] - blk2[0, :, 0],
            blk2[1, :, 1],
            blk2[0, :, 0] + blk2[0, :, 1],
            blk2[1, :, 0] + blk2[1, :, 1],
        ]
    )  # [7, K/2, M3/2]
    w2r = np.ascontiguousarray(S2_.reshape(7, 8, 128, 512), dtype=bf16)
    return {
        "xt": xsr,
        "w0": w0r,
        "b0": b0c,
        "w1": w1r,
        "b1": b1c,
        "w2": w2r,
        "b2": b2c,
    }


def build_in_maps(inputs):
    import concurrent.futures

    import ml_dtypes

    bf16 = ml_dtypes.bfloat16
    b0c = np.ascontiguousarray(inputs["b0"].reshape(16, 128).T, dtype=np.float32)
    b1c = np.ascontiguousarray(inputs["b1"].reshape(16, 128).T, dtype=np.float32)
    b2c = np.ascontiguousarray(
        np.broadcast_to(inputs["b2"], (P, H3)), dtype=np.float32
    )
    with concurrent.futures.ThreadPoolExecutor(max_workers=T) as ex:
        in_maps = list(
            ex.map(lambda t: _task_in_map(inputs, t, bf16, b0c, b1c, b2c), range(T))
        )
    return in_maps


def kernel(**inputs):
    from concourse import bass_utils

    nc = _get_nc()
    in_maps = build_in_maps(inputs)
    res = bass_utils.run_bass_kernel_spmd(nc, in_maps, core_ids=list(range(T)))
    return np.stack([r["out"] for r in res.results], axis=0)


# revision 18
# speedup vs baseline: 1.1271x; 1.0070x over previous
"""Trainium2 Bass kernel for 3-layer per-task LoRA MLP.

Full-input contract: kernel(**inputs) takes the unsharded tensors and returns
the full [8, 1024, 1024] output. Internally the task axis (t=8) is sharded
across 8 NeuronCores (one task per core).

Strategy:
  - LoRA is folded on the host into per-task effective weights
    W_eff = k + (alpha/r) * d @ u  (standard LoRA weight merging), so the
    device kernel is a plain 3-layer MLP — no rank-8 matmuls on the PE.
  - weights and activations are bf16 on device (1 cycle/row on the PE, same
    as f32r, but half the DMA traffic and SBUF footprint); PSUM accumulation
    stays f32.
  - every layer runs one level of Strassen over 2x2 quadrant blocks: 7
    half-size products instead of 8 cuts PE row-streaming by 12.5% per
    layer. Weight-side operand combinations are free on the host; the
    activation-side combinations and C-quadrant accumulations are spread
    across the Pool/DVE/Act engines (Pool cannot read PSUM; DVE PSUM reads
    cost ~0.76us so each PSUM group gets at most one), overlapped so the
    PE stream stays gap-free. Measured pipeline error ~1e-2 relative
    (gate is 2e-2).
  - activations live as h^T [feat(part), batch(free)] with zero on-device
    transposes; the final layer uses h1^T slices as the *stationary*
    operand, producing natural-layout [batch, feat] output directly.
  - all host-side packs match the exact SBUF tile layout so every DMA is a
    contiguous >=1KB-per-partition stream, one DMA per consumed tile
    (consumers gate on whole-tile writes), spread over the three DMA
    channels (SP/Act HWDGE + Pool SWDGE) in consumption order.
  - a short PE warmup ramps the tensor-engine clock to 2.4 GHz while the
    first DMAs land; the final output quadrant drains in 128-wide chunks
    to shorten the post-PE tail.
"""

import sys

if "/opt/trn_rl_repo" not in sys.path:
    sys.path.insert(0, "/opt/trn_rl_repo")

import numpy as np

T, B, D = 8, 1024, 1024
H1, H2, H3 = 2048, 2048, 1024
SCALING = 2.0  # alpha/rank = 16/8
P = 128
NT = 512  # PSUM free-dim tile (fp32 one-bank limit)

_CACHE = {}


def _build():
    import concourse.mybir as mybir
    from concourse import bacc
    from concourse.tile import TileContext
    from concourse.bass import ts

    f32 = mybir.dt.float32
    bf16 = mybir.dt.bfloat16
    AF = mybir.ActivationFunctionType

    nc = bacc.Bacc(None, target_bir_lowering=False, name="lora_mlp")

    KT0 = D // P      # 8  k-tiles, layer 0
    KT1 = H1 // P     # 16 k-tiles, layer 1
    KT2 = H2 // P     # 16 k-tiles, layer 2
    MT0 = H1 // P     # 16 m-tiles, layer 0
    MT1 = H2 // P     # 16 m-tiles, layer 1
    BT = B // P       # 8  batch 128-tiles
    NB = B // NT      # 2  batch 512-halves (free dim, layers 0/1)
    N2 = H3 // NT     # 2  feature 512-halves (free dim, layer 2)

    # layer-0 inputs arrive Strassen-ready: xs = the 7 moving-operand
    # combos of x^T quadrants [K/2, B/2]; w0 = the 7 stationary operands
    xs = nc.dram_tensor("xt", (7, D // 2, B // 2), bf16, kind="ExternalInput")
    w0 = nc.dram_tensor("w0", (7, 8, P, 4 * P), bf16, kind="ExternalInput")
    b0 = nc.dram_tensor("b0", (P, MT0), f32, kind="ExternalInput")
    # layer-1 weights arrive as the 7 Strassen stationary operands
    # S_i [K/2, M/2], host-combined and packed [i, j(m-tile), p, kk*128+c]
    w1 = nc.dram_tensor("w1", (7, 8, P, 8 * P), bf16, kind="ExternalInput")
    b1 = nc.dram_tensor("b1", (P, MT1), f32, kind="ExternalInput")
    w2 = nc.dram_tensor("w2", (7, 8, P, H3 // 2), bf16, kind="ExternalInput")
    b2 = nc.dram_tensor("b2", (P, H3), f32, kind="ExternalInput")
    out = nc.dram_tensor("out", (B, H3), f32, kind="ExternalOutput")

    with TileContext(nc) as tc:
        with (
            tc.tile_pool(name="main", bufs=1) as pool,
            tc.tile_pool(name="psum", bufs=1, space="PSUM") as pp,
        ):
            # PE warmup: the tensor engine clock ramps with sustained use
            # (0.65 -> 1.2 -> 2.4 GHz over ~3us). Run throwaway matmuls on a
            # memset tile while the first input DMAs land, so the real
            # matmuls start at full clock.
            wu = pool.tile([P, 2 * P], bf16, tag="wu", bufs=1)
            nc.vector.memset(wu, 0.125)
            wps = pp.tile([P, 2 * P], f32, tag="wps", bufs=1)
            for _ in range(10):
                nc.tensor.matmul(wps, wu[:, :P], wu, start=True, stop=True)

            b0_sb = pool.tile([P, MT0], f32, tag="b0", bufs=1)
            b1_sb = pool.tile([P, MT1], f32, tag="b1", bufs=1)
            b2_sb = pool.tile([P, H3], f32, tag="b2", bufs=1)

            # =================== layer 0 (one-level Strassen) ===================
            # C = W0^T @ x over (512 x 1024) quadrant blocks; both operand
            # combos come pre-built on the host, so the device only runs the
            # 7 products and the C-quadrant accumulation. The product loop is
            # OUTER so each product's 4 moving tiles (TX ring, prefetched one
            # product ahead on Pool/Act queues) serve 8 consecutive groups —
            # the DMA stream never races the PE. First-product copies into
            # each C quadrant run on the Act engine, accumulating adds on
            # DVE, bias+relu on Pool (tensor_scalar add+max).
            h0T = [
                pool.tile([P, B], bf16, tag="H0", bufs=MT0, name=f"h0T{m}")
                for m in range(MT0)
            ]
            lo, hi = ts(0, NT), ts(1, NT)
            PORD0 = (2, 1, 3, 4, 5, 6, 7)
            alu = mybir.AluOpType

            def prelu(dst, src, bcol):
                nc.gpsimd.tensor_scalar(dst, src, bcol, 0.0, alu.add, alu.max)

            TX = {}

            def tx_load(i):
                TX[i] = [
                    pool.tile([P, NT], bf16, tag="TX", bufs=8, name=f"TX{i}_{kk}")
                    for kk in range(4)
                ]
                for kk in range(4):
                    eng = nc.gpsimd if kk < 2 else nc.scalar
                    eng.dma_start(out=TX[i][kk], in_=xs[i - 1, ts(kk, P), :])

            C = {q: [None] * 8 for q in (11, 12, 21, 22)}
            KH = KT1 // 2  # 8 k-tiles per K-half of layer 1
            Tc = {
                i: [
                    pool.tile([P, NT], bf16, tag="T", bufs=40, name=f"T{i}_{kk}")
                    for kk in range(KH)
                ]
                for i in (1, 3, 4, 6, 7)
            }
            with tc.high_priority():
                tx_load(PORD0[0])
                tx_load(PORD0[1])
                nc.gpsimd.dma_start(out=b0_sb, in_=b0[:, :])
                nc.gpsimd.dma_start(out=b1_sb, in_=b1[:, :])
            for idx, i in enumerate(PORD0):
                if idx + 2 < len(PORD0):
                    tx_load(PORD0[idx + 2])
                if i == 7:
                    # layer-1's T7 combos (Q21+Q22) only need L0's C21/C22
                    # quadrants — emit them ahead of i7's Pool relus, split
                    # across Pool and DVE so they emerge 2x faster than
                    # L1's second product consumes them
                    for kk in range(KH):
                        eng = nc.gpsimd if kk % 2 == 0 else nc.vector
                        eng.tensor_add(
                            Tc[7][kk], h0T[8 + kk][:, lo], h0T[8 + kk][:, hi]
                        )
                for j in range(8):
                    st = pool.tile([P, 4 * P], bf16, tag="W0s", bufs=6, name=f"s0_{i}_{j}")
                    nc.sync.dma_start(out=st, in_=w0[i - 1, j])
                    ps = pp.tile([P, NT], f32, tag="pm", bufs=6, name=f"q{i}_{j}")
                    for kk in range(4):
                        nc.tensor.matmul(
                            ps,
                            st[:, ts(kk, P)],
                            TX[i][kk],
                            start=(kk == 0),
                            stop=(kk == 3),
                        )
                    # Pool cannot read PSUM, and one DVE PSUM-read op costs
                    # ~0.76us vs the 0.85us group cadence — so each group
                    # carries exactly one DVE op; ACT copies shared products
                    # (M4, M5) to SBUF where Pool runs the second add.
                    if i == 2:
                        C[21][j] = pool.tile([P, NT], bf16, tag="X", bufs=32, name=f"d21_{j}")
                        C[22][j] = pool.tile([P, NT], bf16, tag="X", bufs=32, name=f"d22_{j}")
                        nc.scalar.copy(C[21][j], ps)
                        nc.vector.tensor_scalar_mul(C[22][j], ps, -1.0)
                    elif i == 1:
                        C[11][j] = pool.tile([P, NT], bf16, tag="X", bufs=32, name=f"d11_{j}")
                        nc.scalar.copy(C[11][j], ps)
                        nc.vector.tensor_add(C[22][j], C[22][j], ps)
                    elif i == 3:
                        C[12][j] = pool.tile([P, NT], bf16, tag="X", bufs=32, name=f"d12_{j}")
                        nc.scalar.copy(C[12][j], ps)
                        nc.vector.tensor_add(C[22][j], C[22][j], ps)
                    elif i == 4:
                        e4 = pool.tile([P, NT], bf16, tag="E", bufs=3, name=f"e4_{j}")
                        nc.scalar.copy(e4, ps)
                        nc.vector.tensor_add(C[11][j], C[11][j], ps)
                        nc.gpsimd.tensor_add(C[21][j], C[21][j], e4)
                        prelu(h0T[8 + j][:, lo], C[21][j], b0_sb[:, ts(8 + j, 1)])
                    elif i == 5:
                        e5 = pool.tile([P, NT], bf16, tag="E", bufs=3, name=f"e5_{j}")
                        nc.scalar.copy(e5, ps)
                        nc.vector.tensor_sub(C[11][j], C[11][j], ps)
                        nc.gpsimd.tensor_add(C[12][j], C[12][j], e5)
                        prelu(h0T[j][:, hi], C[12][j], b0_sb[:, ts(j, 1)])
                    elif i == 6:
                        nc.vector.tensor_add(C[22][j], C[22][j], ps)
                        prelu(h0T[8 + j][:, hi], C[22][j], b0_sb[:, ts(8 + j, 1)])
                    elif i == 7:
                        nc.vector.tensor_add(C[11][j], C[11][j], ps)
                        prelu(h0T[j][:, lo], C[11][j], b0_sb[:, ts(j, 1)])

            # =================== layer 1 (one-level Strassen) ===================
            # C = W1^T @ h0 over 1024x1024 quadrants: 7 half-size products
            # instead of 8 (PE rows 229376 vs 262144). Weight-side combos
            # S1..S7 are free on the host; activation-side combos T run on
            # the otherwise-idle Pool engine; products accumulate into the
            # four C quadrants via DVE reads of each product's PSUM bank.
            #   quadrant views of h0: Q11/Q12 = h0T[kk] cols lo/hi,
            #                         Q21/Q22 = h0T[8+kk] cols lo/hi
            # Product order is chosen by operand availability: L0 finishes
            # quadrants in the order C21(i4), C12(i5), C22(i6), C11(i7), so
            # L1 opens with M5 (pure Q22) and M7 (Q21+Q22) whose inputs are
            # ready a product-width before L0's PE stream even ends (T7's
            # combos were emitted inside the L0 loop for the same reason);
            # remaining combos are product-major so the Pool engine always
            # has a full product window of slack.
            for i, fn, sel in (
                (1, nc.gpsimd.tensor_add, lambda a, b_: (a[:, lo], b_[:, hi])),  # Q11+Q22
                (3, nc.gpsimd.tensor_sub, lambda a, b_: (a[:, hi], b_[:, hi])),  # Q12-Q22
                (4, nc.gpsimd.tensor_sub, lambda a, b_: (b_[:, lo], a[:, lo])),  # Q21-Q11
                (6, nc.gpsimd.tensor_add, lambda a, b_: (a[:, lo], a[:, hi])),   # Q11+Q12
            ):
                for kk in range(KH):
                    x0, x1 = sel(h0T[kk], h0T[8 + kk])
                    fn(Tc[i][kk], x0, x1)

            def mov1(i, kk):
                if i == 2:
                    return h0T[kk][:, lo]       # Q11
                if i == 5:
                    return h0T[8 + kk][:, hi]   # Q22
                return Tc[i][kk]

            h1T = [
                pool.tile([P, B], bf16, tag="H1", bufs=MT1, name=f"h1T{m}")
                for m in range(MT1)
            ]
            # product order by L0-output availability (see combo comment):
            #   C11 = M1 + M4 - M5 + M7   (built as -M5, +M7, +M1, +M4)
            #   C12 = M3 + M5             (copy M5, +M3)
            #   C21 = M2 + M4             (copy M2, +M4)
            #   C22 = M1 - M2 + M3 + M6   (built as -M2, +M1, +M3, +M6)
            PORDER = (5, 7, 2, 1, 3, 4, 6)
            for j in range(8):
                for i in PORDER:
                    st = pool.tile([P, KH * P], bf16, tag="W1", bufs=6, name=f"s{i}_{j}")
                    nc.sync.dma_start(out=st, in_=w1[i - 1, j])
                    ps = pp.tile([P, NT], f32, tag="pm", bufs=6, name=f"p{i}_{j}")
                    for kk in range(KH):
                        nc.tensor.matmul(
                            ps,
                            st[:, ts(kk, P)],
                            mov1(i, kk),
                            start=(kk == 0),
                            stop=(kk == KH - 1),
                        )
                    if i == 5:
                        c11 = pool.tile([P, NT], bf16, tag="X", bufs=32, name=f"c11_{j}")
                        c12 = pool.tile([P, NT], bf16, tag="X", bufs=32, name=f"c12_{j}")
                        nc.vector.tensor_scalar_mul(c11, ps, -1.0)
                        nc.scalar.copy(c12, ps)
                    elif i == 7:
                        nc.vector.tensor_add(c11, c11, ps)
                    elif i == 2:
                        c21 = pool.tile([P, NT], bf16, tag="X", bufs=32, name=f"c21_{j}")
                        c22 = pool.tile([P, NT], bf16, tag="X", bufs=32, name=f"c22_{j}")
                        nc.scalar.copy(c21, ps)
                        nc.vector.tensor_scalar_mul(c22, ps, -1.0)
                    elif i == 1:
                        nc.vector.tensor_add(c11, c11, ps)
                        nc.vector.tensor_add(c22, c22, ps)
                    elif i == 3:
                        nc.vector.tensor_add(c12, c12, ps)
                        nc.vector.tensor_add(c22, c22, ps)
                        nc.scalar.activation(
                            h1T[j][:, hi], c12, AF.Relu, bias=b1_sb[:, ts(j, 1)]
                        )
                    elif i == 4:
                        nc.vector.tensor_add(c11, c11, ps)
                        nc.vector.tensor_add(c21, c21, ps)
                        nc.scalar.activation(
                            h1T[8 + j][:, lo], c21, AF.Relu, bias=b1_sb[:, ts(8 + j, 1)]
                        )
                        nc.scalar.activation(
                            h1T[j][:, lo], c11, AF.Relu, bias=b1_sb[:, ts(j, 1)]
                        )
                    elif i == 6:
                        nc.vector.tensor_add(c22, c22, ps)
                        nc.scalar.activation(
                            h1T[8 + j][:, hi], c22, AF.Relu, bias=b1_sb[:, ts(8 + j, 1)]
                        )

            # =================== layer 2 (one-level Strassen) ===================
            # out = h1 @ W2 over quadrants: the W2-side combos arrive from the
            # host as the 7 moving operands (TW ring, product-outer like L0);
            # the h1-side stationary combos SC run on Pool, emitted in the
            # order products consume them. Finished C quadrants get their
            # (free-dim) bias added on Pool and store immediately.
            nc.gpsimd.dma_start(out=b2_sb, in_=b2[:, :])
            TW = {}

            def tw_load(i):
                TW[i] = [
                    pool.tile([P, NT], bf16, tag="TW", bufs=16, name=f"TW{i}_{kk}")
                    for kk in range(KH)
                ]
                for kk in range(KH):
                    nc.sync.dma_start(out=TW[i][kk], in_=w2[i - 1, kk])

            SC = {
                i: [
                    pool.tile([P, NT], bf16, tag="T", bufs=40, name=f"SC{i}_{kk}")
                    for kk in range(KH)
                ]
                for i in (1, 2, 5, 6, 7)
            }
            for i, fn, sel in (
                (5, nc.gpsimd.tensor_add, lambda a, b_: (a[:, lo], b_[:, lo])),  # P11+P12
                (6, nc.gpsimd.tensor_sub, lambda a, b_: (a[:, hi], a[:, lo])),   # P21-P11
                (2, nc.gpsimd.tensor_add, lambda a, b_: (a[:, hi], b_[:, hi])),  # P21+P22
                (1, nc.gpsimd.tensor_add, lambda a, b_: (a[:, lo], b_[:, hi])),  # P11+P22
                (7, nc.gpsimd.tensor_sub, lambda a, b_: (b_[:, lo], b_[:, hi])),  # P12-P22
            ):
                for kk in range(KH):
                    x0, x1 = sel(h1T[kk], h1T[8 + kk])
                    fn(SC[i][kk], x0, x1)

            def stat2(i, j, kk):
                if i == 3:
                    return h1T[kk][:, ts(j, P)]                    # P11
                if i == 4:
                    return h1T[8 + kk][:, NT + j * P : NT + (j + 1) * P]  # P22
                return SC[i][kk][:, ts(j, P)]

            # quadrant -> (out row block base, out col half)
            QOUT = {11: (0, 0), 12: (0, 1), 21: (1, 0), 22: (1, 1)}

            def store_quad(q, j, ctile, chunks=1):
                rbase, chalf = QOUT[q]
                cw2 = NT // chunks
                for c in range(chunks):
                    o5 = pool.tile([P, cw2], f32, tag="O5", bufs=8, name=f"o{q}_{j}_{c}")
                    nc.gpsimd.tensor_add(
                        o5, ctile[:, ts(c, cw2)],
                        b2_sb[:, chalf * NT + c * cw2 : chalf * NT + (c + 1) * cw2],
                    )
                    nc.scalar.dma_start(
                        out=out[
                            rbase * NT + j * P : rbase * NT + (j + 1) * P,
                            chalf * NT + c * cw2 : chalf * NT + (c + 1) * cw2,
                        ],
                        in_=o5,
                    )

            # C11 = M1+M4-M5+M7, C12 = M3+M5, C21 = M2+M4, C22 = M1-M2+M3+M6
            PORD2 = (5, 3, 2, 6, 1, 4, 7)
            D2 = {q: [None] * 4 for q in (11, 12, 21, 22)}
            tw_load(PORD2[0])
            tw_load(PORD2[1])
            for idx, i in enumerate(PORD2):
                if idx + 2 < len(PORD2):
                    tw_load(PORD2[idx + 2])
                for j in range(4):
                    ps = pp.tile([P, NT], f32, tag="pm", bufs=6, name=f"r{i}_{j}")
                    for kk in range(KH):
                        nc.tensor.matmul(
                            ps,
                            stat2(i, j, kk),
                            TW[i][kk],
                            start=(kk == 0),
                            stop=(kk == KH - 1),
                        )
                    if i == 5:
                        D2[11][j] = pool.tile([P, NT], bf16, tag="X", bufs=32, name=f"g11_{j}")
                        D2[12][j] = pool.tile([P, NT], bf16, tag="X", bufs=32, name=f"g12_{j}")
                        nc.vector.tensor_scalar_mul(D2[11][j], ps, -1.0)
                        nc.scalar.copy(D2[12][j], ps)
                    elif i == 3:
                        D2[22][j] = pool.tile([P, NT], bf16, tag="X", bufs=32, name=f"g22_{j}")
                        nc.vector.tensor_add(D2[12][j], D2[12][j], ps)
                        nc.scalar.copy(D2[22][j], ps)
                        store_quad(12, j, D2[12][j])
                    elif i == 2:
                        D2[21][j] = pool.tile([P, NT], bf16, tag="X", bufs=32, name=f"g21_{j}")
                        nc.vector.tensor_sub(D2[22][j], D2[22][j], ps)
                        nc.scalar.copy(D2[21][j], ps)
                    elif i == 6:
                        nc.vector.tensor_add(D2[22][j], D2[22][j], ps)
                    elif i == 1:
                        nc.vector.tensor_add(D2[22][j], D2[22][j], ps)
                        nc.vector.tensor_add(D2[11][j], D2[11][j], ps)
                        store_quad(22, j, D2[22][j])
                    elif i == 4:
                        nc.vector.tensor_add(D2[21][j], D2[21][j], ps)
                        nc.vector.tensor_add(D2[11][j], D2[11][j], ps)
                        store_quad(21, j, D2[21][j])
                    elif i == 7:
                        if j < 3:
                            nc.vector.tensor_add(D2[11][j], D2[11][j], ps)
                            store_quad(11, j, D2[11][j])
                        else:
                            # final quadrant: chunk the whole accumulate ->
                            # bias -> store chain so the post-PE tail is short
                            cw2 = NT // 4
                            for c in range(4):
                                sl = ts(c, cw2)
                                nc.vector.tensor_add(
                                    D2[11][j][:, sl], D2[11][j][:, sl], ps[:, sl]
                                )
                                o5 = pool.tile([P, cw2], f32, tag="O5", bufs=8, name=f"of_{c}")
                                nc.gpsimd.tensor_add(
                                    o5, D2[11][j][:, sl], b2_sb[:, c * cw2 : (c + 1) * cw2]
                                )
                                nc.scalar.dma_start(
                                    out=out[j * P : (j + 1) * P, c * cw2 : (c + 1) * cw2],
                                    in_=o5,
                                )

    if not nc.is_finalized():
        nc.finalize()
    return nc


def _get_nc():
    if "nc" not in _CACHE:
        _CACHE["nc"] = _build()
    return _CACHE["nc"]


def _task_in_map(inputs, t, bf16, b0c, b1c, b2c):
    W0 = inputs["k0"] + SCALING * (inputs["d0"][:, :, t] @ inputs["u0"][:, :, t])
    W1 = inputs["k1"] + SCALING * (inputs["d1"][:, :, t] @ inputs["u1"][:, :, t])
    W2 = inputs["k2"] + SCALING * (inputs["d2"][:, :, t] @ inputs["u2"][:, :, t])
    # layer-0 Strassen: both operand sets host-combined.
    # stationary S_i from W0 (512 x 1024) blocks
    blk0 = W0.reshape(2, 512, 2, 1024)
    S0 = np.stack(
        [
            blk0[0, :, 0] + blk0[1, :, 1],
            blk0[0, :, 1] + blk0[1, :, 1],
            blk0[0, :, 0],
            blk0[1, :, 1],
            blk0[0, :, 0] + blk0[1, :, 0],
            blk0[0, :, 1] - blk0[0, :, 0],
            blk0[1, :, 0] - blk0[1, :, 1],
        ]
    )  # [7, K/2, M/2]
    w0r = np.ascontiguousarray(
        S0.reshape(7, 4, 128, 8, 128).transpose(0, 3, 2, 1, 4).reshape(7, 8, 128, 512),
        dtype=bf16,
    )
    # moving combos T_i from x^T quadrants
    xT = inputs["x"][t].T
    Q11, Q12 = xT[:512, :512], xT[:512, 512:]
    Q21, Q22 = xT[512:, :512], xT[512:, 512:]
    xsr = np.ascontiguousarray(
        np.stack(
            [Q11 + Q22, Q11, Q12 - Q22, Q21 - Q11, Q22, Q11 + Q12, Q21 + Q22]
        ),
        dtype=bf16,
    )
    # layer-1 Strassen stationary operands from W1 quadrants blk[r, c]
    blk = W1.reshape(2, 1024, 2, 1024)
    S = np.stack(
        [
            blk[0, :, 0] + blk[1, :, 1],  # (P11+P22)^T
            blk[0, :, 1] + blk[1, :, 1],  # (P21+P22)^T
            blk[0, :, 0],                 # P11^T
            blk[1, :, 1],                 # P22^T
            blk[0, :, 0] + blk[1, :, 0],  # (P11+P12)^T
            blk[0, :, 1] - blk[0, :, 0],  # (P21-P11)^T
            blk[1, :, 0] - blk[1, :, 1],  # (P12-P22)^T
        ]
    )  # [7, K/2, M/2]
    w1r = np.ascontiguousarray(
        S.reshape(7, 8, 128, 8, 128).transpose(0, 3, 2, 1, 4).reshape(7, 8, 128, 1024),
        dtype=bf16,
    )
    # layer-2 Strassen moving operands from W2 (1024 x 512) blocks
    blk2 = W2.reshape(2, 1024, 2, 512)
    S2_ = np.stack(
        [
            blk2[0, :, 0] + blk2[1, :, 1],
            blk2[0, :, 0],
            blk2[0, :, 1] - blk2[1, :, 1],
            blk2[1, :, 0] - blk2[0, :, 0],
            blk2[1, :, 1],
            blk2[0, :, 0] + blk2[0, :, 1],
            blk2[1, :, 0] + blk2[1, :, 1],
        ]
    )  # [7, K/2, M3/2]
    w2r = np.ascontiguousarray(S2_.reshape(7, 8, 128, 512), dtype=bf16)
    return {
        "xt": xsr,
        "w0": w0r,
        "b0": b0c,
        "w1": w1r,
        "b1": b1c,
        "w2": w2r,
        "b2": b2c,
    }


def build_in_maps(inputs):
    import concurrent.futures

    import ml_dtypes

    bf16 = ml_dtypes.bfloat16
    b0c = np.ascontiguousarray(inputs["b0"].reshape(16, 128).T, dtype=np.float32)
    b1c = np.ascontiguousarray(inputs["b1"].reshape(16, 128).T, dtype=np.float32)
    b2c = np.ascontiguousarray(
        np.broadcast_to(inputs["b2"], (P, H3)), dtype=np.float32
    )
    with concurrent.futures.ThreadPoolExecutor(max_workers=T) as ex:
        in_maps = list(
            ex.map(lambda t: _task_in_map(inputs, t, bf16, b0c, b1c, b2c), range(T))
        )
    return in_maps


def kernel(**inputs):
    from concourse import bass_utils

    nc = _get_nc()
    in_maps = build_in_maps(inputs)
    res = bass_utils.run_bass_kernel_spmd(nc, in_maps, core_ids=list(range(T)))
    return np.stack([r["out"] for r in res.results], axis=0)


# revision 19
# speedup vs baseline: 1.1339x; 1.0060x over previous
"""Trainium2 Bass kernel for 3-layer per-task LoRA MLP.

Full-input contract: kernel(**inputs) takes the unsharded tensors and returns
the full [8, 1024, 1024] output. Internally the task axis (t=8) is sharded
across 8 NeuronCores (one task per core).

Strategy:
  - LoRA is folded on the host into per-task effective weights
    W_eff = k + (alpha/r) * d @ u  (standard LoRA weight merging), so the
    device kernel is a plain 3-layer MLP — no rank-8 matmuls on the PE.
  - weights and activations are bf16 on device (1 cycle/row on the PE, same
    as f32r, but half the DMA traffic and SBUF footprint); PSUM accumulation
    stays f32.
  - every layer runs one level of Strassen over 2x2 quadrant blocks: 7
    half-size products instead of 8 cuts PE row-streaming by 12.5% per
    layer. Weight-side operand combinations are free on the host; the
    activation-side combinations and C-quadrant accumulations are spread
    across the Pool/DVE/Act engines (Pool cannot read PSUM; DVE PSUM reads
    cost ~0.76us so each PSUM group gets at most one), overlapped so the
    PE stream stays gap-free. Measured pipeline error ~1e-2 relative
    (gate is 2e-2).
  - activations live as h^T [feat(part), batch(free)] with zero on-device
    transposes; the final layer uses h1^T slices as the *stationary*
    operand, producing natural-layout [batch, feat] output directly.
  - all host-side packs match the exact SBUF tile layout so every DMA is a
    contiguous >=1KB-per-partition stream, one DMA per consumed tile
    (consumers gate on whole-tile writes), spread over the three DMA
    channels (SP/Act HWDGE + Pool SWDGE) in consumption order.
  - a short PE warmup ramps the tensor-engine clock to 2.4 GHz while the
    first DMAs land; the final output quadrant drains in 128-wide chunks
    to shorten the post-PE tail.
"""

import sys

if "/opt/trn_rl_repo" not in sys.path:
    sys.path.insert(0, "/opt/trn_rl_repo")

import numpy as np

T, B, D = 8, 1024, 1024
H1, H2, H3 = 2048, 2048, 1024
SCALING = 2.0  # alpha/rank = 16/8
P = 128
NT = 512  # PSUM free-dim tile (fp32 one-bank limit)

_CACHE = {}


def _build():
    import concourse.mybir as mybir
    from concourse import bacc
    from concourse.tile import TileContext
    from concourse.bass import ts

    f32 = mybir.dt.float32
    bf16 = mybir.dt.bfloat16
    AF = mybir.ActivationFunctionType

    nc = bacc.Bacc(None, target_bir_lowering=False, name="lora_mlp")

    KT0 = D // P      # 8  k-tiles, layer 0
    KT1 = H1 // P     # 16 k-tiles, layer 1
    KT2 = H2 // P     # 16 k-tiles, layer 2
    MT0 = H1 // P     # 16 m-tiles, layer 0
    MT1 = H2 // P     # 16 m-tiles, layer 1
    BT = B // P       # 8  batch 128-tiles
    NB = B // NT      # 2  batch 512-halves (free dim, layers 0/1)
    N2 = H3 // NT     # 2  feature 512-halves (free dim, layer 2)

    # layer-0 inputs arrive Strassen-ready: xs = the 7 moving-operand
    # combos of x^T quadrants [K/2, B/2]; w0 = the 7 stationary operands
    xs = nc.dram_tensor("xt", (7, D // 2, B // 2), bf16, kind="ExternalInput")
    w0 = nc.dram_tensor("w0", (7, 8, P, 4 * P), bf16, kind="ExternalInput")
    b0 = nc.dram_tensor("b0", (P, MT0), f32, kind="ExternalInput")
    # layer-1 weights arrive as the 7 Strassen stationary operands
    # S_i [K/2, M/2], host-combined and packed [i, j(m-tile), p, kk*128+c]
    w1 = nc.dram_tensor("w1", (7, 8, P, 8 * P), bf16, kind="ExternalInput")
    b1 = nc.dram_tensor("b1", (P, MT1), f32, kind="ExternalInput")
    w2 = nc.dram_tensor("w2", (7, 8, P, H3 // 2), bf16, kind="ExternalInput")
    b2 = nc.dram_tensor("b2", (P, H3), f32, kind="ExternalInput")
    out = nc.dram_tensor("out", (B, H3), f32, kind="ExternalOutput")

    with TileContext(nc) as tc:
        with (
            tc.tile_pool(name="main", bufs=1) as pool,
            tc.tile_pool(name="psum", bufs=1, space="PSUM") as pp,
        ):
            # PE warmup: the tensor engine clock ramps with sustained use
            # (0.65 -> 1.2 -> 2.4 GHz over ~3us). Run throwaway matmuls on a
            # memset tile while the first input DMAs land, so the real
            # matmuls start at full clock.
            wu = pool.tile([P, 2 * P], bf16, tag="wu", bufs=1)
            nc.vector.memset(wu, 0.125)
            wps = pp.tile([P, 2 * P], f32, tag="wps", bufs=1)
            for _ in range(10):
                nc.tensor.matmul(wps, wu[:, :P], wu, start=True, stop=True)

            b0_sb = pool.tile([P, MT0], f32, tag="b0", bufs=1)
            b1_sb = pool.tile([P, MT1], f32, tag="b1", bufs=1)
            b2_sb = pool.tile([P, H3], f32, tag="b2", bufs=1)

            # =================== layer 0 (one-level Strassen) ===================
            # C = W0^T @ x over (512 x 1024) quadrant blocks; both operand
            # combos come pre-built on the host, so the device only runs the
            # 7 products and the C-quadrant accumulation. The product loop is
            # OUTER so each product's 4 moving tiles (TX ring, prefetched one
            # product ahead on Pool/Act queues) serve 8 consecutive groups —
            # the DMA stream never races the PE. First-product copies into
            # each C quadrant run on the Act engine, accumulating adds on
            # DVE, bias+relu on Pool (tensor_scalar add+max).
            h0T = [
                pool.tile([P, B], bf16, tag="H0", bufs=MT0, name=f"h0T{m}")
                for m in range(MT0)
            ]
            lo, hi = ts(0, NT), ts(1, NT)
            PORD0 = (2, 1, 3, 4, 5, 6, 7)
            alu = mybir.AluOpType

            def prelu(dst, src, bcol):
                nc.gpsimd.tensor_scalar(dst, src, bcol, 0.0, alu.add, alu.max)

            TX = {}

            def tx_load(i):
                TX[i] = [
                    pool.tile([P, NT], bf16, tag="TX", bufs=8, name=f"TX{i}_{kk}")
                    for kk in range(4)
                ]
                for kk in range(4):
                    eng = nc.gpsimd if kk < 2 else nc.scalar
                    eng.dma_start(out=TX[i][kk], in_=xs[i - 1, ts(kk, P), :])

            C = {q: [None] * 8 for q in (11, 12, 21, 22)}
            KH = KT1 // 2  # 8 k-tiles per K-half of layer 1
            Tc = {
                i: [
                    pool.tile([P, NT], bf16, tag="T", bufs=40, name=f"T{i}_{kk}")
                    for kk in range(KH)
                ]
                for i in (1, 3, 4, 6, 7)
            }
            with tc.high_priority():
                tx_load(PORD0[0])
                tx_load(PORD0[1])
                nc.gpsimd.dma_start(out=b0_sb, in_=b0[:, :])
                nc.gpsimd.dma_start(out=b1_sb, in_=b1[:, :])
            for idx, i in enumerate(PORD0):
                if idx + 2 < len(PORD0):
                    tx_load(PORD0[idx + 2])
                if i == 7:
                    # layer-1's T7 combos (Q21+Q22) only need L0's C21/C22
                    # quadrants — emit them ahead of i7's Pool relus, split
                    # across Pool and DVE so they emerge 2x faster than
                    # L1's second product consumes them
                    for kk in range(KH):
                        eng = nc.gpsimd if kk % 2 == 0 else nc.vector
                        eng.tensor_add(
                            Tc[7][kk], h0T[8 + kk][:, lo], h0T[8 + kk][:, hi]
                        )
                for j in range(8):
                    st = pool.tile([P, 4 * P], bf16, tag="W0s", bufs=6, name=f"s0_{i}_{j}")
                    nc.sync.dma_start(out=st, in_=w0[i - 1, j])
                    ps = pp.tile([P, NT], f32, tag="pm", bufs=6, name=f"q{i}_{j}")
                    for kk in range(4):
                        nc.tensor.matmul(
                            ps,
                            st[:, ts(kk, P)],
                            TX[i][kk],
                            start=(kk == 0),
                            stop=(kk == 3),
                        )
                    # Pool cannot read PSUM, and one DVE PSUM-read op costs
                    # ~0.76us vs the 0.85us group cadence — so each group
                    # carries exactly one DVE op; ACT copies shared products
                    # (M4, M5) to SBUF where Pool runs the second add.
                    if i == 2:
                        C[21][j] = pool.tile([P, NT], bf16, tag="X", bufs=32, name=f"d21_{j}")
                        C[22][j] = pool.tile([P, NT], bf16, tag="X", bufs=32, name=f"d22_{j}")
                        nc.scalar.copy(C[21][j], ps)
                        nc.vector.tensor_scalar_mul(C[22][j], ps, -1.0)
                    elif i == 1:
                        C[11][j] = pool.tile([P, NT], bf16, tag="X", bufs=32, name=f"d11_{j}")
                        nc.scalar.copy(C[11][j], ps)
                        nc.vector.tensor_add(C[22][j], C[22][j], ps)
                    elif i == 3:
                        C[12][j] = pool.tile([P, NT], bf16, tag="X", bufs=32, name=f"d12_{j}")
                        nc.scalar.copy(C[12][j], ps)
                        nc.vector.tensor_add(C[22][j], C[22][j], ps)
                    elif i == 4:
                        e4 = pool.tile([P, NT], bf16, tag="E", bufs=3, name=f"e4_{j}")
                        nc.scalar.copy(e4, ps)
                        nc.vector.tensor_add(C[11][j], C[11][j], ps)
                        nc.gpsimd.tensor_add(C[21][j], C[21][j], e4)
                        prelu(h0T[8 + j][:, lo], C[21][j], b0_sb[:, ts(8 + j, 1)])
                    elif i == 5:
                        e5 = pool.tile([P, NT], bf16, tag="E", bufs=3, name=f"e5_{j}")
                        nc.scalar.copy(e5, ps)
                        nc.vector.tensor_sub(C[11][j], C[11][j], ps)
                        nc.gpsimd.tensor_add(C[12][j], C[12][j], e5)
                        prelu(h0T[j][:, hi], C[12][j], b0_sb[:, ts(j, 1)])
                    elif i == 6:
                        nc.vector.tensor_add(C[22][j], C[22][j], ps)
                        prelu(h0T[8 + j][:, hi], C[22][j], b0_sb[:, ts(8 + j, 1)])
                    elif i == 7:
                        nc.vector.tensor_add(C[11][j], C[11][j], ps)
                        prelu(h0T[j][:, lo], C[11][j], b0_sb[:, ts(j, 1)])

            # =================== layer 1 (one-level Strassen) ===================
            # C = W1^T @ h0 over 1024x1024 quadrants: 7 half-size products
            # instead of 8 (PE rows 229376 vs 262144). Weight-side combos
            # S1..S7 are free on the host; activation-side combos T run on
            # the otherwise-idle Pool engine; products accumulate into the
            # four C quadrants via DVE reads of each product's PSUM bank.
            #   quadrant views of h0: Q11/Q12 = h0T[kk] cols lo/hi,
            #                         Q21/Q22 = h0T[8+kk] cols lo/hi
            # Product order is chosen by operand availability: L0 finishes
            # quadrants in the order C21(i4), C12(i5), C22(i6), C11(i7), so
            # L1 opens with M5 (pure Q22) and M7 (Q21+Q22) whose inputs are
            # ready a product-width before L0's PE stream even ends (T7's
            # combos were emitted inside the L0 loop for the same reason);
            # remaining combos are product-major so the Pool engine always
            # has a full product window of slack.
            for i, fn, sel in (
                (1, nc.gpsimd.tensor_add, lambda a, b_: (a[:, lo], b_[:, hi])),  # Q11+Q22
                (3, nc.gpsimd.tensor_sub, lambda a, b_: (a[:, hi], b_[:, hi])),  # Q12-Q22
                (4, nc.gpsimd.tensor_sub, lambda a, b_: (b_[:, lo], a[:, lo])),  # Q21-Q11
                (6, nc.gpsimd.tensor_add, lambda a, b_: (a[:, lo], a[:, hi])),   # Q11+Q12
            ):
                for kk in range(KH):
                    x0, x1 = sel(h0T[kk], h0T[8 + kk])
                    fn(Tc[i][kk], x0, x1)

            def mov1(i, kk):
                if i == 2:
                    return h0T[kk][:, lo]       # Q11
                if i == 5:
                    return h0T[8 + kk][:, hi]   # Q22
                return Tc[i][kk]

            h1T = [
                pool.tile([P, B], bf16, tag="H1", bufs=MT1, name=f"h1T{m}")
                for m in range(MT1)
            ]
            # product order by L0-output availability (see combo comment):
            #   C11 = M1 + M4 - M5 + M7   (built as -M5, +M7, +M1, +M4)
            #   C12 = M3 + M5             (copy M5, +M3)
            #   C21 = M2 + M4             (copy M2, +M4)
            #   C22 = M1 - M2 + M3 + M6   (built as -M2, +M1, +M3, +M6)
            PORDER = (5, 7, 2, 1, 3, 4, 6)
            for j in range(8):
                for i in PORDER:
                    st = pool.tile([P, KH * P], bf16, tag="W1", bufs=6, name=f"s{i}_{j}")
                    nc.sync.dma_start(out=st, in_=w1[i - 1, j])
                    ps = pp.tile([P, NT], f32, tag="pm", bufs=6, name=f"p{i}_{j}")
                    for kk in range(KH):
                        nc.tensor.matmul(
                            ps,
                            st[:, ts(kk, P)],
                            mov1(i, kk),
                            start=(kk == 0),
                            stop=(kk == KH - 1),
                        )
                    if i == 5:
                        c11 = pool.tile([P, NT], bf16, tag="X", bufs=32, name=f"c11_{j}")
                        c12 = pool.tile([P, NT], bf16, tag="X", bufs=32, name=f"c12_{j}")
                        nc.vector.tensor_scalar_mul(c11, ps, -1.0)
                        nc.scalar.copy(c12, ps)
                    elif i == 7:
                        nc.vector.tensor_add(c11, c11, ps)
                    elif i == 2:
                        c21 = pool.tile([P, NT], bf16, tag="X", bufs=32, name=f"c21_{j}")
                        c22 = pool.tile([P, NT], bf16, tag="X", bufs=32, name=f"c22_{j}")
                        nc.scalar.copy(c21, ps)
                        nc.vector.tensor_scalar_mul(c22, ps, -1.0)
                    elif i == 1:
                        nc.vector.tensor_add(c11, c11, ps)
                        nc.vector.tensor_add(c22, c22, ps)
                    elif i == 3:
                        nc.vector.tensor_add(c12, c12, ps)
                        nc.vector.tensor_add(c22, c22, ps)
                        nc.scalar.activation(
                            h1T[j][:, hi], c12, AF.Relu, bias=b1_sb[:, ts(j, 1)]
                        )
                    elif i == 4:
                        nc.vector.tensor_add(c11, c11, ps)
                        nc.vector.tensor_add(c21, c21, ps)
                        nc.scalar.activation(
                            h1T[8 + j][:, lo], c21, AF.Relu, bias=b1_sb[:, ts(8 + j, 1)]
                        )
                        nc.scalar.activation(
                            h1T[j][:, lo], c11, AF.Relu, bias=b1_sb[:, ts(j, 1)]
                        )
                    elif i == 6:
                        nc.vector.tensor_add(c22, c22, ps)
                        nc.scalar.activation(
                            h1T[8 + j][:, hi], c22, AF.Relu, bias=b1_sb[:, ts(8 + j, 1)]
                        )

            # =================== layer 2 (one-level Strassen) ===================
            # out = h1 @ W2 over quadrants: the W2-side combos arrive from the
            # host as the 7 moving operands (TW ring, product-outer like L0);
            # the h1-side stationary combos SC run on Pool, emitted in the
            # order products consume them. Finished C quadrants get their
            # (free-dim) bias added on Pool and store immediately.
            nc.gpsimd.dma_start(out=b2_sb, in_=b2[:, :])
            TW = {}

            def tw_load(i):
                TW[i] = [
                    pool.tile([P, NT], bf16, tag="TW", bufs=16, name=f"TW{i}_{kk}")
                    for kk in range(KH)
                ]
                for kk in range(KH):
                    nc.sync.dma_start(out=TW[i][kk], in_=w2[i - 1, kk])

            SC = {
                i: [
                    pool.tile([P, NT], bf16, tag="T", bufs=40, name=f"SC{i}_{kk}")
                    for kk in range(KH)
                ]
                for i in (1, 2, 5, 6, 7)
            }
            for i, fn, sel in (
                (5, nc.gpsimd.tensor_add, lambda a, b_: (a[:, lo], b_[:, lo])),  # P11+P12
                (6, nc.gpsimd.tensor_sub, lambda a, b_: (a[:, hi], a[:, lo])),   # P21-P11
                (2, nc.gpsimd.tensor_add, lambda a, b_: (a[:, hi], b_[:, hi])),  # P21+P22
                (1, nc.gpsimd.tensor_add, lambda a, b_: (a[:, lo], b_[:, hi])),  # P11+P22
                (7, nc.gpsimd.tensor_sub, lambda a, b_: (b_[:, lo], b_[:, hi])),  # P12-P22
            ):
                for kk in range(KH):
                    x0, x1 = sel(h1T[kk], h1T[8 + kk])
                    fn(SC[i][kk], x0, x1)

            def stat2(i, j, kk):
                if i == 3:
                    return h1T[kk][:, ts(j, P)]                    # P11
                if i == 4:
                    return h1T[8 + kk][:, NT + j * P : NT + (j + 1) * P]  # P22
                return SC[i][kk][:, ts(j, P)]

            # quadrant -> (out row block base, out col half)
            QOUT = {11: (0, 0), 12: (0, 1), 21: (1, 0), 22: (1, 1)}

            def store_quad(q, j, ctile, chunks=1):
                rbase, chalf = QOUT[q]
                cw2 = NT // chunks
                for c in range(chunks):
                    o5 = pool.tile([P, cw2], f32, tag="O5", bufs=8, name=f"o{q}_{j}_{c}")
                    nc.gpsimd.tensor_add(
                        o5, ctile[:, ts(c, cw2)],
                        b2_sb[:, chalf * NT + c * cw2 : chalf * NT + (c + 1) * cw2],
                    )
                    nc.scalar.dma_start(
                        out=out[
                            rbase * NT + j * P : rbase * NT + (j + 1) * P,
                            chalf * NT + c * cw2 : chalf * NT + (c + 1) * cw2,
                        ],
                        in_=o5,
                    )

            # C11 = M1+M4-M5+M7, C12 = M3+M5, C21 = M2+M4, C22 = M1-M2+M3+M6
            PORD2 = (5, 3, 2, 6, 1, 4, 7)
            D2 = {q: [None] * 4 for q in (11, 12, 21, 22)}
            tw_load(PORD2[0])
            tw_load(PORD2[1])
            for idx, i in enumerate(PORD2):
                if idx + 2 < len(PORD2):
                    tw_load(PORD2[idx + 2])
                for j in range(4):
                    if i == 7 and j == 3:
                        # very last group: run it as four 128-wide PSUM
                        # sub-groups so each chunk's drain (DVE add -> Pool
                        # bias -> store, alternating Act/sync queues)
                        # pipelines against the PE's remaining sub-groups
                        cw2 = NT // 4
                        for c in range(4):
                            sl = ts(c, cw2)
                            psc = pp.tile([P, cw2], f32, tag="pm", bufs=6, name=f"rf_{c}")
                            for kk in range(KH):
                                nc.tensor.matmul(
                                    psc,
                                    stat2(i, j, kk),
                                    TW[i][kk][:, sl],
                                    start=(kk == 0),
                                    stop=(kk == KH - 1),
                                )
                            nc.vector.tensor_add(
                                D2[11][j][:, sl], D2[11][j][:, sl], psc
                            )
                            o5 = pool.tile([P, cw2], f32, tag="O5", bufs=8, name=f"of_{c}")
                            nc.gpsimd.tensor_add(
                                o5, D2[11][j][:, sl], b2_sb[:, c * cw2 : (c + 1) * cw2]
                            )
                            eng = nc.scalar if c % 2 == 0 else nc.sync
                            eng.dma_start(
                                out=out[j * P : (j + 1) * P, c * cw2 : (c + 1) * cw2],
                                in_=o5,
                            )
                        continue
                    ps = pp.tile([P, NT], f32, tag="pm", bufs=6, name=f"r{i}_{j}")
                    for kk in range(KH):
                        nc.tensor.matmul(
                            ps,
                            stat2(i, j, kk),
                            TW[i][kk],
                            start=(kk == 0),
                            stop=(kk == KH - 1),
                        )
                    if i == 5:
                        D2[11][j] = pool.tile([P, NT], bf16, tag="X", bufs=32, name=f"g11_{j}")
                        D2[12][j] = pool.tile([P, NT], bf16, tag="X", bufs=32, name=f"g12_{j}")
                        nc.vector.tensor_scalar_mul(D2[11][j], ps, -1.0)
                        nc.scalar.copy(D2[12][j], ps)
                    elif i == 3:
                        D2[22][j] = pool.tile([P, NT], bf16, tag="X", bufs=32, name=f"g22_{j}")
                        nc.vector.tensor_add(D2[12][j], D2[12][j], ps)
                        nc.scalar.copy(D2[22][j], ps)
                        store_quad(12, j, D2[12][j])
                    elif i == 2:
                        D2[21][j] = pool.tile([P, NT], bf16, tag="X", bufs=32, name=f"g21_{j}")
                        nc.vector.tensor_sub(D2[22][j], D2[22][j], ps)
                        nc.scalar.copy(D2[21][j], ps)
                    elif i == 6:
                        nc.vector.tensor_add(D2[22][j], D2[22][j], ps)
                    elif i == 1:
                        nc.vector.tensor_add(D2[22][j], D2[22][j], ps)
                        nc.vector.tensor_add(D2[11][j], D2[11][j], ps)
                        store_quad(22, j, D2[22][j])
                    elif i == 4:
                        nc.vector.tensor_add(D2[21][j], D2[21][j], ps)
                        nc.vector.tensor_add(D2[11][j], D2[11][j], ps)
                        store_quad(21, j, D2[21][j])
                    elif i == 7:
                        if j < 3:
                            nc.vector.tensor_add(D2[11][j], D2[11][j], ps)
                            store_quad(11, j, D2[11][j])
                        else:
                            # final quadrant: chunk the whole accumulate ->
                            # bias -> store chain so the post-PE tail is short
                            cw2 = NT // 4
                            for c in range(4):
                                sl = ts(c, cw2)
                                nc.vector.tensor_add(
                                    D2[11][j][:, sl], D2[11][j][:, sl], ps[:, sl]
                                )
                                o5 = pool.tile([P, cw2], f32, tag="O5", bufs=8, name=f"of_{c}")
                                nc.gpsimd.tensor_add(
                                    o5, D2[11][j][:, sl], b2_sb[:, c * cw2 : (c + 1) * cw2]
                                )
                                nc.scalar.dma_start(
                                    out=out[j * P : (j + 1) * P, c * cw2 : (c + 1) * cw2],
                                    in_=o5,
                                )

    if not nc.is_finalized():
        nc.finalize()
    return nc


def _get_nc():
    if "nc" not in _CACHE:
        _CACHE["nc"] = _build()
    return _CACHE["nc"]


def _task_in_map(inputs, t, bf16, b0c, b1c, b2c):
    W0 = inputs["k0"] + SCALING * (inputs["d0"][:, :, t] @ inputs["u0"][:, :, t])
    W1 = inputs["k1"] + SCALING * (inputs["d1"][:, :, t] @ inputs["u1"][:, :, t])
    W2 = inputs["k2"] + SCALING * (inputs["d2"][:, :, t] @ inputs["u2"][:, :, t])
    # layer-0 Strassen: both operand sets host-combined.
    # stationary S_i from W0 (512 x 1024) blocks
    blk0 = W0.reshape(2, 512, 2, 1024)
    S0 = np.stack(
        [
            blk0[0, :, 0] + blk0[1, :, 1],
            blk0[0, :, 1] + blk0[1, :, 1],
            blk0[0, :, 0],
            blk0[1, :, 1],
            blk0[0, :, 0] + blk0[1, :, 0],
            blk0[0, :, 1] - blk0[0, :, 0],
            blk0[1, :, 0] - blk0[1, :, 1],
        ]
    )  # [7, K/2, M/2]
    w0r = np.ascontiguousarray(
        S0.reshape(7, 4, 128, 8, 128).transpose(0, 3, 2, 1, 4).reshape(7, 8, 128, 512),
        dtype=bf16,
    )
    # moving combos T_i from x^T quadrants
    xT = inputs["x"][t].T
    Q11, Q12 = xT[:512, :512], xT[:512, 512:]
    Q21, Q22 = xT[512:, :512], xT[512:, 512:]
    xsr = np.ascontiguousarray(
        np.stack(
            [Q11 + Q22, Q11, Q12 - Q22, Q21 - Q11, Q22, Q11 + Q12, Q21 + Q22]
        ),
        dtype=bf16,
    )
    # layer-1 Strassen stationary operands from W1 quadrants blk[r, c]
    blk = W1.reshape(2, 1024, 2, 1024)
    S = np.stack(
        [
            blk[0, :, 0] + blk[1, :, 1],  # (P11+P22)^T
            blk[0, :, 1] + blk[1, :, 1],  # (P21+P22)^T
            blk[0, :, 0],                 # P11^T
            blk[1, :, 1],                 # P22^T
            blk[0, :, 0] + blk[1, :, 0],  # (P11+P12)^T
            blk[0, :, 1] - blk[0, :, 0],  # (P21-P11)^T
            blk[1, :, 0] - blk[1, :, 1],  # (P12-P22)^T
        ]
    )  # [7, K/2, M/2]
    w1r = np.ascontiguousarray(
        S.reshape(7, 8, 128, 8, 128).transpose(0, 3, 2, 1, 4).reshape(7, 8, 128, 1024),
        dtype=bf16,
    )
    # layer-2 Strassen moving operands from W2 (1024 x 512) blocks
    blk2 = W2.reshape(2, 1024, 2, 512)
    S2_ = np.stack(
        [
            blk2[0, :, 0] + blk2[1, :, 1],
            blk2[0, :, 0],
            blk2[0, :, 1] - blk2[1, :, 1],
            blk2[1, :, 0] - blk2[0, :, 0],
            blk2[1, :, 1],
            blk2[0, :, 0] + blk2[0, :, 1],
            blk2[1, :, 0] + blk2[1, :, 1],
        ]
    )  # [7, K/2, M3/2]
    w2r = np.ascontiguousarray(S2_.reshape(7, 8, 128, 512), dtype=bf16)
    return {
        "xt": xsr,
        "w0": w0r,
        "b0": b0c,
        "w1": w1r,
        "b1": b1c,
        "w2": w2r,
        "b2": b2c,
    }


def build_in_maps(inputs):
    import concurrent.futures

    import ml_dtypes

    bf16 = ml_dtypes.bfloat16
    b0c = np.ascontiguousarray(inputs["b0"].reshape(16, 128).T, dtype=np.float32)
    b1c = np.ascontiguousarray(inputs["b1"].reshape(16, 128).T, dtype=np.float32)
    b2c = np.ascontiguousarray(
        np.broadcast_to(inputs["b2"], (P, H3)), dtype=np.float32
    )
    with concurrent.futures.ThreadPoolExecutor(max_workers=T) as ex:
        in_maps = list(
            ex.map(lambda t: _task_in_map(inputs, t, bf16, b0c, b1c, b2c), range(T))
        )
    return in_maps


def kernel(**inputs):
    from concourse import bass_utils

    nc = _get_nc()
    in_maps = build_in_maps(inputs)
    res = bass_utils.run_bass_kernel_spmd(nc, in_maps, core_ids=list(range(T)))
    return np.stack([r["out"] for r in res.results], axis=0)


# revision 20
# speedup vs baseline: 1.1351x; 1.0011x over previous
"""Trainium2 Bass kernel for 3-layer per-task LoRA MLP.

Full-input contract: kernel(**inputs) takes the unsharded tensors and returns
the full [8, 1024, 1024] output. Internally the task axis (t=8) is sharded
across 8 NeuronCores (one task per core).

Strategy:
  - LoRA is folded on the host into per-task effective weights
    W_eff = k + (alpha/r) * d @ u  (standard LoRA weight merging), so the
    device kernel is a plain 3-layer MLP — no rank-8 matmuls on the PE.
  - weights and activations are bf16 on device (1 cycle/row on the PE, same
    as f32r, but half the DMA traffic and SBUF footprint); PSUM accumulation
    stays f32.
  - every layer runs one level of Strassen over 2x2 quadrant blocks: 7
    half-size products instead of 8 cuts PE row-streaming by 12.5% per
    layer. Weight-side operand combinations are free on the host; the
    activation-side combinations and C-quadrant accumulations are spread
    across the Pool/DVE/Act engines (Pool cannot read PSUM; DVE PSUM reads
    cost ~0.76us so each PSUM group gets at most one), overlapped so the
    PE stream stays gap-free. Measured pipeline error ~1e-2 relative
    (gate is 2e-2).
  - activations live as h^T [feat(part), batch(free)] with zero on-device
    transposes; the final layer uses h1^T slices as the *stationary*
    operand, producing natural-layout [batch, feat] output directly.
  - all host-side packs match the exact SBUF tile layout so every DMA is a
    contiguous >=1KB-per-partition stream, one DMA per consumed tile
    (consumers gate on whole-tile writes), spread over the three DMA
    channels (SP/Act HWDGE + Pool SWDGE) in consumption order.
  - a short PE warmup ramps the tensor-engine clock to 2.4 GHz while the
    first DMAs land; the final output quadrant drains in 128-wide chunks
    to shorten the post-PE tail.
"""

import sys

if "/opt/trn_rl_repo" not in sys.path:
    sys.path.insert(0, "/opt/trn_rl_repo")

import numpy as np

T, B, D = 8, 1024, 1024
H1, H2, H3 = 2048, 2048, 1024
SCALING = 2.0  # alpha/rank = 16/8
P = 128
NT = 512  # PSUM free-dim tile (fp32 one-bank limit)

_CACHE = {}


def _build():
    import concourse.mybir as mybir
    from concourse import bacc
    from concourse.tile import TileContext
    from concourse.bass import ts

    f32 = mybir.dt.float32
    bf16 = mybir.dt.bfloat16
    AF = mybir.ActivationFunctionType

    nc = bacc.Bacc(None, target_bir_lowering=False, name="lora_mlp")

    KT0 = D // P      # 8  k-tiles, layer 0
    KT1 = H1 // P     # 16 k-tiles, layer 1
    KT2 = H2 // P     # 16 k-tiles, layer 2
    MT0 = H1 // P     # 16 m-tiles, layer 0
    MT1 = H2 // P     # 16 m-tiles, layer 1
    BT = B // P       # 8  batch 128-tiles
    NB = B // NT      # 2  batch 512-halves (free dim, layers 0/1)
    N2 = H3 // NT     # 2  feature 512-halves (free dim, layer 2)

    # layer-0 inputs arrive Strassen-ready: xs = the 7 moving-operand
    # combos of x^T quadrants [K/2, B/2]; w0 = the 7 stationary operands
    xs = nc.dram_tensor("xt", (7, D // 2, B // 2), bf16, kind="ExternalInput")
    w0 = nc.dram_tensor("w0", (7, 8, P, 4 * P), bf16, kind="ExternalInput")
    b0 = nc.dram_tensor("b0", (P, MT0), f32, kind="ExternalInput")
    # layer-1 weights arrive as the 7 Strassen stationary operands
    # S_i [K/2, M/2], host-combined and packed [i, j(m-tile), p, kk*128+c]
    w1 = nc.dram_tensor("w1", (7, 8, P, 8 * P), bf16, kind="ExternalInput")
    b1 = nc.dram_tensor("b1", (P, MT1), f32, kind="ExternalInput")
    w2 = nc.dram_tensor("w2", (7, 8, P, H3 // 2), bf16, kind="ExternalInput")
    b2 = nc.dram_tensor("b2", (P, H3), f32, kind="ExternalInput")
    out = nc.dram_tensor("out", (B, H3), f32, kind="ExternalOutput")

    with TileContext(nc) as tc:
        with (
            tc.tile_pool(name="main", bufs=1) as pool,
            tc.tile_pool(name="psum", bufs=1, space="PSUM") as pp,
        ):
            # PE warmup: the tensor engine clock ramps with sustained use
            # (0.65 -> 1.2 -> 2.4 GHz over ~3us). Run throwaway matmuls on a
            # memset tile while the first input DMAs land, so the real
            # matmuls start at full clock.
            wu = pool.tile([P, 2 * P], bf16, tag="wu", bufs=1)
            nc.vector.memset(wu, 0.125)
            wps = pp.tile([P, 2 * P], f32, tag="wps", bufs=1)
            for _ in range(8):
                nc.tensor.matmul(wps, wu[:, :P], wu, start=True, stop=True)

            b0_sb = pool.tile([P, MT0], f32, tag="b0", bufs=1)
            b1_sb = pool.tile([P, MT1], f32, tag="b1", bufs=1)
            b2_sb = pool.tile([P, H3], f32, tag="b2", bufs=1)

            # =================== layer 0 (one-level Strassen) ===================
            # C = W0^T @ x over (512 x 1024) quadrant blocks; both operand
            # combos come pre-built on the host, so the device only runs the
            # 7 products and the C-quadrant accumulation. The product loop is
            # OUTER so each product's 4 moving tiles (TX ring, prefetched one
            # product ahead on Pool/Act queues) serve 8 consecutive groups —
            # the DMA stream never races the PE. First-product copies into
            # each C quadrant run on the Act engine, accumulating adds on
            # DVE, bias+relu on Pool (tensor_scalar add+max).
            h0T = [
                pool.tile([P, B], bf16, tag="H0", bufs=MT0, name=f"h0T{m}")
                for m in range(MT0)
            ]
            lo, hi = ts(0, NT), ts(1, NT)
            PORD0 = (2, 1, 3, 4, 5, 6, 7)
            alu = mybir.AluOpType

            def prelu(dst, src, bcol):
                nc.gpsimd.tensor_scalar(dst, src, bcol, 0.0, alu.add, alu.max)

            TX = {}

            def tx_load(i):
                TX[i] = [
                    pool.tile([P, NT], bf16, tag="TX", bufs=8, name=f"TX{i}_{kk}")
                    for kk in range(4)
                ]
                for kk in range(4):
                    eng = nc.gpsimd if kk < 2 else nc.scalar
                    eng.dma_start(out=TX[i][kk], in_=xs[i - 1, ts(kk, P), :])

            C = {q: [None] * 8 for q in (11, 12, 21, 22)}
            KH = KT1 // 2  # 8 k-tiles per K-half of layer 1
            Tc = {
                i: [
                    pool.tile([P, NT], bf16, tag="T", bufs=40, name=f"T{i}_{kk}")
                    for kk in range(KH)
                ]
                for i in (1, 3, 4, 6, 7)
            }
            with tc.high_priority():
                tx_load(PORD0[0])
                tx_load(PORD0[1])
                nc.gpsimd.dma_start(out=b0_sb, in_=b0[:, :])
                nc.gpsimd.dma_start(out=b1_sb, in_=b1[:, :])
            for idx, i in enumerate(PORD0):
                if idx + 2 < len(PORD0):
                    tx_load(PORD0[idx + 2])
                if i == 7:
                    # layer-1's T7 combos (Q21+Q22) only need L0's C21/C22
                    # quadrants — emit them ahead of i7's Pool relus, split
                    # across Pool and DVE so they emerge 2x faster than
                    # L1's second product consumes them
                    for kk in range(KH):
                        eng = nc.gpsimd if kk % 2 == 0 else nc.vector
                        eng.tensor_add(
                            Tc[7][kk], h0T[8 + kk][:, lo], h0T[8 + kk][:, hi]
                        )
                for j in range(8):
                    st = pool.tile([P, 4 * P], bf16, tag="W0s", bufs=6, name=f"s0_{i}_{j}")
                    nc.sync.dma_start(out=st, in_=w0[i - 1, j])
                    ps = pp.tile([P, NT], f32, tag="pm", bufs=6, name=f"q{i}_{j}")
                    for kk in range(4):
                        nc.tensor.matmul(
                            ps,
                            st[:, ts(kk, P)],
                            TX[i][kk],
                            start=(kk == 0),
                            stop=(kk == 3),
                        )
                    # Pool cannot read PSUM, and one DVE PSUM-read op costs
                    # ~0.76us vs the 0.85us group cadence — so each group
                    # carries exactly one DVE op; ACT copies shared products
                    # (M4, M5) to SBUF where Pool runs the second add.
                    if i == 2:
                        C[21][j] = pool.tile([P, NT], bf16, tag="X", bufs=32, name=f"d21_{j}")
                        C[22][j] = pool.tile([P, NT], bf16, tag="X", bufs=32, name=f"d22_{j}")
                        nc.scalar.copy(C[21][j], ps)
                        nc.vector.tensor_scalar_mul(C[22][j], ps, -1.0)
                    elif i == 1:
                        C[11][j] = pool.tile([P, NT], bf16, tag="X", bufs=32, name=f"d11_{j}")
                        nc.scalar.copy(C[11][j], ps)
                        nc.vector.tensor_add(C[22][j], C[22][j], ps)
                    elif i == 3:
                        C[12][j] = pool.tile([P, NT], bf16, tag="X", bufs=32, name=f"d12_{j}")
                        nc.scalar.copy(C[12][j], ps)
                        nc.vector.tensor_add(C[22][j], C[22][j], ps)
                    elif i == 4:
                        e4 = pool.tile([P, NT], bf16, tag="E", bufs=3, name=f"e4_{j}")
                        nc.scalar.copy(e4, ps)
                        nc.vector.tensor_add(C[11][j], C[11][j], ps)
                        nc.gpsimd.tensor_add(C[21][j], C[21][j], e4)
                        prelu(h0T[8 + j][:, lo], C[21][j], b0_sb[:, ts(8 + j, 1)])
                    elif i == 5:
                        e5 = pool.tile([P, NT], bf16, tag="E", bufs=3, name=f"e5_{j}")
                        nc.scalar.copy(e5, ps)
                        nc.vector.tensor_sub(C[11][j], C[11][j], ps)
                        nc.gpsimd.tensor_add(C[12][j], C[12][j], e5)
                        prelu(h0T[j][:, hi], C[12][j], b0_sb[:, ts(j, 1)])
                    elif i == 6:
                        nc.vector.tensor_add(C[22][j], C[22][j], ps)
                        prelu(h0T[8 + j][:, hi], C[22][j], b0_sb[:, ts(8 + j, 1)])
                    elif i == 7:
                        nc.vector.tensor_add(C[11][j], C[11][j], ps)
                        prelu(h0T[j][:, lo], C[11][j], b0_sb[:, ts(j, 1)])

            # =================== layer 1 (one-level Strassen) ===================
            # C = W1^T @ h0 over 1024x1024 quadrants: 7 half-size products
            # instead of 8 (PE rows 229376 vs 262144). Weight-side combos
            # S1..S7 are free on the host; activation-side combos T run on
            # the otherwise-idle Pool engine; products accumulate into the
            # four C quadrants via DVE reads of each product's PSUM bank.
            #   quadrant views of h0: Q11/Q12 = h0T[kk] cols lo/hi,
            #                         Q21/Q22 = h0T[8+kk] cols lo/hi
            # Product order is chosen by operand availability: L0 finishes
            # quadrants in the order C21(i4), C12(i5), C22(i6), C11(i7), so
            # L1 opens with M5 (pure Q22) and M7 (Q21+Q22) whose inputs are
            # ready a product-width before L0's PE stream even ends (T7's
            # combos were emitted inside the L0 loop for the same reason);
            # remaining combos are product-major so the Pool engine always
            # has a full product window of slack.
            for i, fn, sel in (
                (1, nc.gpsimd.tensor_add, lambda a, b_: (a[:, lo], b_[:, hi])),  # Q11+Q22
                (3, nc.gpsimd.tensor_sub, lambda a, b_: (a[:, hi], b_[:, hi])),  # Q12-Q22
                (4, nc.gpsimd.tensor_sub, lambda a, b_: (b_[:, lo], a[:, lo])),  # Q21-Q11
                (6, nc.gpsimd.tensor_add, lambda a, b_: (a[:, lo], a[:, hi])),   # Q11+Q12
            ):
                for kk in range(KH):
                    x0, x1 = sel(h0T[kk], h0T[8 + kk])
                    fn(Tc[i][kk], x0, x1)

            def mov1(i, kk):
                if i == 2:
                    return h0T[kk][:, lo]       # Q11
                if i == 5:
                    return h0T[8 + kk][:, hi]   # Q22
                return Tc[i][kk]

            h1T = [
                pool.tile([P, B], bf16, tag="H1", bufs=MT1, name=f"h1T{m}")
                for m in range(MT1)
            ]
            # product order by L0-output availability (see combo comment):
            #   C11 = M1 + M4 - M5 + M7   (built as -M5, +M7, +M1, +M4)
            #   C12 = M3 + M5             (copy M5, +M3)
            #   C21 = M2 + M4             (copy M2, +M4)
            #   C22 = M1 - M2 + M3 + M6   (built as -M2, +M1, +M3, +M6)
            PORDER = (5, 7, 2, 1, 3, 4, 6)
            for j in range(8):
                for i in PORDER:
                    st = pool.tile([P, KH * P], bf16, tag="W1", bufs=6, name=f"s{i}_{j}")
                    nc.sync.dma_start(out=st, in_=w1[i - 1, j])
                    ps = pp.tile([P, NT], f32, tag="pm", bufs=6, name=f"p{i}_{j}")
                    for kk in range(KH):
                        nc.tensor.matmul(
                            ps,
                            st[:, ts(kk, P)],
                            mov1(i, kk),
                            start=(kk == 0),
                            stop=(kk == KH - 1),
                        )
                    if i == 5:
                        c11 = pool.tile([P, NT], bf16, tag="X", bufs=32, name=f"c11_{j}")
                        c12 = pool.tile([P, NT], bf16, tag="X", bufs=32, name=f"c12_{j}")
                        nc.vector.tensor_scalar_mul(c11, ps, -1.0)
                        nc.scalar.copy(c12, ps)
                    elif i == 7:
                        nc.vector.tensor_add(c11, c11, ps)
                    elif i == 2:
                        c21 = pool.tile([P, NT], bf16, tag="X", bufs=32, name=f"c21_{j}")
                        c22 = pool.tile([P, NT], bf16, tag="X", bufs=32, name=f"c22_{j}")
                        nc.scalar.copy(c21, ps)
                        nc.vector.tensor_scalar_mul(c22, ps, -1.0)
                    elif i == 1:
                        nc.vector.tensor_add(c11, c11, ps)
                        nc.vector.tensor_add(c22, c22, ps)
                    elif i == 3:
                        nc.vector.tensor_add(c12, c12, ps)
                        nc.vector.tensor_add(c22, c22, ps)
                        nc.scalar.activation(
                            h1T[j][:, hi], c12, AF.Relu, bias=b1_sb[:, ts(j, 1)]
                        )
                    elif i == 4:
                        nc.vector.tensor_add(c11, c11, ps)
                        nc.vector.tensor_add(c21, c21, ps)
                        nc.scalar.activation(
                            h1T[8 + j][:, lo], c21, AF.Relu, bias=b1_sb[:, ts(8 + j, 1)]
                        )
                        nc.scalar.activation(
                            h1T[j][:, lo], c11, AF.Relu, bias=b1_sb[:, ts(j, 1)]
                        )
                    elif i == 6:
                        nc.vector.tensor_add(c22, c22, ps)
                        nc.scalar.activation(
                            h1T[8 + j][:, hi], c22, AF.Relu, bias=b1_sb[:, ts(8 + j, 1)]
                        )

            # =================== layer 2 (one-level Strassen) ===================
            # out = h1 @ W2 over quadrants: the W2-side combos arrive from the
            # host as the 7 moving operands (TW ring, product-outer like L0);
            # the h1-side stationary combos SC run on Pool, emitted in the
            # order products consume them. Finished C quadrants get their
            # (free-dim) bias added on Pool and store immediately.
            nc.gpsimd.dma_start(out=b2_sb, in_=b2[:, :])
            TW = {}

            def tw_load(i):
                TW[i] = [
                    pool.tile([P, NT], bf16, tag="TW", bufs=16, name=f"TW{i}_{kk}")
                    for kk in range(KH)
                ]
                for kk in range(KH):
                    nc.sync.dma_start(out=TW[i][kk], in_=w2[i - 1, kk])

            SC = {
                i: [
                    pool.tile([P, NT], bf16, tag="T", bufs=40, name=f"SC{i}_{kk}")
                    for kk in range(KH)
                ]
                for i in (1, 2, 5, 6, 7)
            }
            for i, fn, sel in (
                (5, nc.gpsimd.tensor_add, lambda a, b_: (a[:, lo], b_[:, lo])),  # P11+P12
                (6, nc.gpsimd.tensor_sub, lambda a, b_: (a[:, hi], a[:, lo])),   # P21-P11
                (2, nc.gpsimd.tensor_add, lambda a, b_: (a[:, hi], b_[:, hi])),  # P21+P22
                (1, nc.gpsimd.tensor_add, lambda a, b_: (a[:, lo], b_[:, hi])),  # P11+P22
                (7, nc.gpsimd.tensor_sub, lambda a, b_: (b_[:, lo], b_[:, hi])),  # P12-P22
            ):
                for kk in range(KH):
                    x0, x1 = sel(h1T[kk], h1T[8 + kk])
                    fn(SC[i][kk], x0, x1)

            def stat2(i, j, kk):
                if i == 3:
                    return h1T[kk][:, ts(j, P)]                    # P11
                if i == 4:
                    return h1T[8 + kk][:, NT + j * P : NT + (j + 1) * P]  # P22
                return SC[i][kk][:, ts(j, P)]

            # quadrant -> (out row block base, out col half)
            QOUT = {11: (0, 0), 12: (0, 1), 21: (1, 0), 22: (1, 1)}

            def store_quad(q, j, ctile, chunks=1):
                rbase, chalf = QOUT[q]
                cw2 = NT // chunks
                for c in range(chunks):
                    o5 = pool.tile([P, cw2], f32, tag="O5", bufs=8, name=f"o{q}_{j}_{c}")
                    nc.gpsimd.tensor_add(
                        o5, ctile[:, ts(c, cw2)],
                        b2_sb[:, chalf * NT + c * cw2 : chalf * NT + (c + 1) * cw2],
                    )
                    nc.scalar.dma_start(
                        out=out[
                            rbase * NT + j * P : rbase * NT + (j + 1) * P,
                            chalf * NT + c * cw2 : chalf * NT + (c + 1) * cw2,
                        ],
                        in_=o5,
                    )

            # C11 = M1+M4-M5+M7, C12 = M3+M5, C21 = M2+M4, C22 = M1-M2+M3+M6
            PORD2 = (5, 3, 2, 6, 1, 4, 7)
            D2 = {q: [None] * 4 for q in (11, 12, 21, 22)}
            tw_load(PORD2[0])
            tw_load(PORD2[1])
            for idx, i in enumerate(PORD2):
                if idx + 2 < len(PORD2):
                    tw_load(PORD2[idx + 2])
                for j in range(4):
                    if i == 7 and j == 3:
                        # very last group: run it as four 128-wide PSUM
                        # sub-groups so each chunk's drain (DVE add -> Pool
                        # bias -> store, alternating Act/sync queues)
                        # pipelines against the PE's remaining sub-groups
                        cw2 = NT // 4
                        for c in range(4):
                            sl = ts(c, cw2)
                            psc = pp.tile([P, cw2], f32, tag="pm", bufs=6, name=f"rf_{c}")
                            for kk in range(KH):
                                nc.tensor.matmul(
                                    psc,
                                    stat2(i, j, kk),
                                    TW[i][kk][:, sl],
                                    start=(kk == 0),
                                    stop=(kk == KH - 1),
                                )
                            nc.vector.tensor_add(
                                D2[11][j][:, sl], D2[11][j][:, sl], psc
                            )
                            o5 = pool.tile([P, cw2], f32, tag="O5", bufs=8, name=f"of_{c}")
                            nc.gpsimd.tensor_add(
                                o5, D2[11][j][:, sl], b2_sb[:, c * cw2 : (c + 1) * cw2]
                            )
                            eng = nc.scalar if c % 2 == 0 else nc.sync
                            eng.dma_start(
                                out=out[j * P : (j + 1) * P, c * cw2 : (c + 1) * cw2],
                                in_=o5,
                            )
                        continue
                    ps = pp.tile([P, NT], f32, tag="pm", bufs=6, name=f"r{i}_{j}")
                    for kk in range(KH):
                        nc.tensor.matmul(
                            ps,
                            stat2(i, j, kk),
                            TW[i][kk],
                            start=(kk == 0),
                            stop=(kk == KH - 1),
                        )
                    if i == 5:
                        D2[11][j] = pool.tile([P, NT], bf16, tag="X", bufs=32, name=f"g11_{j}")
                        D2[12][j] = pool.tile([P, NT], bf16, tag="X", bufs=32, name=f"g12_{j}")
                        nc.vector.tensor_scalar_mul(D2[11][j], ps, -1.0)
                        nc.scalar.copy(D2[12][j], ps)
                    elif i == 3:
                        D2[22][j] = pool.tile([P, NT], bf16, tag="X", bufs=32, name=f"g22_{j}")
                        nc.vector.tensor_add(D2[12][j], D2[12][j], ps)
                        nc.scalar.copy(D2[22][j], ps)
                        store_quad(12, j, D2[12][j])
                    elif i == 2:
                        D2[21][j] = pool.tile([P, NT], bf16, tag="X", bufs=32, name=f"g21_{j}")
                        nc.vector.tensor_sub(D2[22][j], D2[22][j], ps)
                        nc.scalar.copy(D2[21][j], ps)
                    elif i == 6:
                        nc.vector.tensor_add(D2[22][j], D2[22][j], ps)
                    elif i == 1:
                        nc.vector.tensor_add(D2[22][j], D2[22][j], ps)
                        nc.vector.tensor_add(D2[11][j], D2[11][j], ps)
                        store_quad(22, j, D2[22][j])
                    elif i == 4:
                        nc.vector.tensor_add(D2[21][j], D2[21][j], ps)
                        nc.vector.tensor_add(D2[11][j], D2[11][j], ps)
                        store_quad(21, j, D2[21][j])
                    elif i == 7:
                        if j < 3:
                            nc.vector.tensor_add(D2[11][j], D2[11][j], ps)
                            store_quad(11, j, D2[11][j])
                        else:
                            # final quadrant: chunk the whole accumulate ->
                            # bias -> store chain so the post-PE tail is short
                            cw2 = NT // 4
                            for c in range(4):
                                sl = ts(c, cw2)
                                nc.vector.tensor_add(
                                    D2[11][j][:, sl], D2[11][j][:, sl], ps[:, sl]
                                )
                                o5 = pool.tile([P, cw2], f32, tag="O5", bufs=8, name=f"of_{c}")
                                nc.gpsimd.tensor_add(
                                    o5, D2[11][j][:, sl], b2_sb[:, c * cw2 : (c + 1) * cw2]
                                )
                                nc.scalar.dma_start(
                                    out=out[j * P : (j + 1) * P, c * cw2 : (c + 1) * cw2],
                                    in_=o5,
                                )

    if not nc.is_finalized():
        nc.finalize()
    return nc


def _get_nc():
    if "nc" not in _CACHE:
        _CACHE["nc"] = _build()
    return _CACHE["nc"]


def _task_in_map(inputs, t, bf16, b0c, b1c, b2c):
    W0 = inputs["k0"] + SCALING * (inputs["d0"][:, :, t] @ inputs["u0"][:, :, t])
    W1 = inputs["k1"] + SCALING * (inputs["d1"][:, :, t] @ inputs["u1"][:, :, t])
    W2 = inputs["k2"] + SCALING * (inputs["d2"][:, :, t] @ inputs["u2"][:, :, t])
    # layer-0 Strassen: both operand sets host-combined.
    # stationary S_i from W0 (512 x 1024) blocks
    blk0 = W0.reshape(2, 512, 2, 1024)
    S0 = np.stack(
        [
            blk0[0, :, 0] + blk0[1, :, 1],
            blk0[0, :, 1] + blk0[1, :, 1],
            blk0[0, :, 0],
            blk0[1, :, 1],
            blk0[0, :, 0] + blk0[1, :, 0],
            blk0[0, :, 1] - blk0[0, :, 0],
            blk0[1, :, 0] - blk0[1, :, 1],
        ]
    )  # [7, K/2, M/2]
    w0r = np.ascontiguousarray(
        S0.reshape(7, 4, 128, 8, 128).transpose(0, 3, 2, 1, 4).reshape(7, 8, 128, 512),
        dtype=bf16,
    )
    # moving combos T_i from x^T quadrants
    xT = inputs["x"][t].T
    Q11, Q12 = xT[:512, :512], xT[:512, 512:]
    Q21, Q22 = xT[512:, :512], xT[512:, 512:]
    xsr = np.ascontiguousarray(
        np.stack(
            [Q11 + Q22, Q11, Q12 - Q22, Q21 - Q11, Q22, Q11 + Q12, Q21 + Q22]
        ),
        dtype=bf16,
    )
    # layer-1 Strassen stationary operands from W1 quadrants blk[r, c]
    blk = W1.reshape(2, 1024, 2, 1024)
    S = np.stack(
        [
            blk[0, :, 0] + blk[1, :, 1],  # (P11+P22)^T
            blk[0, :, 1] + blk[1, :, 1],  # (P21+P22)^T
            blk[0, :, 0],                 # P11^T
            blk[1, :, 1],                 # P22^T
            blk[0, :, 0] + blk[1, :, 0],  # (P11+P12)^T
            blk[0, :, 1] - blk[0, :, 0],  # (P21-P11)^T
            blk[1, :, 0] - blk[1, :, 1],  # (P12-P22)^T
        ]
    )  # [7, K/2, M/2]
    w1r = np.ascontiguousarray(
        S.reshape(7, 8, 128, 8, 128).transpose(0, 3, 2, 1, 4).reshape(7, 8, 128, 1024),
        dtype=bf16,
    )
    # layer-2 Strassen moving operands from W2 (1024 x 512) blocks
    blk2 = W2.reshape(2, 1024, 2, 512)
    S2_ = np.stack(
        [
            blk2[0, :, 0] + blk2[1, :, 1],
            blk2[0, :, 0],
            blk2[0, :, 1] - blk2[1, :, 1],
            blk2[1, :, 0] - blk2[0, :, 0],
            blk2[1, :, 1],
            blk2[0, :, 0] + blk2[0, :, 1],
            blk2[1, :, 0] + blk2[1, :, 1],
        ]
    )  # [7, K/2, M3/2]
    w2r = np.ascontiguousarray(S2_.reshape(7, 8, 128, 512), dtype=bf16)
    return {
        "xt": xsr,
        "w0": w0r,
        "b0": b0c,
        "w1": w1r,
        "b1": b1c,
        "w2": w2r,
        "b2": b2c,
    }


def build_in_maps(inputs):
    import concurrent.futures

    import ml_dtypes

    bf16 = ml_dtypes.bfloat16
    b0c = np.ascontiguousarray(inputs["b0"].reshape(16, 128).T, dtype=np.float32)
    b1c = np.ascontiguousarray(inputs["b1"].reshape(16, 128).T, dtype=np.float32)
    b2c = np.ascontiguousarray(
        np.broadcast_to(inputs["b2"], (P, H3)), dtype=np.float32
    )
    with concurrent.futures.ThreadPoolExecutor(max_workers=T) as ex:
        in_maps = list(
            ex.map(lambda t: _task_in_map(inputs, t, bf16, b0c, b1c, b2c), range(T))
        )
    return in_maps


def kernel(**inputs):
    from concourse import bass_utils

    nc = _get_nc()
    in_maps = build_in_maps(inputs)
    res = bass_utils.run_bass_kernel_spmd(nc, in_maps, core_ids=list(range(T)))
    return np.stack([r["out"] for r in res.results], axis=0)


# revision 21
# speedup vs baseline: 1.1362x; 1.0010x over previous
"""Trainium2 Bass kernel for 3-layer per-task LoRA MLP.

Full-input contract: kernel(**inputs) takes the unsharded tensors and returns
the full [8, 1024, 1024] output. Internally the task axis (t=8) is sharded
across 8 NeuronCores (one task per core).

Strategy:
  - LoRA is folded on the host into per-task effective weights
    W_eff = k + (alpha/r) * d @ u  (standard LoRA weight merging), so the
    device kernel is a plain 3-layer MLP — no rank-8 matmuls on the PE.
  - weights and activations are bf16 on device (1 cycle/row on the PE, same
    as f32r, but half the DMA traffic and SBUF footprint); PSUM accumulation
    stays f32.
  - every layer runs one level of Strassen over 2x2 quadrant blocks: 7
    half-size products instead of 8 cuts PE row-streaming by 12.5% per
    layer. Weight-side operand combinations are free on the host; the
    activation-side combinations and C-quadrant accumulations are spread
    across the Pool/DVE/Act engines (Pool cannot read PSUM; DVE PSUM reads
    cost ~0.76us so each PSUM group gets at most one), overlapped so the
    PE stream stays gap-free. Measured pipeline error ~1e-2 relative
    (gate is 2e-2).
  - activations live as h^T [feat(part), batch(free)] with zero on-device
    transposes; the final layer uses h1^T slices as the *stationary*
    operand, producing natural-layout [batch, feat] output directly.
  - all host-side packs match the exact SBUF tile layout so every DMA is a
    contiguous >=1KB-per-partition stream, one DMA per consumed tile
    (consumers gate on whole-tile writes), spread over the three DMA
    channels (SP/Act HWDGE + Pool SWDGE) in consumption order.
  - a short PE warmup ramps the tensor-engine clock to 2.4 GHz while the
    first DMAs land; the final output quadrant drains in 128-wide chunks
    to shorten the post-PE tail.
"""

import sys

if "/opt/trn_rl_repo" not in sys.path:
    sys.path.insert(0, "/opt/trn_rl_repo")

import numpy as np

T, B, D = 8, 1024, 1024
H1, H2, H3 = 2048, 2048, 1024
SCALING = 2.0  # alpha/rank = 16/8
P = 128
NT = 512  # PSUM free-dim tile (fp32 one-bank limit)

_CACHE = {}


def _build():
    import concourse.mybir as mybir
    from concourse import bacc
    from concourse.tile import TileContext
    from concourse.bass import ts

    f32 = mybir.dt.float32
    bf16 = mybir.dt.bfloat16
    AF = mybir.ActivationFunctionType

    nc = bacc.Bacc(None, target_bir_lowering=False, name="lora_mlp")

    KT0 = D // P      # 8  k-tiles, layer 0
    KT1 = H1 // P     # 16 k-tiles, layer 1
    KT2 = H2 // P     # 16 k-tiles, layer 2
    MT0 = H1 // P     # 16 m-tiles, layer 0
    MT1 = H2 // P     # 16 m-tiles, layer 1
    BT = B // P       # 8  batch 128-tiles
    NB = B // NT      # 2  batch 512-halves (free dim, layers 0/1)
    N2 = H3 // NT     # 2  feature 512-halves (free dim, layer 2)

    # layer-0 inputs arrive Strassen-ready: xs = the 7 moving-operand
    # combos of x^T quadrants [K/2, B/2]; w0 = the 7 stationary operands
    xs = nc.dram_tensor("xt", (7, D // 2, B // 2), bf16, kind="ExternalInput")
    w0 = nc.dram_tensor("w0", (7, 8, P, 4 * P), bf16, kind="ExternalInput")
    b0 = nc.dram_tensor("b0", (P, MT0), f32, kind="ExternalInput")
    # layer-1 weights arrive as the 7 Strassen stationary operands
    # S_i [K/2, M/2], host-combined and packed [i, j(m-tile), p, kk*128+c]
    w1 = nc.dram_tensor("w1", (7, 8, P, 8 * P), bf16, kind="ExternalInput")
    b1 = nc.dram_tensor("b1", (P, MT1), f32, kind="ExternalInput")
    w2 = nc.dram_tensor("w2", (7, 8, P, H3 // 2), bf16, kind="ExternalInput")
    b2 = nc.dram_tensor("b2", (P, H3), f32, kind="ExternalInput")
    out = nc.dram_tensor("out", (B, H3), f32, kind="ExternalOutput")

    with TileContext(nc) as tc:
        with (
            tc.tile_pool(name="main", bufs=1) as pool,
            tc.tile_pool(name="psum", bufs=1, space="PSUM") as pp,
        ):
            # PE warmup: the tensor engine clock ramps with sustained use
            # (0.65 -> 1.2 -> 2.4 GHz over ~3us). Run throwaway matmuls on a
            # memset tile while the first input DMAs land, so the real
            # matmuls start at full clock.
            wu = pool.tile([P, 2 * P], bf16, tag="wu", bufs=1)
            nc.vector.memset(wu, 0.125)
            wps = pp.tile([P, 2 * P], f32, tag="wps", bufs=1)
            for _ in range(8):
                nc.tensor.matmul(wps, wu[:, :P], wu, start=True, stop=True)

            b0_sb = pool.tile([P, MT0], f32, tag="b0", bufs=1)
            b1_sb = pool.tile([P, MT1], f32, tag="b1", bufs=1)
            b2_sb = pool.tile([P, H3], f32, tag="b2", bufs=1)

            # =================== layer 0 (one-level Strassen) ===================
            # C = W0^T @ x over (512 x 1024) quadrant blocks; both operand
            # combos come pre-built on the host, so the device only runs the
            # 7 products and the C-quadrant accumulation. The product loop is
            # OUTER so each product's 4 moving tiles (TX ring, prefetched one
            # product ahead on Pool/Act queues) serve 8 consecutive groups —
            # the DMA stream never races the PE. First-product copies into
            # each C quadrant run on the Act engine, accumulating adds on
            # DVE, bias+relu on Pool (tensor_scalar add+max).
            h0T = [
                pool.tile([P, B], bf16, tag="H0", bufs=MT0, name=f"h0T{m}")
                for m in range(MT0)
            ]
            lo, hi = ts(0, NT), ts(1, NT)
            PORD0 = (2, 1, 3, 4, 5, 6, 7)
            alu = mybir.AluOpType

            def prelu(dst, src, bcol):
                nc.gpsimd.tensor_scalar(dst, src, bcol, 0.0, alu.add, alu.max)

            TX = {}

            def tx_load(i):
                TX[i] = [
                    pool.tile([P, NT], bf16, tag="TX", bufs=8, name=f"TX{i}_{kk}")
                    for kk in range(4)
                ]
                for kk in range(4):
                    eng = nc.gpsimd if kk < 2 else nc.scalar
                    eng.dma_start(out=TX[i][kk], in_=xs[i - 1, ts(kk, P), :])

            C = {q: [None] * 8 for q in (11, 12, 21, 22)}
            KH = KT1 // 2  # 8 k-tiles per K-half of layer 1
            Tc = {
                i: [
                    pool.tile([P, NT], bf16, tag="T", bufs=40, name=f"T{i}_{kk}")
                    for kk in range(KH)
                ]
                for i in (1, 3, 4, 6, 7)
            }
            with tc.high_priority():
                tx_load(PORD0[0])
                tx_load(PORD0[1])
                nc.gpsimd.dma_start(out=b0_sb, in_=b0[:, :])
                nc.gpsimd.dma_start(out=b1_sb, in_=b1[:, :])
            for idx, i in enumerate(PORD0):
                if idx + 2 < len(PORD0):
                    tx_load(PORD0[idx + 2])
                if i == 7:
                    # layer-1's T7 combos (Q21+Q22) only need L0's C21/C22
                    # quadrants — emit them ahead of i7's Pool relus, split
                    # across Pool and DVE so they emerge 2x faster than
                    # L1's second product consumes them
                    for kk in range(KH):
                        eng = nc.gpsimd if kk % 2 == 0 else nc.vector
                        eng.tensor_add(
                            Tc[7][kk], h0T[8 + kk][:, lo], h0T[8 + kk][:, hi]
                        )
                for j in range(8):
                    st = pool.tile([P, 4 * P], bf16, tag="W0s", bufs=6, name=f"s0_{i}_{j}")
                    nc.sync.dma_start(out=st, in_=w0[i - 1, j])
                    ps = pp.tile([P, NT], f32, tag="pm", bufs=6, name=f"q{i}_{j}")
                    for kk in range(4):
                        nc.tensor.matmul(
                            ps,
                            st[:, ts(kk, P)],
                            TX[i][kk],
                            start=(kk == 0),
                            stop=(kk == 3),
                        )
                    # Pool cannot read PSUM, and one DVE PSUM-read op costs
                    # ~0.76us vs the 0.85us group cadence — so each group
                    # carries exactly one DVE op; ACT copies shared products
                    # (M4, M5) to SBUF where Pool runs the second add.
                    if i == 2:
                        C[21][j] = pool.tile([P, NT], bf16, tag="X", bufs=32, name=f"d21_{j}")
                        C[22][j] = pool.tile([P, NT], bf16, tag="X", bufs=32, name=f"d22_{j}")
                        nc.scalar.copy(C[21][j], ps)
                        nc.vector.tensor_scalar_mul(C[22][j], ps, -1.0)
                    elif i == 1:
                        C[11][j] = pool.tile([P, NT], bf16, tag="X", bufs=32, name=f"d11_{j}")
                        nc.scalar.copy(C[11][j], ps)
                        nc.vector.tensor_add(C[22][j], C[22][j], ps)
                    elif i == 3:
                        C[12][j] = pool.tile([P, NT], bf16, tag="X", bufs=32, name=f"d12_{j}")
                        nc.scalar.copy(C[12][j], ps)
                        nc.vector.tensor_add(C[22][j], C[22][j], ps)
                    elif i == 4:
                        e4 = pool.tile([P, NT], bf16, tag="E", bufs=3, name=f"e4_{j}")
                        nc.scalar.copy(e4, ps)
                        nc.vector.tensor_add(C[11][j], C[11][j], ps)
                        nc.gpsimd.tensor_add(C[21][j], C[21][j], e4)
                        prelu(h0T[8 + j][:, lo], C[21][j], b0_sb[:, ts(8 + j, 1)])
                    elif i == 5:
                        e5 = pool.tile([P, NT], bf16, tag="E", bufs=3, name=f"e5_{j}")
                        nc.scalar.copy(e5, ps)
                        nc.vector.tensor_sub(C[11][j], C[11][j], ps)
                        nc.gpsimd.tensor_add(C[12][j], C[12][j], e5)
                        prelu(h0T[j][:, hi], C[12][j], b0_sb[:, ts(j, 1)])
                    elif i == 6:
                        nc.vector.tensor_add(C[22][j], C[22][j], ps)
                        prelu(h0T[8 + j][:, hi], C[22][j], b0_sb[:, ts(8 + j, 1)])
                    elif i == 7:
                        nc.vector.tensor_add(C[11][j], C[11][j], ps)
                        prelu(h0T[j][:, lo], C[11][j], b0_sb[:, ts(j, 1)])

            # =================== layer 1 (one-level Strassen) ===================
            # C = W1^T @ h0 over 1024x1024 quadrants: 7 half-size products
            # instead of 8 (PE rows 229376 vs 262144). Weight-side combos
            # S1..S7 are free on the host; activation-side combos T run on
            # the otherwise-idle Pool engine; products accumulate into the
            # four C quadrants via DVE reads of each product's PSUM bank.
            #   quadrant views of h0: Q11/Q12 = h0T[kk] cols lo/hi,
            #                         Q21/Q22 = h0T[8+kk] cols lo/hi
            # Product order is chosen by operand availability: L0 finishes
            # quadrants in the order C21(i4), C12(i5), C22(i6), C11(i7), so
            # L1 opens with M5 (pure Q22) and M7 (Q21+Q22) whose inputs are
            # ready a product-width before L0's PE stream even ends (T7's
            # combos were emitted inside the L0 loop for the same reason);
            # remaining combos are product-major so the Pool engine always
            # has a full product window of slack.
            for i, fn, sel in (
                (1, nc.gpsimd.tensor_add, lambda a, b_: (a[:, lo], b_[:, hi])),  # Q11+Q22
                (3, nc.gpsimd.tensor_sub, lambda a, b_: (a[:, hi], b_[:, hi])),  # Q12-Q22
                (4, nc.gpsimd.tensor_sub, lambda a, b_: (b_[:, lo], a[:, lo])),  # Q21-Q11
                (6, nc.gpsimd.tensor_add, lambda a, b_: (a[:, lo], a[:, hi])),   # Q11+Q12
            ):
                for kk in range(KH):
                    x0, x1 = sel(h0T[kk], h0T[8 + kk])
                    fn(Tc[i][kk], x0, x1)

            def mov1(i, kk):
                if i == 2:
                    return h0T[kk][:, lo]       # Q11
                if i == 5:
                    return h0T[8 + kk][:, hi]   # Q22
                return Tc[i][kk]

            h1T = [
                pool.tile([P, B], bf16, tag="H1", bufs=MT1, name=f"h1T{m}")
                for m in range(MT1)
            ]
            # product order by L0-output availability (see combo comment):
            #   C11 = M1 + M4 - M5 + M7   (built as -M5, +M7, +M1, +M4)
            #   C12 = M3 + M5             (copy M5, +M3)
            #   C21 = M2 + M4             (copy M2, +M4)
            #   C22 = M1 - M2 + M3 + M6   (built as -M2, +M1, +M3, +M6)
            PORDER = (5, 7, 2, 1, 3, 4, 6)
            for j in range(8):
                for i in PORDER:
                    st = pool.tile([P, KH * P], bf16, tag="W1", bufs=6, name=f"s{i}_{j}")
                    nc.sync.dma_start(out=st, in_=w1[i - 1, j])
                    ps = pp.tile([P, NT], f32, tag="pm", bufs=6, name=f"p{i}_{j}")
                    for kk in range(KH):
                        nc.tensor.matmul(
                            ps,
                            st[:, ts(kk, P)],
                            mov1(i, kk),
                            start=(kk == 0),
                            stop=(kk == KH - 1),
                        )
                    if i == 5:
                        c11 = pool.tile([P, NT], bf16, tag="X", bufs=32, name=f"c11_{j}")
                        c12 = pool.tile([P, NT], bf16, tag="X", bufs=32, name=f"c12_{j}")
                        nc.vector.tensor_scalar_mul(c11, ps, -1.0)
                        nc.scalar.copy(c12, ps)
                    elif i == 7:
                        nc.vector.tensor_add(c11, c11, ps)
                    elif i == 2:
                        c21 = pool.tile([P, NT], bf16, tag="X", bufs=32, name=f"c21_{j}")
                        c22 = pool.tile([P, NT], bf16, tag="X", bufs=32, name=f"c22_{j}")
                        nc.scalar.copy(c21, ps)
                        nc.vector.tensor_scalar_mul(c22, ps, -1.0)
                    elif i == 1:
                        nc.vector.tensor_add(c11, c11, ps)
                        nc.vector.tensor_add(c22, c22, ps)
                    elif i == 3:
                        nc.vector.tensor_add(c12, c12, ps)
                        nc.vector.tensor_add(c22, c22, ps)
                        nc.scalar.activation(
                            h1T[j][:, hi], c12, AF.Relu, bias=b1_sb[:, ts(j, 1)]
                        )
                    elif i == 4:
                        nc.vector.tensor_add(c11, c11, ps)
                        nc.vector.tensor_add(c21, c21, ps)
                        nc.scalar.activation(
                            h1T[8 + j][:, lo], c21, AF.Relu, bias=b1_sb[:, ts(8 + j, 1)]
                        )
                        nc.scalar.activation(
                            h1T[j][:, lo], c11, AF.Relu, bias=b1_sb[:, ts(j, 1)]
                        )
                    elif i == 6:
                        nc.vector.tensor_add(c22, c22, ps)
                        nc.scalar.activation(
                            h1T[8 + j][:, hi], c22, AF.Relu, bias=b1_sb[:, ts(8 + j, 1)]
                        )

            # =================== layer 2 (one-level Strassen) ===================
            # out = h1 @ W2 over quadrants: the W2-side combos arrive from the
            # host as the 7 moving operands (TW ring, product-outer like L0);
            # the h1-side stationary combos SC run on Pool, emitted in the
            # order products consume them. Finished C quadrants get their
            # (free-dim) bias added on Pool and store immediately.
            nc.gpsimd.dma_start(out=b2_sb, in_=b2[:, :])
            TW = {}

            def tw_load(i):
                TW[i] = [
                    pool.tile([P, NT], bf16, tag="TW", bufs=16, name=f"TW{i}_{kk}")
                    for kk in range(KH)
                ]
                for kk in range(KH):
                    nc.sync.dma_start(out=TW[i][kk], in_=w2[i - 1, kk])

            SC = {
                i: [
                    pool.tile([P, NT], bf16, tag="T", bufs=40, name=f"SC{i}_{kk}")
                    for kk in range(KH)
                ]
                for i in (1, 2, 5, 6, 7)
            }
            for i, fn, sel in (
                (5, nc.gpsimd.tensor_add, lambda a, b_: (a[:, lo], b_[:, lo])),  # P11+P12
                (6, nc.gpsimd.tensor_sub, lambda a, b_: (a[:, hi], a[:, lo])),   # P21-P11
                (2, nc.gpsimd.tensor_add, lambda a, b_: (a[:, hi], b_[:, hi])),  # P21+P22
                (1, nc.gpsimd.tensor_add, lambda a, b_: (a[:, lo], b_[:, hi])),  # P11+P22
                (7, nc.gpsimd.tensor_sub, lambda a, b_: (b_[:, lo], b_[:, hi])),  # P12-P22
            ):
                for kk in range(KH):
                    x0, x1 = sel(h1T[kk], h1T[8 + kk])
                    fn(SC[i][kk], x0, x1)

            def stat2(i, j, kk):
                if i == 3:
                    return h1T[kk][:, ts(j, P)]                    # P11
                if i == 4:
                    return h1T[8 + kk][:, NT + j * P : NT + (j + 1) * P]  # P22
                return SC[i][kk][:, ts(j, P)]

            # quadrant -> (out row block base, out col half)
            QOUT = {11: (0, 0), 12: (0, 1), 21: (1, 0), 22: (1, 1)}

            def store_quad(q, j, ctile, chunks=1):
                rbase, chalf = QOUT[q]
                cw2 = NT // chunks
                for c in range(chunks):
                    o5 = pool.tile([P, cw2], f32, tag="O5", bufs=8, name=f"o{q}_{j}_{c}")
                    nc.gpsimd.tensor_add(
                        o5, ctile[:, ts(c, cw2)],
                        b2_sb[:, chalf * NT + c * cw2 : chalf * NT + (c + 1) * cw2],
                    )
                    nc.scalar.dma_start(
                        out=out[
                            rbase * NT + j * P : rbase * NT + (j + 1) * P,
                            chalf * NT + c * cw2 : chalf * NT + (c + 1) * cw2,
                        ],
                        in_=o5,
                    )

            # C11 = M1+M4-M5+M7, C12 = M3+M5, C21 = M2+M4, C22 = M1-M2+M3+M6
            PORD2 = (5, 3, 2, 6, 1, 4, 7)
            D2 = {q: [None] * 4 for q in (11, 12, 21, 22)}
            tw_load(PORD2[0])
            tw_load(PORD2[1])
            for idx, i in enumerate(PORD2):
                if idx + 2 < len(PORD2):
                    tw_load(PORD2[idx + 2])
                for j in range(4):
                    if i == 7 and j == 3:
                        # very last group: run it as four 128-wide PSUM
                        # sub-groups so each chunk's drain (DVE add -> Pool
                        # bias -> store, alternating Act/sync queues)
                        # pipelines against the PE's remaining sub-groups
                        cw2 = NT // 4
                        for c in range(4):
                            sl = ts(c, cw2)
                            psc = pp.tile([P, cw2], f32, tag="pm", bufs=6, name=f"rf_{c}")
                            for kk in range(KH):
                                nc.tensor.matmul(
                                    psc,
                                    stat2(i, j, kk),
                                    TW[i][kk][:, sl],
                                    start=(kk == 0),
                                    stop=(kk == KH - 1),
                                )
                            nc.vector.tensor_add(
                                D2[11][j][:, sl], D2[11][j][:, sl], psc
                            )
                            o5 = pool.tile([P, cw2], f32, tag="O5", bufs=8, name=f"of_{c}")
                            nc.gpsimd.tensor_add(
                                o5, D2[11][j][:, sl], b2_sb[:, c * cw2 : (c + 1) * cw2]
                            )
                            # last chunk stores via Pool SWDGE: it directly
                            # follows the bias-add on the same engine, so it
                            # never queues behind another store
                            eng = (nc.scalar, nc.sync, nc.scalar, nc.gpsimd)[c]
                            eng.dma_start(
                                out=out[j * P : (j + 1) * P, c * cw2 : (c + 1) * cw2],
                                in_=o5,
                            )
                        continue
                    ps = pp.tile([P, NT], f32, tag="pm", bufs=6, name=f"r{i}_{j}")
                    for kk in range(KH):
                        nc.tensor.matmul(
                            ps,
                            stat2(i, j, kk),
                            TW[i][kk],
                            start=(kk == 0),
                            stop=(kk == KH - 1),
                        )
                    if i == 5:
                        D2[11][j] = pool.tile([P, NT], bf16, tag="X", bufs=32, name=f"g11_{j}")
                        D2[12][j] = pool.tile([P, NT], bf16, tag="X", bufs=32, name=f"g12_{j}")
                        nc.vector.tensor_scalar_mul(D2[11][j], ps, -1.0)
                        nc.scalar.copy(D2[12][j], ps)
                    elif i == 3:
                        D2[22][j] = pool.tile([P, NT], bf16, tag="X", bufs=32, name=f"g22_{j}")
                        nc.vector.tensor_add(D2[12][j], D2[12][j], ps)
                        nc.scalar.copy(D2[22][j], ps)
                        store_quad(12, j, D2[12][j])
                    elif i == 2:
                        D2[21][j] = pool.tile([P, NT], bf16, tag="X", bufs=32, name=f"g21_{j}")
                        nc.vector.tensor_sub(D2[22][j], D2[22][j], ps)
                        nc.scalar.copy(D2[21][j], ps)
                    elif i == 6:
                        nc.vector.tensor_add(D2[22][j], D2[22][j], ps)
                    elif i == 1:
                        nc.vector.tensor_add(D2[22][j], D2[22][j], ps)
                        nc.vector.tensor_add(D2[11][j], D2[11][j], ps)
                        store_quad(22, j, D2[22][j])
                    elif i == 4:
                        nc.vector.tensor_add(D2[21][j], D2[21][j], ps)
                        nc.vector.tensor_add(D2[11][j], D2[11][j], ps)
                        store_quad(21, j, D2[21][j])
                    elif i == 7:
                        if j < 3:
                            nc.vector.tensor_add(D2[11][j], D2[11][j], ps)
                            store_quad(11, j, D2[11][j])
                        else:
                            # final quadrant: chunk the whole accumulate ->
                            # bias -> store chain so the post-PE tail is short
                            cw2 = NT // 4
                            for c in range(4):
                                sl = ts(c, cw2)
                                nc.vector.tensor_add(
                                    D2[11][j][:, sl], D2[11][j][:, sl], ps[:, sl]
                                )
                                o5 = pool.tile([P, cw2], f32, tag="O5", bufs=8, name=f"of_{c}")
                                nc.gpsimd.tensor_add(
                                    o5, D2[11][j][:, sl], b2_sb[:, c * cw2 : (c + 1) * cw2]
                                )
                                nc.scalar.dma_start(
                                    out=out[j * P : (j + 1) * P, c * cw2 : (c + 1) * cw2],
                                    in_=o5,
                                )

    if not nc.is_finalized():
        nc.finalize()
    return nc


def _get_nc():
    if "nc" not in _CACHE:
        _CACHE["nc"] = _build()
    return _CACHE["nc"]


def _task_in_map(inputs, t, bf16, b0c, b1c, b2c):
    W0 = inputs["k0"] + SCALING * (inputs["d0"][:, :, t] @ inputs["u0"][:, :, t])
    W1 = inputs["k1"] + SCALING * (inputs["d1"][:, :, t] @ inputs["u1"][:, :, t])
    W2 = inputs["k2"] + SCALING * (inputs["d2"][:, :, t] @ inputs["u2"][:, :, t])
    # layer-0 Strassen: both operand sets host-combined.
    # stationary S_i from W0 (512 x 1024) blocks
    blk0 = W0.reshape(2, 512, 2, 1024)
    S0 = np.stack(
        [
            blk0[0, :, 0] + blk0[1, :, 1],
            blk0[0, :, 1] + blk0[1, :, 1],
            blk0[0, :, 0],
            blk0[1, :, 1],
            blk0[0, :, 0] + blk0[1, :, 0],
            blk0[0, :, 1] - blk0[0, :, 0],
            blk0[1, :, 0] - blk0[1, :, 1],
        ]
    )  # [7, K/2, M/2]
    w0r = np.ascontiguousarray(
        S0.reshape(7, 4, 128, 8, 128).transpose(0, 3, 2, 1, 4).reshape(7, 8, 128, 512),
        dtype=bf16,
    )
    # moving combos T_i from x^T quadrants
    xT = inputs["x"][t].T
    Q11, Q12 = xT[:512, :512], xT[:512, 512:]
    Q21, Q22 = xT[512:, :512], xT[512:, 512:]
    xsr = np.ascontiguousarray(
        np.stack(
            [Q11 + Q22, Q11, Q12 - Q22, Q21 - Q11, Q22, Q11 + Q12, Q21 + Q22]
        ),
        dtype=bf16,
    )
    # layer-1 Strassen stationary operands from W1 quadrants blk[r, c]
    blk = W1.reshape(2, 1024, 2, 1024)
    S = np.stack(
        [
            blk[0, :, 0] + blk[1, :, 1],  # (P11+P22)^T
            blk[0, :, 1] + blk[1, :, 1],  # (P21+P22)^T
            blk[0, :, 0],                 # P11^T
            blk[1, :, 1],                 # P22^T
            blk[0, :, 0] + blk[1, :, 0],  # (P11+P12)^T
            blk[0, :, 1] - blk[0, :, 0],  # (P21-P11)^T
            blk[1, :, 0] - blk[1, :, 1],  # (P12-P22)^T
        ]
    )  # [7, K/2, M/2]
    w1r = np.ascontiguousarray(
        S.reshape(7, 8, 128, 8, 128).transpose(0, 3, 2, 1, 4).reshape(7, 8, 128, 1024),
        dtype=bf16,
    )
    # layer-2 Strassen moving operands from W2 (1024 x 512) blocks
    blk2 = W2.reshape(2, 1024, 2, 512)
    S2_ = np.stack(
        [
            blk2[0, :, 0] + blk2[1, :, 1],
            blk2[0, :, 0],
            blk2[0, :, 1] - blk2[1, :, 1],
            blk2[1, :, 0] - blk2[0, :, 0],
            blk2[1, :, 1],
            blk2[0, :, 0] + blk2[0, :, 1],
            blk2[1, :, 0] + blk2[1, :, 1],
        ]
    )  # [7, K/2, M3/2]
    w2r = np.ascontiguousarray(S2_.reshape(7, 8, 128, 512), dtype=bf16)
    return {
        "xt": xsr,
        "w0": w0r,
        "b0": b0c,
        "w1": w1r,
        "b1": b1c,
        "w2": w2r,
        "b2": b2c,
    }


def build_in_maps(inputs):
    import concurrent.futures

    import ml_dtypes

    bf16 = ml_dtypes.bfloat16
    b0c = np.ascontiguousarray(inputs["b0"].reshape(16, 128).T, dtype=np.float32)
    b1c = np.ascontiguousarray(inputs["b1"].reshape(16, 128).T, dtype=np.float32)
    b2c = np.ascontiguousarray(
        np.broadcast_to(inputs["b2"], (P, H3)), dtype=np.float32
    )
    with concurrent.futures.ThreadPoolExecutor(max_workers=T) as ex:
        in_maps = list(
            ex.map(lambda t: _task_in_map(inputs, t, bf16, b0c, b1c, b2c), range(T))
        )
    return in_maps


def kernel(**inputs):
    from concourse import bass_utils

    nc = _get_nc()
    in_maps = build_in_maps(inputs)
    res = bass_utils.run_bass_kernel_spmd(nc, in_maps, core_ids=list(range(T)))
    return np.stack([r["out"] for r in res.results], axis=0)


# revision 22
# speedup vs baseline: 1.1407x; 1.0039x over previous
"""Trainium2 Bass kernel for 3-layer per-task LoRA MLP.

Full-input contract: kernel(**inputs) takes the unsharded tensors and returns
the full [8, 1024, 1024] output. Internally the task axis (t=8) is sharded
across 8 NeuronCores (one task per core).

Strategy:
  - LoRA is folded on the host into per-task effective weights
    W_eff = k + (alpha/r) * d @ u  (standard LoRA weight merging), so the
    device kernel is a plain 3-layer MLP — no rank-8 matmuls on the PE.
  - weights and activations are bf16 on device (1 cycle/row on the PE, same
    as f32r, but half the DMA traffic and SBUF footprint); PSUM accumulation
    stays f32.
  - every layer runs one level of Strassen over 2x2 quadrant blocks: 7
    half-size products instead of 8 cuts PE row-streaming by 12.5% per
    layer. Weight-side operand combinations are free on the host; the
    activation-side combinations and C-quadrant accumulations are spread
    across the Pool/DVE/Act engines (Pool cannot read PSUM; DVE PSUM reads
    cost ~0.76us so each PSUM group gets at most one), overlapped so the
    PE stream stays gap-free. Measured pipeline error ~1e-2 relative
    (gate is 2e-2).
  - activations live as h^T [feat(part), batch(free)] with zero on-device
    transposes; the final layer uses h1^T slices as the *stationary*
    operand, producing natural-layout [batch, feat] output directly.
  - all host-side packs match the exact SBUF tile layout so every DMA is a
    contiguous >=1KB-per-partition stream, one DMA per consumed tile
    (consumers gate on whole-tile writes), spread over the three DMA
    channels (SP/Act HWDGE + Pool SWDGE) in consumption order.
  - a short PE warmup ramps the tensor-engine clock to 2.4 GHz while the
    first DMAs land; the final output quadrant drains in 128-wide chunks
    to shorten the post-PE tail.
"""

import sys

if "/opt/trn_rl_repo" not in sys.path:
    sys.path.insert(0, "/opt/trn_rl_repo")

import numpy as np

T, B, D = 8, 1024, 1024
H1, H2, H3 = 2048, 2048, 1024
SCALING = 2.0  # alpha/rank = 16/8
P = 128
NT = 512  # PSUM free-dim tile (fp32 one-bank limit)

_CACHE = {}


def _build():
    import concourse.mybir as mybir
    from concourse import bacc
    from concourse.tile import TileContext
    from concourse.bass import ts

    f32 = mybir.dt.float32
    bf16 = mybir.dt.bfloat16
    AF = mybir.ActivationFunctionType

    nc = bacc.Bacc(None, target_bir_lowering=False, name="lora_mlp")

    KT0 = D // P      # 8  k-tiles, layer 0
    KT1 = H1 // P     # 16 k-tiles, layer 1
    KT2 = H2 // P     # 16 k-tiles, layer 2
    MT0 = H1 // P     # 16 m-tiles, layer 0
    MT1 = H2 // P     # 16 m-tiles, layer 1
    BT = B // P       # 8  batch 128-tiles
    NB = B // NT      # 2  batch 512-halves (free dim, layers 0/1)
    N2 = H3 // NT     # 2  feature 512-halves (free dim, layer 2)

    # layer-0 inputs arrive Strassen-ready: xs = the 7 moving-operand
    # combos of x^T quadrants [K/2, B/2]; w0 = the 7 stationary operands
    xs = nc.dram_tensor("xt", (7, D // 2, B // 2), bf16, kind="ExternalInput")
    w0 = nc.dram_tensor("w0", (7, 8, P, 4 * P), bf16, kind="ExternalInput")
    b0 = nc.dram_tensor("b0", (P, MT0), f32, kind="ExternalInput")
    # layer-1 weights arrive as the 7 Strassen stationary operands
    # S_i [K/2, M/2], host-combined and packed [i, j(m-tile), p, kk*128+c]
    w1 = nc.dram_tensor("w1", (7, 8, P, 8 * P), bf16, kind="ExternalInput")
    b1 = nc.dram_tensor("b1", (P, MT1), f32, kind="ExternalInput")
    w2 = nc.dram_tensor("w2", (7, 8, P, H3 // 2), bf16, kind="ExternalInput")
    b2 = nc.dram_tensor("b2", (P, H3), f32, kind="ExternalInput")
    out = nc.dram_tensor("out", (B, H3), f32, kind="ExternalOutput")

    with TileContext(nc) as tc:
        with (
            tc.tile_pool(name="main", bufs=1) as pool,
            tc.tile_pool(name="psum", bufs=1, space="PSUM") as pp,
        ):
            # PE warmup: the tensor engine clock ramps with sustained use
            # (0.65 -> 1.2 -> 2.4 GHz over ~3us). Run throwaway matmuls on a
            # memset tile while the first input DMAs land, so the real
            # matmuls start at full clock.
            wu = pool.tile([P, P], bf16, tag="wu", bufs=1)
            nc.vector.memset(wu, 0.125)
            wps = pp.tile([P, P], f32, tag="wps", bufs=1)
            for _ in range(4):
                nc.tensor.matmul(wps, wu, wu, start=True, stop=True)

            b0_sb = pool.tile([P, MT0], f32, tag="b0", bufs=1)
            b1_sb = pool.tile([P, MT1], f32, tag="b1", bufs=1)
            b2_sb = pool.tile([P, H3], f32, tag="b2", bufs=1)

            # =================== layer 0 (one-level Strassen) ===================
            # C = W0^T @ x over (512 x 1024) quadrant blocks; both operand
            # combos come pre-built on the host, so the device only runs the
            # 7 products and the C-quadrant accumulation. The product loop is
            # OUTER so each product's 4 moving tiles (TX ring, prefetched one
            # product ahead on Pool/Act queues) serve 8 consecutive groups —
            # the DMA stream never races the PE. First-product copies into
            # each C quadrant run on the Act engine, accumulating adds on
            # DVE, bias+relu on Pool (tensor_scalar add+max).
            h0T = [
                pool.tile([P, B], bf16, tag="H0", bufs=MT0, name=f"h0T{m}")
                for m in range(MT0)
            ]
            lo, hi = ts(0, NT), ts(1, NT)
            PORD0 = (2, 1, 3, 4, 5, 6, 7)
            alu = mybir.AluOpType

            def prelu(dst, src, bcol):
                nc.gpsimd.tensor_scalar(dst, src, bcol, 0.0, alu.add, alu.max)

            TX = {}

            def tx_load(i):
                TX[i] = [
                    pool.tile([P, NT], bf16, tag="TX", bufs=8, name=f"TX{i}_{kk}")
                    for kk in range(4)
                ]
                for kk in range(4):
                    eng = nc.gpsimd if kk < 2 else nc.scalar
                    eng.dma_start(out=TX[i][kk], in_=xs[i - 1, ts(kk, P), :])

            C = {q: [None] * 8 for q in (11, 12, 21, 22)}
            KH = KT1 // 2  # 8 k-tiles per K-half of layer 1
            Tc = {
                i: [
                    pool.tile([P, NT], bf16, tag="T", bufs=40, name=f"T{i}_{kk}")
                    for kk in range(KH)
                ]
                for i in (1, 3, 4, 6, 7)
            }
            with tc.high_priority():
                tx_load(PORD0[0])
                tx_load(PORD0[1])
                nc.gpsimd.dma_start(out=b0_sb, in_=b0[:, :])
                nc.gpsimd.dma_start(out=b1_sb, in_=b1[:, :])
            for idx, i in enumerate(PORD0):
                if idx + 2 < len(PORD0):
                    tx_load(PORD0[idx + 2])
                if i == 7:
                    # layer-1's T7 combos (Q21+Q22) only need L0's C21/C22
                    # quadrants — emit them ahead of i7's Pool relus, split
                    # across Pool and DVE so they emerge 2x faster than
                    # L1's second product consumes them
                    for kk in range(KH):
                        eng = nc.gpsimd if kk % 2 == 0 else nc.vector
                        eng.tensor_add(
                            Tc[7][kk], h0T[8 + kk][:, lo], h0T[8 + kk][:, hi]
                        )
                for j in range(8):
                    st = pool.tile([P, 4 * P], bf16, tag="W0s", bufs=6, name=f"s0_{i}_{j}")
                    nc.sync.dma_start(out=st, in_=w0[i - 1, j])
                    ps = pp.tile([P, NT], f32, tag="pm", bufs=6, name=f"q{i}_{j}")
                    for kk in range(4):
                        nc.tensor.matmul(
                            ps,
                            st[:, ts(kk, P)],
                            TX[i][kk],
                            start=(kk == 0),
                            stop=(kk == 3),
                        )
                    # Pool cannot read PSUM, and one DVE PSUM-read op costs
                    # ~0.76us vs the 0.85us group cadence — so each group
                    # carries exactly one DVE op; ACT copies shared products
                    # (M4, M5) to SBUF where Pool runs the second add.
                    if i == 2:
                        C[21][j] = pool.tile([P, NT], bf16, tag="X", bufs=32, name=f"d21_{j}")
                        C[22][j] = pool.tile([P, NT], bf16, tag="X", bufs=32, name=f"d22_{j}")
                        nc.scalar.copy(C[21][j], ps)
                        nc.vector.tensor_scalar_mul(C[22][j], ps, -1.0)
                    elif i == 1:
                        C[11][j] = pool.tile([P, NT], bf16, tag="X", bufs=32, name=f"d11_{j}")
                        nc.scalar.copy(C[11][j], ps)
                        nc.vector.tensor_add(C[22][j], C[22][j], ps)
                    elif i == 3:
                        C[12][j] = pool.tile([P, NT], bf16, tag="X", bufs=32, name=f"d12_{j}")
                        nc.scalar.copy(C[12][j], ps)
                        nc.vector.tensor_add(C[22][j], C[22][j], ps)
                    elif i == 4:
                        e4 = pool.tile([P, NT], bf16, tag="E", bufs=3, name=f"e4_{j}")
                        nc.scalar.copy(e4, ps)
                        nc.vector.tensor_add(C[11][j], C[11][j], ps)
                        nc.gpsimd.tensor_add(C[21][j], C[21][j], e4)
                        prelu(h0T[8 + j][:, lo], C[21][j], b0_sb[:, ts(8 + j, 1)])
                    elif i == 5:
                        e5 = pool.tile([P, NT], bf16, tag="E", bufs=3, name=f"e5_{j}")
                        nc.scalar.copy(e5, ps)
                        nc.vector.tensor_sub(C[11][j], C[11][j], ps)
                        nc.gpsimd.tensor_add(C[12][j], C[12][j], e5)
                        prelu(h0T[j][:, hi], C[12][j], b0_sb[:, ts(j, 1)])
                    elif i == 6:
                        nc.vector.tensor_add(C[22][j], C[22][j], ps)
                        prelu(h0T[8 + j][:, hi], C[22][j], b0_sb[:, ts(8 + j, 1)])
                    elif i == 7:
                        nc.vector.tensor_add(C[11][j], C[11][j], ps)
                        prelu(h0T[j][:, lo], C[11][j], b0_sb[:, ts(j, 1)])

            # =================== layer 1 (one-level Strassen) ===================
            # C = W1^T @ h0 over 1024x1024 quadrants: 7 half-size products
            # instead of 8 (PE rows 229376 vs 262144). Weight-side combos
            # S1..S7 are free on the host; activation-side combos T run on
            # the otherwise-idle Pool engine; products accumulate into the
            # four C quadrants via DVE reads of each product's PSUM bank.
            #   quadrant views of h0: Q11/Q12 = h0T[kk] cols lo/hi,
            #                         Q21/Q22 = h0T[8+kk] cols lo/hi
            # Product order is chosen by operand availability: L0 finishes
            # quadrants in the order C21(i4), C12(i5), C22(i6), C11(i7), so
            # L1 opens with M5 (pure Q22) and M7 (Q21+Q22) whose inputs are
            # ready a product-width before L0's PE stream even ends (T7's
            # combos were emitted inside the L0 loop for the same reason);
            # remaining combos are product-major so the Pool engine always
            # has a full product window of slack.
            for i, fn, sel in (
                (1, nc.gpsimd.tensor_add, lambda a, b_: (a[:, lo], b_[:, hi])),  # Q11+Q22
                (3, nc.gpsimd.tensor_sub, lambda a, b_: (a[:, hi], b_[:, hi])),  # Q12-Q22
                (4, nc.gpsimd.tensor_sub, lambda a, b_: (b_[:, lo], a[:, lo])),  # Q21-Q11
                (6, nc.gpsimd.tensor_add, lambda a, b_: (a[:, lo], a[:, hi])),   # Q11+Q12
            ):
                for kk in range(KH):
                    x0, x1 = sel(h0T[kk], h0T[8 + kk])
                    fn(Tc[i][kk], x0, x1)

            def mov1(i, kk):
                if i == 2:
                    return h0T[kk][:, lo]       # Q11
                if i == 5:
                    return h0T[8 + kk][:, hi]   # Q22
                return Tc[i][kk]

            h1T = [
                pool.tile([P, B], bf16, tag="H1", bufs=MT1, name=f"h1T{m}")
                for m in range(MT1)
            ]
            # product order by L0-output availability (see combo comment):
            #   C11 = M1 + M4 - M5 + M7   (built as -M5, +M7, +M1, +M4)
            #   C12 = M3 + M5             (copy M5, +M3)
            #   C21 = M2 + M4             (copy M2, +M4)
            #   C22 = M1 - M2 + M3 + M6   (built as -M2, +M1, +M3, +M6)
            PORDER = (5, 7, 2, 1, 3, 4, 6)
            for j in range(8):
                for i in PORDER:
                    st = pool.tile([P, KH * P], bf16, tag="W1", bufs=6, name=f"s{i}_{j}")
                    nc.sync.dma_start(out=st, in_=w1[i - 1, j])
                    ps = pp.tile([P, NT], f32, tag="pm", bufs=6, name=f"p{i}_{j}")
                    for kk in range(KH):
                        nc.tensor.matmul(
                            ps,
                            st[:, ts(kk, P)],
                            mov1(i, kk),
                            start=(kk == 0),
                            stop=(kk == KH - 1),
                        )
                    if i == 5:
                        c11 = pool.tile([P, NT], bf16, tag="X", bufs=32, name=f"c11_{j}")
                        c12 = pool.tile([P, NT], bf16, tag="X", bufs=32, name=f"c12_{j}")
                        nc.vector.tensor_scalar_mul(c11, ps, -1.0)
                        nc.scalar.copy(c12, ps)
                    elif i == 7:
                        nc.vector.tensor_add(c11, c11, ps)
                    elif i == 2:
                        c21 = pool.tile([P, NT], bf16, tag="X", bufs=32, name=f"c21_{j}")
                        c22 = pool.tile([P, NT], bf16, tag="X", bufs=32, name=f"c22_{j}")
                        nc.scalar.copy(c21, ps)
                        nc.vector.tensor_scalar_mul(c22, ps, -1.0)
                    elif i == 1:
                        nc.vector.tensor_add(c11, c11, ps)
                        nc.vector.tensor_add(c22, c22, ps)
                    elif i == 3:
                        nc.vector.tensor_add(c12, c12, ps)
                        nc.vector.tensor_add(c22, c22, ps)
                        nc.scalar.activation(
                            h1T[j][:, hi], c12, AF.Relu, bias=b1_sb[:, ts(j, 1)]
                        )
                    elif i == 4:
                        nc.vector.tensor_add(c11, c11, ps)
                        nc.vector.tensor_add(c21, c21, ps)
                        nc.scalar.activation(
                            h1T[8 + j][:, lo], c21, AF.Relu, bias=b1_sb[:, ts(8 + j, 1)]
                        )
                        nc.scalar.activation(
                            h1T[j][:, lo], c11, AF.Relu, bias=b1_sb[:, ts(j, 1)]
                        )
                    elif i == 6:
                        nc.vector.tensor_add(c22, c22, ps)
                        nc.scalar.activation(
                            h1T[8 + j][:, hi], c22, AF.Relu, bias=b1_sb[:, ts(8 + j, 1)]
                        )

            # =================== layer 2 (one-level Strassen) ===================
            # out = h1 @ W2 over quadrants: the W2-side combos arrive from the
            # host as the 7 moving operands (TW ring, product-outer like L0);
            # the h1-side stationary combos SC run on Pool, emitted in the
            # order products consume them. Finished C quadrants get their
            # (free-dim) bias added on Pool and store immediately.
            nc.gpsimd.dma_start(out=b2_sb, in_=b2[:, :])
            TW = {}

            def tw_load(i):
                TW[i] = [
                    pool.tile([P, NT], bf16, tag="TW", bufs=16, name=f"TW{i}_{kk}")
                    for kk in range(KH)
                ]
                for kk in range(KH):
                    nc.sync.dma_start(out=TW[i][kk], in_=w2[i - 1, kk])

            SC = {
                i: [
                    pool.tile([P, NT], bf16, tag="T", bufs=40, name=f"SC{i}_{kk}")
                    for kk in range(KH)
                ]
                for i in (1, 2, 5, 6, 7)
            }
            for i, fn, sel in (
                (5, nc.gpsimd.tensor_add, lambda a, b_: (a[:, lo], b_[:, lo])),  # P11+P12
                (6, nc.gpsimd.tensor_sub, lambda a, b_: (a[:, hi], a[:, lo])),   # P21-P11
                (2, nc.gpsimd.tensor_add, lambda a, b_: (a[:, hi], b_[:, hi])),  # P21+P22
                (1, nc.gpsimd.tensor_add, lambda a, b_: (a[:, lo], b_[:, hi])),  # P11+P22
                (7, nc.gpsimd.tensor_sub, lambda a, b_: (b_[:, lo], b_[:, hi])),  # P12-P22
            ):
                for kk in range(KH):
                    x0, x1 = sel(h1T[kk], h1T[8 + kk])
                    fn(SC[i][kk], x0, x1)

            def stat2(i, j, kk):
                if i == 3:
                    return h1T[kk][:, ts(j, P)]                    # P11
                if i == 4:
                    return h1T[8 + kk][:, NT + j * P : NT + (j + 1) * P]  # P22
                return SC[i][kk][:, ts(j, P)]

            # quadrant -> (out row block base, out col half)
            QOUT = {11: (0, 0), 12: (0, 1), 21: (1, 0), 22: (1, 1)}

            def store_quad(q, j, ctile, chunks=1):
                rbase, chalf = QOUT[q]
                cw2 = NT // chunks
                for c in range(chunks):
                    o5 = pool.tile([P, cw2], f32, tag="O5", bufs=8, name=f"o{q}_{j}_{c}")
                    nc.gpsimd.tensor_add(
                        o5, ctile[:, ts(c, cw2)],
                        b2_sb[:, chalf * NT + c * cw2 : chalf * NT + (c + 1) * cw2],
                    )
                    nc.scalar.dma_start(
                        out=out[
                            rbase * NT + j * P : rbase * NT + (j + 1) * P,
                            chalf * NT + c * cw2 : chalf * NT + (c + 1) * cw2,
                        ],
                        in_=o5,
                    )

            # C11 = M1+M4-M5+M7, C12 = M3+M5, C21 = M2+M4, C22 = M1-M2+M3+M6
            PORD2 = (5, 3, 2, 6, 1, 4, 7)
            D2 = {q: [None] * 4 for q in (11, 12, 21, 22)}
            tw_load(PORD2[0])
            tw_load(PORD2[1])
            for idx, i in enumerate(PORD2):
                if idx + 2 < len(PORD2):
                    tw_load(PORD2[idx + 2])
                for j in range(4):
                    if i == 7 and j == 3:
                        # very last group: run it as four 128-wide PSUM
                        # sub-groups so each chunk's drain (DVE add -> Pool
                        # bias -> store, alternating Act/sync queues)
                        # pipelines against the PE's remaining sub-groups
                        cw2 = NT // 4
                        for c in range(4):
                            sl = ts(c, cw2)
                            psc = pp.tile([P, cw2], f32, tag="pm", bufs=6, name=f"rf_{c}")
                            for kk in range(KH):
                                nc.tensor.matmul(
                                    psc,
                                    stat2(i, j, kk),
                                    TW[i][kk][:, sl],
                                    start=(kk == 0),
                                    stop=(kk == KH - 1),
                                )
                            nc.vector.tensor_add(
                                D2[11][j][:, sl], D2[11][j][:, sl], psc
                            )
                            o5 = pool.tile([P, cw2], f32, tag="O5", bufs=8, name=f"of_{c}")
                            nc.gpsimd.tensor_add(
                                o5, D2[11][j][:, sl], b2_sb[:, c * cw2 : (c + 1) * cw2]
                            )
                            # last chunk stores via Pool SWDGE: it directly
                            # follows the bias-add on the same engine, so it
                            # never queues behind another store
                            eng = (nc.scalar, nc.sync, nc.scalar, nc.gpsimd)[c]
                            eng.dma_start(
                                out=out[j * P : (j + 1) * P, c * cw2 : (c + 1) * cw2],
                                in_=o5,
                            )
                        continue
                    ps = pp.tile([P, NT], f32, tag="pm", bufs=6, name=f"r{i}_{j}")
                    for kk in range(KH):
                        nc.tensor.matmul(
                            ps,
                            stat2(i, j, kk),
                            TW[i][kk],
                            start=(kk == 0),
                            stop=(kk == KH - 1),
                        )
                    if i == 5:
                        D2[11][j] = pool.tile([P, NT], bf16, tag="X", bufs=32, name=f"g11_{j}")
                        D2[12][j] = pool.tile([P, NT], bf16, tag="X", bufs=32, name=f"g12_{j}")
                        nc.vector.tensor_scalar_mul(D2[11][j], ps, -1.0)
                        nc.scalar.copy(D2[12][j], ps)
                    elif i == 3:
                        D2[22][j] = pool.tile([P, NT], bf16, tag="X", bufs=32, name=f"g22_{j}")
                        nc.vector.tensor_add(D2[12][j], D2[12][j], ps)
                        nc.scalar.copy(D2[22][j], ps)
                        store_quad(12, j, D2[12][j])
                    elif i == 2:
                        D2[21][j] = pool.tile([P, NT], bf16, tag="X", bufs=32, name=f"g21_{j}")
                        nc.vector.tensor_sub(D2[22][j], D2[22][j], ps)
                        nc.scalar.copy(D2[21][j], ps)
                    elif i == 6:
                        nc.vector.tensor_add(D2[22][j], D2[22][j], ps)
                    elif i == 1:
                        nc.vector.tensor_add(D2[22][j], D2[22][j], ps)
                        nc.vector.tensor_add(D2[11][j], D2[11][j], ps)
                        store_quad(22, j, D2[22][j])
                    elif i == 4:
                        nc.vector.tensor_add(D2[21][j], D2[21][j], ps)
                        nc.vector.tensor_add(D2[11][j], D2[11][j], ps)
                        store_quad(21, j, D2[21][j])
                    elif i == 7:
                        if j < 3:
                            nc.vector.tensor_add(D2[11][j], D2[11][j], ps)
                            store_quad(11, j, D2[11][j])
                        else:
                            # final quadrant: chunk the whole accumulate ->
                            # bias -> store chain so the post-PE tail is short
                            cw2 = NT // 4
                            for c in range(4):
                                sl = ts(c, cw2)
                                nc.vector.tensor_add(
                                    D2[11][j][:, sl], D2[11][j][:, sl], ps[:, sl]
                                )
                                o5 = pool.tile([P, cw2], f32, tag="O5", bufs=8, name=f"of_{c}")
                                nc.gpsimd.tensor_add(
                                    o5, D2[11][j][:, sl], b2_sb[:, c * cw2 : (c + 1) * cw2]
                                )
                                nc.scalar.dma_start(
                                    out=out[j * P : (j + 1) * P, c * cw2 : (c + 1) * cw2],
                                    in_=o5,
                                )

    if not nc.is_finalized():
        nc.finalize()
    return nc


def _get_nc():
    if "nc" not in _CACHE:
        _CACHE["nc"] = _build()
    return _CACHE["nc"]


def _task_in_map(inputs, t, bf16, b0c, b1c, b2c):
    W0 = inputs["k0"] + SCALING * (inputs["d0"][:, :, t] @ inputs["u0"][:, :, t])
    W1 = inputs["k1"] + SCALING * (inputs["d1"][:, :, t] @ inputs["u1"][:, :, t])
    W2 = inputs["k2"] + SCALING * (inputs["d2"][:, :, t] @ inputs["u2"][:, :, t])
    # layer-0 Strassen: both operand sets host-combined.
    # stationary S_i from W0 (512 x 1024) blocks
    blk0 = W0.reshape(2, 512, 2, 1024)
    S0 = np.stack(
        [
            blk0[0, :, 0] + blk0[1, :, 1],
            blk0[0, :, 1] + blk0[1, :, 1],
            blk0[0, :, 0],
            blk0[1, :, 1],
            blk0[0, :, 0] + blk0[1, :, 0],
            blk0[0, :, 1] - blk0[0, :, 0],
            blk0[1, :, 0] - blk0[1, :, 1],
        ]
    )  # [7, K/2, M/2]
    w0r = np.ascontiguousarray(
        S0.reshape(7, 4, 128, 8, 128).transpose(0, 3, 2, 1, 4).reshape(7, 8, 128, 512),
        dtype=bf16,
    )
    # moving combos T_i from x^T quadrants
    xT = inputs["x"][t].T
    Q11, Q12 = xT[:512, :512], xT[:512, 512:]
    Q21, Q22 = xT[512:, :512], xT[512:, 512:]
    xsr = np.ascontiguousarray(
        np.stack(
            [Q11 + Q22, Q11, Q12 - Q22, Q21 - Q11, Q22, Q11 + Q12, Q21 + Q22]
        ),
        dtype=bf16,
    )
    # layer-1 Strassen stationary operands from W1 quadrants blk[r, c]
    blk = W1.reshape(2, 1024, 2, 1024)
    S = np.stack(
        [
            blk[0, :, 0] + blk[1, :, 1],  # (P11+P22)^T
            blk[0, :, 1] + blk[1, :, 1],  # (P21+P22)^T
            blk[0, :, 0],                 # P11^T
            blk[1, :, 1],                 # P22^T
            blk[0, :, 0] + blk[1, :, 0],  # (P11+P12)^T
            blk[0, :, 1] - blk[0, :, 0],  # (P21-P11)^T
            blk[1, :, 0] - blk[1, :, 1],  # (P12-P22)^T
        ]
    )  # [7, K/2, M/2]
    w1r = np.ascontiguousarray(
        S.reshape(7, 8, 128, 8, 128).transpose(0, 3, 2, 1, 4).reshape(7, 8, 128, 1024),
        dtype=bf16,
    )
    # layer-2 Strassen moving operands from W2 (1024 x 512) blocks
    blk2 = W2.reshape(2, 1024, 2, 512)
    S2_ = np.stack(
        [
            blk2[0, :, 0] + blk2[1, :, 1],
            blk2[0, :, 0],
            blk2[0, :, 1] - blk2[1, :, 1],
            blk2[1, :, 0] - blk2[0, :, 0],
            blk2[1, :, 1],
            blk2[0, :, 0] + blk2[0, :, 1],
            blk2[1, :, 0] + blk2[1, :, 1],
        ]
    )  # [7, K/2, M3/2]
    w2r = np.ascontiguousarray(S2_.reshape(7, 8, 128, 512), dtype=bf16)
    return {
        "xt": xsr,
        "w0": w0r,
        "b0": b0c,
        "w1": w1r,
        "b1": b1c,
        "w2": w2r,
        "b2": b2c,
    }


def build_in_maps(inputs):
    import concurrent.futures

    import ml_dtypes

    bf16 = ml_dtypes.bfloat16
    b0c = np.ascontiguousarray(inputs["b0"].reshape(16, 128).T, dtype=np.float32)
    b1c = np.ascontiguousarray(inputs["b1"].reshape(16, 128).T, dtype=np.float32)
    b2c = np.ascontiguousarray(
        np.broadcast_to(inputs["b2"], (P, H3)), dtype=np.float32
    )
    with concurrent.futures.ThreadPoolExecutor(max_workers=T) as ex:
        in_maps = list(
            ex.map(lambda t: _task_in_map(inputs, t, bf16, b0c, b1c, b2c), range(T))
        )
    return in_maps


def kernel(**inputs):
    from concourse import bass_utils

    nc = _get_nc()
    in_maps = build_in_maps(inputs)
    res = bass_utils.run_bass_kernel_spmd(nc, in_maps, core_ids=list(range(T)))
    return np.stack([r["out"] for r in res.results], axis=0)


# revision 23
# speedup vs baseline: 1.1419x; 1.0011x over previous
"""Trainium2 Bass kernel for 3-layer per-task LoRA MLP.

Full-input contract: kernel(**inputs) takes the unsharded tensors and returns
the full [8, 1024, 1024] output. Internally the task axis (t=8) is sharded
across 8 NeuronCores (one task per core).

Strategy:
  - LoRA is folded on the host into per-task effective weights
    W_eff = k + (alpha/r) * d @ u  (standard LoRA weight merging), so the
    device kernel is a plain 3-layer MLP — no rank-8 matmuls on the PE.
  - weights and activations are bf16 on device (1 cycle/row on the PE, same
    as f32r, but half the DMA traffic and SBUF footprint); PSUM accumulation
    stays f32.
  - every layer runs one level of Strassen over 2x2 quadrant blocks: 7
    half-size products instead of 8 cuts PE row-streaming by 12.5% per
    layer. Weight-side operand combinations are free on the host; the
    activation-side combinations and C-quadrant accumulations are spread
    across the Pool/DVE/Act engines (Pool cannot read PSUM; DVE PSUM reads
    cost ~0.76us so each PSUM group gets at most one), overlapped so the
    PE stream stays gap-free. Measured pipeline error ~1e-2 relative
    (gate is 2e-2).
  - activations live as h^T [feat(part), batch(free)] with zero on-device
    transposes; the final layer uses h1^T slices as the *stationary*
    operand, producing natural-layout [batch, feat] output directly.
  - all host-side packs match the exact SBUF tile layout so every DMA is a
    contiguous >=1KB-per-partition stream, one DMA per consumed tile
    (consumers gate on whole-tile writes), spread over the three DMA
    channels (SP/Act HWDGE + Pool SWDGE) in consumption order.
  - a short PE warmup ramps the tensor-engine clock to 2.4 GHz while the
    first DMAs land; the final output quadrant drains in 128-wide chunks
    to shorten the post-PE tail.
"""

import sys

if "/opt/trn_rl_repo" not in sys.path:
    sys.path.insert(0, "/opt/trn_rl_repo")

import numpy as np

T, B, D = 8, 1024, 1024
H1, H2, H3 = 2048, 2048, 1024
SCALING = 2.0  # alpha/rank = 16/8
P = 128
NT = 512  # PSUM free-dim tile (fp32 one-bank limit)

_CACHE = {}


def _build():
    import concourse.mybir as mybir
    from concourse import bacc
    from concourse.tile import TileContext
    from concourse.bass import ts

    f32 = mybir.dt.float32
    bf16 = mybir.dt.bfloat16
    AF = mybir.ActivationFunctionType

    nc = bacc.Bacc(None, target_bir_lowering=False, name="lora_mlp")

    KT0 = D // P      # 8  k-tiles, layer 0
    KT1 = H1 // P     # 16 k-tiles, layer 1
    KT2 = H2 // P     # 16 k-tiles, layer 2
    MT0 = H1 // P     # 16 m-tiles, layer 0
    MT1 = H2 // P     # 16 m-tiles, layer 1
    BT = B // P       # 8  batch 128-tiles
    NB = B // NT      # 2  batch 512-halves (free dim, layers 0/1)
    N2 = H3 // NT     # 2  feature 512-halves (free dim, layer 2)

    # layer-0 inputs arrive Strassen-ready: xs = the 7 moving-operand
    # combos of x^T quadrants [K/2, B/2]; w0 = the 7 stationary operands
    xs = nc.dram_tensor("xt", (7, D // 2, B // 2), bf16, kind="ExternalInput")
    w0 = nc.dram_tensor("w0", (7, 8, P, 4 * P), bf16, kind="ExternalInput")
    b0 = nc.dram_tensor("b0", (P, MT0), f32, kind="ExternalInput")
    # layer-1 weights arrive as the 7 Strassen stationary operands
    # S_i [K/2, M/2], host-combined and packed [i, j(m-tile), p, kk*128+c]
    w1 = nc.dram_tensor("w1", (7, 8, P, 8 * P), bf16, kind="ExternalInput")
    b1 = nc.dram_tensor("b1", (P, MT1), f32, kind="ExternalInput")
    w2 = nc.dram_tensor("w2", (7, 8, P, H3 // 2), bf16, kind="ExternalInput")
    b2 = nc.dram_tensor("b2", (P, H3), f32, kind="ExternalInput")
    out = nc.dram_tensor("out", (B, H3), f32, kind="ExternalOutput")

    with TileContext(nc) as tc:
        with (
            tc.tile_pool(name="main", bufs=1) as pool,
            tc.tile_pool(name="psum", bufs=1, space="PSUM") as pp,
        ):
            # PE warmup: the tensor engine clock ramps with sustained use
            # (0.65 -> 1.2 -> 2.4 GHz over ~3us). Run throwaway matmuls on a
            # memset tile while the first input DMAs land, so the real
            # matmuls start at full clock.
            wu = pool.tile([P, P], bf16, tag="wu", bufs=1)
            nc.vector.memset(wu, 0.125)
            wps = pp.tile([P, P], f32, tag="wps", bufs=1)
            for _ in range(4):
                nc.tensor.matmul(wps, wu, wu, start=True, stop=True)

            b0_sb = pool.tile([P, MT0], f32, tag="b0", bufs=1)
            b1_sb = pool.tile([P, MT1], f32, tag="b1", bufs=1)
            b2_sb = pool.tile([P, H3], f32, tag="b2", bufs=1)

            # =================== layer 0 (one-level Strassen) ===================
            # C = W0^T @ x over (512 x 1024) quadrant blocks; both operand
            # combos come pre-built on the host, so the device only runs the
            # 7 products and the C-quadrant accumulation. The product loop is
            # OUTER so each product's 4 moving tiles (TX ring, prefetched one
            # product ahead on Pool/Act queues) serve 8 consecutive groups —
            # the DMA stream never races the PE. First-product copies into
            # each C quadrant run on the Act engine, accumulating adds on
            # DVE, bias+relu on Pool (tensor_scalar add+max).
            h0T = [
                pool.tile([P, B], bf16, tag="H0", bufs=MT0, name=f"h0T{m}")
                for m in range(MT0)
            ]
            lo, hi = ts(0, NT), ts(1, NT)
            PORD0 = (2, 1, 3, 4, 5, 6, 7)
            alu = mybir.AluOpType

            def prelu(dst, src, bcol):
                nc.gpsimd.tensor_scalar(dst, src, bcol, 0.0, alu.add, alu.max)

            TX = {}

            def tx_load(i):
                TX[i] = [
                    pool.tile([P, NT], bf16, tag="TX", bufs=8, name=f"TX{i}_{kk}")
                    for kk in range(4)
                ]
                for kk in range(4):
                    eng = nc.gpsimd if kk < 2 else nc.scalar
                    eng.dma_start(out=TX[i][kk], in_=xs[i - 1, ts(kk, P), :])

            C = {q: [None] * 8 for q in (11, 12, 21, 22)}
            KH = KT1 // 2  # 8 k-tiles per K-half of layer 1
            Tc = {
                i: [
                    pool.tile([P, NT], bf16, tag="T", bufs=40, name=f"T{i}_{kk}")
                    for kk in range(KH)
                ]
                for i in (1, 3, 4, 6, 7)
            }
            with tc.high_priority():
                tx_load(PORD0[0])
                tx_load(PORD0[1])
                nc.gpsimd.dma_start(out=b0_sb, in_=b0[:, :])
                nc.gpsimd.dma_start(out=b1_sb, in_=b1[:, :])
            for idx, i in enumerate(PORD0):
                if idx + 2 < len(PORD0):
                    tx_load(PORD0[idx + 2])
                if i == 7:
                    # layer-1's T7 combos (Q21+Q22) only need L0's C21/C22
                    # quadrants — emit them ahead of i7's Pool relus, split
                    # across Pool and DVE so they emerge 2x faster than
                    # L1's second product consumes them
                    for kk in range(KH):
                        eng = nc.gpsimd if kk % 2 == 0 else nc.vector
                        eng.tensor_add(
                            Tc[7][kk], h0T[8 + kk][:, lo], h0T[8 + kk][:, hi]
                        )
                for j in range(8):
                    st = pool.tile([P, 4 * P], bf16, tag="W0s", bufs=6, name=f"s0_{i}_{j}")
                    nc.sync.dma_start(out=st, in_=w0[i - 1, j])
                    ps = pp.tile([P, NT], f32, tag="pm", bufs=6, name=f"q{i}_{j}")
                    for kk in range(4):
                        nc.tensor.matmul(
                            ps,
                            st[:, ts(kk, P)],
                            TX[i][kk],
                            start=(kk == 0),
                            stop=(kk == 3),
                        )
                    # Pool cannot read PSUM, and one DVE PSUM-read op costs
                    # ~0.76us vs the 0.85us group cadence — so each group
                    # carries exactly one DVE op; ACT copies shared products
                    # (M4, M5) to SBUF where Pool runs the second add.
                    if i == 2:
                        C[21][j] = pool.tile([P, NT], bf16, tag="X", bufs=32, name=f"d21_{j}")
                        C[22][j] = pool.tile([P, NT], bf16, tag="X", bufs=32, name=f"d22_{j}")
                        nc.scalar.copy(C[21][j], ps)
                        nc.vector.tensor_scalar_mul(C[22][j], ps, -1.0)
                    elif i == 1:
                        C[11][j] = pool.tile([P, NT], bf16, tag="X", bufs=32, name=f"d11_{j}")
                        nc.scalar.copy(C[11][j], ps)
                        nc.vector.tensor_add(C[22][j], C[22][j], ps)
                    elif i == 3:
                        C[12][j] = pool.tile([P, NT], bf16, tag="X", bufs=32, name=f"d12_{j}")
                        nc.scalar.copy(C[12][j], ps)
                        nc.vector.tensor_add(C[22][j], C[22][j], ps)
                    elif i == 4:
                        e4 = pool.tile([P, NT], bf16, tag="E", bufs=3, name=f"e4_{j}")
                        nc.scalar.copy(e4, ps)
                        nc.vector.tensor_add(C[11][j], C[11][j], ps)
                        nc.gpsimd.tensor_add(C[21][j], C[21][j], e4)
                        prelu(h0T[8 + j][:, lo], C[21][j], b0_sb[:, ts(8 + j, 1)])
                    elif i == 5:
                        e5 = pool.tile([P, NT], bf16, tag="E", bufs=3, name=f"e5_{j}")
                        nc.scalar.copy(e5, ps)
                        nc.vector.tensor_sub(C[11][j], C[11][j], ps)
                        nc.gpsimd.tensor_add(C[12][j], C[12][j], e5)
                        prelu(h0T[j][:, hi], C[12][j], b0_sb[:, ts(j, 1)])
                    elif i == 6:
                        nc.vector.tensor_add(C[22][j], C[22][j], ps)
                        prelu(h0T[8 + j][:, hi], C[22][j], b0_sb[:, ts(8 + j, 1)])
                    elif i == 7:
                        nc.vector.tensor_add(C[11][j], C[11][j], ps)
                        prelu(h0T[j][:, lo], C[11][j], b0_sb[:, ts(j, 1)])

            # =================== layer 1 (one-level Strassen) ===================
            # C = W1^T @ h0 over 1024x1024 quadrants: 7 half-size products
            # instead of 8 (PE rows 229376 vs 262144). Weight-side combos
            # S1..S7 are free on the host; activation-side combos T run on
            # the otherwise-idle Pool engine; products accumulate into the
            # four C quadrants via DVE reads of each product's PSUM bank.
            #   quadrant views of h0: Q11/Q12 = h0T[kk] cols lo/hi,
            #                         Q21/Q22 = h0T[8+kk] cols lo/hi
            # Product order is chosen by operand availability: L0 finishes
            # quadrants in the order C21(i4), C12(i5), C22(i6), C11(i7), so
            # L1 opens with M5 (pure Q22) and M7 (Q21+Q22) whose inputs are
            # ready a product-width before L0's PE stream even ends (T7's
            # combos were emitted inside the L0 loop for the same reason);
            # remaining combos are product-major so the Pool engine always
            # has a full product window of slack.
            for i, fn, sel in (
                (1, nc.gpsimd.tensor_add, lambda a, b_: (a[:, lo], b_[:, hi])),  # Q11+Q22
                (3, nc.gpsimd.tensor_sub, lambda a, b_: (a[:, hi], b_[:, hi])),  # Q12-Q22
                (4, nc.gpsimd.tensor_sub, lambda a, b_: (b_[:, lo], a[:, lo])),  # Q21-Q11
                (6, nc.gpsimd.tensor_add, lambda a, b_: (a[:, lo], a[:, hi])),   # Q11+Q12
            ):
                for kk in range(KH):
                    x0, x1 = sel(h0T[kk], h0T[8 + kk])
                    fn(Tc[i][kk], x0, x1)

            def mov1(i, kk):
                if i == 2:
                    return h0T[kk][:, lo]       # Q11
                if i == 5:
                    return h0T[8 + kk][:, hi]   # Q22
                return Tc[i][kk]

            h1T = [
                pool.tile([P, B], bf16, tag="H1", bufs=MT1, name=f"h1T{m}")
                for m in range(MT1)
            ]
            # product order by L0-output availability (see combo comment):
            #   C11 = M1 + M4 - M5 + M7   (built as -M5, +M7, +M1, +M4)
            #   C12 = M3 + M5             (copy M5, +M3)
            #   C21 = M2 + M4             (copy M2, +M4)
            #   C22 = M1 - M2 + M3 + M6   (built as -M2, +M1, +M3, +M6)
            PORDER = (5, 7, 2, 1, 3, 4, 6)
            for j in range(8):
                for i in PORDER:
                    st = pool.tile([P, KH * P], bf16, tag="W1", bufs=6, name=f"s{i}_{j}")
                    nc.sync.dma_start(out=st, in_=w1[i - 1, j])
                    ps = pp.tile([P, NT], f32, tag="pm", bufs=6, name=f"p{i}_{j}")
                    for kk in range(KH):
                        nc.tensor.matmul(
                            ps,
                            st[:, ts(kk, P)],
                            mov1(i, kk),
                            start=(kk == 0),
                            stop=(kk == KH - 1),
                        )
                    if i == 5:
                        c11 = pool.tile([P, NT], bf16, tag="X", bufs=32, name=f"c11_{j}")
                        c12 = pool.tile([P, NT], bf16, tag="X", bufs=32, name=f"c12_{j}")
                        nc.vector.tensor_scalar_mul(c11, ps, -1.0)
                        nc.scalar.copy(c12, ps)
                    elif i == 7:
                        nc.vector.tensor_add(c11, c11, ps)
                    elif i == 2:
                        c21 = pool.tile([P, NT], bf16, tag="X", bufs=32, name=f"c21_{j}")
                        c22 = pool.tile([P, NT], bf16, tag="X", bufs=32, name=f"c22_{j}")
                        nc.scalar.copy(c21, ps)
                        nc.vector.tensor_scalar_mul(c22, ps, -1.0)
                    elif i == 1:
                        nc.vector.tensor_add(c11, c11, ps)
                        nc.vector.tensor_add(c22, c22, ps)
                    elif i == 3:
                        nc.vector.tensor_add(c12, c12, ps)
                        nc.vector.tensor_add(c22, c22, ps)
                        nc.scalar.activation(
                            h1T[j][:, hi], c12, AF.Relu, bias=b1_sb[:, ts(j, 1)]
                        )
                    elif i == 4:
                        nc.vector.tensor_add(c11, c11, ps)
                        nc.vector.tensor_add(c21, c21, ps)
                        nc.scalar.activation(
                            h1T[8 + j][:, lo], c21, AF.Relu, bias=b1_sb[:, ts(8 + j, 1)]
                        )
                        nc.scalar.activation(
                            h1T[j][:, lo], c11, AF.Relu, bias=b1_sb[:, ts(j, 1)]
                        )
                    elif i == 6:
                        nc.vector.tensor_add(c22, c22, ps)
                        nc.scalar.activation(
                            h1T[8 + j][:, hi], c22, AF.Relu, bias=b1_sb[:, ts(8 + j, 1)]
                        )

            # =================== layer 2 (one-level Strassen) ===================
            # out = h1 @ W2 over quadrants: the W2-side combos arrive from the
            # host as the 7 moving operands (TW ring, product-outer like L0);
            # the h1-side stationary combos SC run on Pool, emitted in the
            # order products consume them. Finished C quadrants get their
            # (free-dim) bias added on Pool and store immediately.
            nc.gpsimd.dma_start(out=b2_sb, in_=b2[:, :])
            TW = {}

            def tw_load(i):
                TW[i] = [
                    pool.tile([P, NT], bf16, tag="TW", bufs=16, name=f"TW{i}_{kk}")
                    for kk in range(KH)
                ]
                for kk in range(KH):
                    nc.sync.dma_start(out=TW[i][kk], in_=w2[i - 1, kk])

            SC = {
                i: [
                    pool.tile([P, NT], bf16, tag="T", bufs=40, name=f"SC{i}_{kk}")
                    for kk in range(KH)
                ]
                for i in (1, 2, 5, 6, 7)
            }
            for i, fn, sel in (
                (5, nc.gpsimd.tensor_add, lambda a, b_: (a[:, lo], b_[:, lo])),  # P11+P12
                (6, nc.gpsimd.tensor_sub, lambda a, b_: (a[:, hi], a[:, lo])),   # P21-P11
                (2, nc.gpsimd.tensor_add, lambda a, b_: (a[:, hi], b_[:, hi])),  # P21+P22
                (1, nc.gpsimd.tensor_add, lambda a, b_: (a[:, lo], b_[:, hi])),  # P11+P22
                (7, nc.gpsimd.tensor_sub, lambda a, b_: (b_[:, lo], b_[:, hi])),  # P12-P22
            ):
                for kk in range(KH):
                    x0, x1 = sel(h1T[kk], h1T[8 + kk])
                    fn(SC[i][kk], x0, x1)

            def stat2(i, j, kk):
                if i == 3:
                    return h1T[kk][:, ts(j, P)]                    # P11
                if i == 4:
                    return h1T[8 + kk][:, NT + j * P : NT + (j + 1) * P]  # P22
                return SC[i][kk][:, ts(j, P)]

            # quadrant -> (out row block base, out col half)
            QOUT = {11: (0, 0), 12: (0, 1), 21: (1, 0), 22: (1, 1)}

            def store_quad(q, j, ctile, chunks=1):
                rbase, chalf = QOUT[q]
                cw2 = NT // chunks
                for c in range(chunks):
                    o5 = pool.tile([P, cw2], f32, tag="O5", bufs=8, name=f"o{q}_{j}_{c}")
                    nc.gpsimd.tensor_add(
                        o5, ctile[:, ts(c, cw2)],
                        b2_sb[:, chalf * NT + c * cw2 : chalf * NT + (c + 1) * cw2],
                    )
                    nc.scalar.dma_start(
                        out=out[
                            rbase * NT + j * P : rbase * NT + (j + 1) * P,
                            chalf * NT + c * cw2 : chalf * NT + (c + 1) * cw2,
                        ],
                        in_=o5,
                    )

            # C11 = M1+M4-M5+M7, C12 = M3+M5, C21 = M2+M4, C22 = M1-M2+M3+M6
            PORD2 = (5, 3, 2, 6, 1, 4, 7)
            D2 = {q: [None] * 4 for q in (11, 12, 21, 22)}
            tw_load(PORD2[0])
            tw_load(PORD2[1])
            for idx, i in enumerate(PORD2):
                if idx + 2 < len(PORD2):
                    tw_load(PORD2[idx + 2])
                for j in range(4):
                    if i == 7 and j == 3:
                        # very last group: run it as four 128-wide PSUM
                        # sub-groups so each chunk's drain (DVE add -> Pool
                        # bias -> store, alternating Act/sync queues)
                        # pipelines against the PE's remaining sub-groups
                        cw2 = NT // 4
                        for c in range(4):
                            sl = ts(c, cw2)
                            psc = pp.tile([P, cw2], f32, tag="pm", bufs=6, name=f"rf_{c}")
                            for kk in range(KH):
                                nc.tensor.matmul(
                                    psc,
                                    stat2(i, j, kk),
                                    TW[i][kk][:, sl],
                                    start=(kk == 0),
                                    stop=(kk == KH - 1),
                                )
                            # bias was folded at creation; one DVE op
                            # emits the store-ready f32 chunk — no Pool hop
                            o5 = pool.tile([P, cw2], f32, tag="O5", bufs=8, name=f"of_{c}")
                            nc.vector.tensor_add(o5, D2[11][j][:, sl], psc)
                            eng = (nc.scalar, nc.sync, nc.scalar, nc.gpsimd)[c]
                            eng.dma_start(
                                out=out[j * P : (j + 1) * P, c * cw2 : (c + 1) * cw2],
                                in_=o5,
                            )
                        continue
                    ps = pp.tile([P, NT], f32, tag="pm", bufs=6, name=f"r{i}_{j}")
                    for kk in range(KH):
                        nc.tensor.matmul(
                            ps,
                            stat2(i, j, kk),
                            TW[i][kk],
                            start=(kk == 0),
                            stop=(kk == KH - 1),
                        )
                    if i == 5:
                        D2[11][j] = pool.tile([P, NT], bf16, tag="X", bufs=32, name=f"g11_{j}")
                        D2[12][j] = pool.tile([P, NT], bf16, tag="X", bufs=32, name=f"g12_{j}")
                        if j == 3:
                            # bias folded at creation (C11 = b2 - M5 + ...)
                            # so the final drain chain skips the bias hop
                            nc.vector.tensor_sub(D2[11][j], b2_sb[:, lo], ps)
                        else:
                            nc.vector.tensor_scalar_mul(D2[11][j], ps, -1.0)
                        nc.scalar.copy(D2[12][j], ps)
                    elif i == 3:
                        D2[22][j] = pool.tile([P, NT], bf16, tag="X", bufs=32, name=f"g22_{j}")
                        nc.vector.tensor_add(D2[12][j], D2[12][j], ps)
                        nc.scalar.copy(D2[22][j], ps)
                        store_quad(12, j, D2[12][j])
                    elif i == 2:
                        D2[21][j] = pool.tile([P, NT], bf16, tag="X", bufs=32, name=f"g21_{j}")
                        nc.vector.tensor_sub(D2[22][j], D2[22][j], ps)
                        nc.scalar.copy(D2[21][j], ps)
                    elif i == 6:
                        nc.vector.tensor_add(D2[22][j], D2[22][j], ps)
                    elif i == 1:
                        nc.vector.tensor_add(D2[22][j], D2[22][j], ps)
                        nc.vector.tensor_add(D2[11][j], D2[11][j], ps)
                        store_quad(22, j, D2[22][j])
                    elif i == 4:
                        nc.vector.tensor_add(D2[21][j], D2[21][j], ps)
                        nc.vector.tensor_add(D2[11][j], D2[11][j], ps)
                        store_quad(21, j, D2[21][j])
                    elif i == 7:
                        if j < 3:
                            nc.vector.tensor_add(D2[11][j], D2[11][j], ps)
                            store_quad(11, j, D2[11][j])
                        else:
                            # final quadrant: chunk the whole accumulate ->
                            # bias -> store chain so the post-PE tail is short
                            cw2 = NT // 4
                            for c in range(4):
                                sl = ts(c, cw2)
                                nc.vector.tensor_add(
                                    D2[11][j][:, sl], D2[11][j][:, sl], ps[:, sl]
                                )
                                o5 = pool.tile([P, cw2], f32, tag="O5", bufs=8, name=f"of_{c}")
                                nc.gpsimd.tensor_add(
                                    o5, D2[11][j][:, sl], b2_sb[:, c * cw2 : (c + 1) * cw2]
                                )
                                nc.scalar.dma_start(
                                    out=out[j * P : (j + 1) * P, c * cw2 : (c + 1) * cw2],
                                    in_=o5,
                                )

    if not nc.is_finalized():
        nc.finalize()
    return nc


def _get_nc():
    if "nc" not in _CACHE:
        _CACHE["nc"] = _build()
    return _CACHE["nc"]


def _task_in_map(inputs, t, bf16, b0c, b1c, b2c):
    W0 = inputs["k0"] + SCALING * (inputs["d0"][:, :, t] @ inputs["u0"][:, :, t])
    W1 = inputs["k1"] + SCALING * (inputs["d1"][:, :, t] @ inputs["u1"][:, :, t])
    W2 = inputs["k2"] + SCALING * (inputs["d2"][:, :, t] @ inputs["u2"][:, :, t])
    # layer-0 Strassen: both operand sets host-combined.
    # stationary S_i from W0 (512 x 1024) blocks
    blk0 = W0.reshape(2, 512, 2, 1024)
    S0 = np.stack(
        [
            blk0[0, :, 0] + blk0[1, :, 1],
            blk0[0, :, 1] + blk0[1, :, 1],
            blk0[0, :, 0],
            blk0[1, :, 1],
            blk0[0, :, 0] + blk0[1, :, 0],
            blk0[0, :, 1] - blk0[0, :, 0],
            blk0[1, :, 0] - blk0[1, :, 1],
        ]
    )  # [7, K/2, M/2]
    w0r = np.ascontiguousarray(
        S0.reshape(7, 4, 128, 8, 128).transpose(0, 3, 2, 1, 4).reshape(7, 8, 128, 512),
        dtype=bf16,
    )
    # moving combos T_i from x^T quadrants
    xT = inputs["x"][t].T
    Q11, Q12 = xT[:512, :512], xT[:512, 512:]
    Q21, Q22 = xT[512:, :512], xT[512:, 512:]
    xsr = np.ascontiguousarray(
        np.stack(
            [Q11 + Q22, Q11, Q12 - Q22, Q21 - Q11, Q22, Q11 + Q12, Q21 + Q22]
        ),
        dtype=bf16,
    )
    # layer-1 Strassen stationary operands from W1 quadrants blk[r, c]
    blk = W1.reshape(2, 1024, 2, 1024)
    S = np.stack(
        [
            blk[0, :, 0] + blk[1, :, 1],  # (P11+P22)^T
            blk[0, :, 1] + blk[1, :, 1],  # (P21+P22)^T
            blk[0, :, 0],                 # P11^T
            blk[1, :, 1],                 # P22^T
            blk[0, :, 0] + blk[1, :, 0],  # (P11+P12)^T
            blk[0, :, 1] - blk[0, :, 0],  # (P21-P11)^T
            blk[1, :, 0] - blk[1, :, 1],  # (P12-P22)^T
        ]
    )  # [7, K/2, M/2]
    w1r = np.ascontiguousarray(
        S.reshape(7, 8, 128, 8, 128).transpose(0, 3, 2, 1, 4).reshape(7, 8, 128, 1024),
        dtype=bf16,
    )
    # layer-2 Strassen moving operands from W2 (1024 x 512) blocks
    blk2 = W2.reshape(2, 1024, 2, 512)
    S2_ = np.stack(
        [
            blk2[0, :, 0] + blk2[1, :, 1],
            blk2[0, :, 0],
            blk2[0, :, 1] - blk2[1, :, 1],
            blk2[1, :, 0] - blk2[0, :, 0],
            blk2[1, :, 1],
            blk2[0, :, 0] + blk2[0, :, 1],
            blk2[1, :, 0] + blk2[1, :, 1],
        ]
    )  # [7, K/2, M3/2]
    w2r = np.ascontiguousarray(S2_.reshape(7, 8, 128, 512), dtype=bf16)
    return {
        "xt": xsr,
        "w0": w0r,
        "b0": b0c,
        "w1": w1r,
        "b1": b1c,
        "w2": w2r,
        "b2": b2c,
    }


def build_in_maps(inputs):
    import concurrent.futures

    import ml_dtypes

    bf16 = ml_dtypes.bfloat16
    b0c = np.ascontiguousarray(inputs["b0"].reshape(16, 128).T, dtype=np.float32)
    b1c = np.ascontiguousarray(inputs["b1"].reshape(16, 128).T, dtype=np.float32)
    b2c = np.ascontiguousarray(
        np.broadcast_to(inputs["b2"], (P, H3)), dtype=np.float32
    )
    with concurrent.futures.ThreadPoolExecutor(max_workers=T) as ex:
        in_maps = list(
            ex.map(lambda t: _task_in_map(inputs, t, bf16, b0c, b1c, b2c), range(T))
        )
    return in_maps


def kernel(**inputs):
    from concourse import bass_utils

    nc = _get_nc()
    in_maps = build_in_maps(inputs)
    res = bass_utils.run_bass_kernel_spmd(nc, in_maps, core_ids=list(range(T)))
    return np.stack([r["out"] for r in res.results], axis=0)


# revision 26
# speedup vs baseline: 1.1422x; 1.0003x over previous
"""Trainium2 Bass kernel for 3-layer per-task LoRA MLP.

Full-input contract: kernel(**inputs) takes the unsharded tensors and returns
the full [8, 1024, 1024] output. Internally the task axis (t=8) is sharded
across 8 NeuronCores (one task per core).

Strategy:
  - LoRA is folded on the host into per-task effective weights
    W_eff = k + (alpha/r) * d @ u  (standard LoRA weight merging), so the
    device kernel is a plain 3-layer MLP — no rank-8 matmuls on the PE.
  - weights and activations are bf16 on device (1 cycle/row on the PE, same
    as f32r, but half the DMA traffic and SBUF footprint); PSUM accumulation
    stays f32.
  - every layer runs one level of Strassen over 2x2 quadrant blocks: 7
    half-size products instead of 8 cuts PE row-streaming by 12.5% per
    layer. Weight-side operand combinations are free on the host; the
    activation-side combinations and C-quadrant accumulations are spread
    across the Pool/DVE/Act engines (Pool cannot read PSUM; DVE PSUM reads
    cost ~0.76us so each PSUM group gets at most one), overlapped so the
    PE stream stays gap-free. Measured pipeline error ~1e-2 relative
    (gate is 2e-2).
  - activations live as h^T [feat(part), batch(free)] with zero on-device
    transposes; the final layer uses h1^T slices as the *stationary*
    operand, producing natural-layout [batch, feat] output directly.
  - all host-side packs match the exact SBUF tile layout so every DMA is a
    contiguous >=1KB-per-partition stream, one DMA per consumed tile
    (consumers gate on whole-tile writes), spread over the three DMA
    channels (SP/Act HWDGE + Pool SWDGE) in consumption order.
  - a short PE warmup ramps the tensor-engine clock to 2.4 GHz while the
    first DMAs land; the final output quadrant drains in 128-wide chunks
    to shorten the post-PE tail.
"""

import sys

if "/opt/trn_rl_repo" not in sys.path:
    sys.path.insert(0, "/opt/trn_rl_repo")

import numpy as np

T, B, D = 8, 1024, 1024
H1, H2, H3 = 2048, 2048, 1024
SCALING = 2.0  # alpha/rank = 16/8
P = 128
NT = 512  # PSUM free-dim tile (fp32 one-bank limit)

_CACHE = {}


def _build():
    import concourse.mybir as mybir
    from concourse import bacc
    from concourse.tile import TileContext
    from concourse.bass import ts

    f32 = mybir.dt.float32
    bf16 = mybir.dt.bfloat16
    AF = mybir.ActivationFunctionType

    nc = bacc.Bacc(None, target_bir_lowering=False, name="lora_mlp")

    KT0 = D // P      # 8  k-tiles, layer 0
    KT1 = H1 // P     # 16 k-tiles, layer 1
    KT2 = H2 // P     # 16 k-tiles, layer 2
    MT0 = H1 // P     # 16 m-tiles, layer 0
    MT1 = H2 // P     # 16 m-tiles, layer 1
    BT = B // P       # 8  batch 128-tiles
    NB = B // NT      # 2  batch 512-halves (free dim, layers 0/1)
    N2 = H3 // NT     # 2  feature 512-halves (free dim, layer 2)

    # layer-0 inputs arrive Strassen-ready: xs = the 7 moving-operand
    # combos of x^T quadrants [K/2, B/2]; w0 = the 7 stationary operands
    xs = nc.dram_tensor("xt", (7, D // 2, B // 2), bf16, kind="ExternalInput")
    w0 = nc.dram_tensor("w0", (7, 8, P, 4 * P), bf16, kind="ExternalInput")
    b0 = nc.dram_tensor("b0", (P, MT0), f32, kind="ExternalInput")
    # layer-1 weights arrive as the 7 Strassen stationary operands
    # S_i [K/2, M/2], host-combined and packed [i, j(m-tile), p, kk*128+c]
    w1 = nc.dram_tensor("w1", (7, 8, P, 8 * P), bf16, kind="ExternalInput")
    b1 = nc.dram_tensor("b1", (P, MT1), f32, kind="ExternalInput")
    w2 = nc.dram_tensor("w2", (7, 8, P, H3 // 2), bf16, kind="ExternalInput")
    b2 = nc.dram_tensor("b2", (P, H3), f32, kind="ExternalInput")
    out = nc.dram_tensor("out", (B, H3), f32, kind="ExternalOutput")

    with TileContext(nc) as tc:
        with (
            tc.tile_pool(name="main", bufs=1) as pool,
            tc.tile_pool(name="psum", bufs=1, space="PSUM") as pp,
        ):
            # PE warmup: the tensor engine clock ramps with sustained use
            # (0.65 -> 1.2 -> 2.4 GHz over ~3us). Run throwaway matmuls on a
            # memset tile while the first input DMAs land, so the real
            # matmuls start at full clock.
            wu = pool.tile([P, P], bf16, tag="wu", bufs=1)
            nc.vector.memset(wu, 0.125)
            wps = pp.tile([P, P], f32, tag="wps", bufs=1)
            for _ in range(4):
                nc.tensor.matmul(wps, wu, wu, start=True, stop=True)

            b0_sb = pool.tile([P, MT0], f32, tag="b0", bufs=1)
            b1_sb = pool.tile([P, MT1], f32, tag="b1", bufs=1)
            b2_sb = pool.tile([P, H3], f32, tag="b2", bufs=1)

            # =================== layer 0 (one-level Strassen) ===================
            # C = W0^T @ x over (512 x 1024) quadrant blocks; both operand
            # combos come pre-built on the host, so the device only runs the
            # 7 products and the C-quadrant accumulation. The product loop is
            # OUTER so each product's 4 moving tiles (TX ring, prefetched one
            # product ahead on Pool/Act queues) serve 8 consecutive groups —
            # the DMA stream never races the PE. First-product copies into
            # each C quadrant run on the Act engine, accumulating adds on
            # DVE, bias+relu on Pool (tensor_scalar add+max).
            h0T = [
                pool.tile([P, B], bf16, tag="H0", bufs=MT0, name=f"h0T{m}")
                for m in range(MT0)
            ]
            lo, hi = ts(0, NT), ts(1, NT)
            PORD0 = (2, 1, 3, 4, 5, 6, 7)
            alu = mybir.AluOpType

            def prelu(dst, src, bcol):
                nc.gpsimd.tensor_scalar(dst, src, bcol, 0.0, alu.add, alu.max)

            TX = {}

            def tx_load(i):
                TX[i] = [
                    pool.tile([P, NT], bf16, tag="TX", bufs=8, name=f"TX{i}_{kk}")
                    for kk in range(4)
                ]
                for kk in range(4):
                    eng = nc.gpsimd if kk < 2 else nc.scalar
                    eng.dma_start(out=TX[i][kk], in_=xs[i - 1, ts(kk, P), :])

            C = {q: [None] * 8 for q in (11, 12, 21, 22)}
            KH = KT1 // 2  # 8 k-tiles per K-half of layer 1
            Tc = {
                i: [
                    pool.tile([P, NT], bf16, tag="T", bufs=40, name=f"T{i}_{kk}")
                    for kk in range(KH)
                ]
                for i in (1, 3, 4, 6, 7)
            }
            with tc.high_priority():
                tx_load(PORD0[0])
                tx_load(PORD0[1])
                nc.gpsimd.dma_start(out=b0_sb, in_=b0[:, :])
                nc.gpsimd.dma_start(out=b1_sb, in_=b1[:, :])
            for idx, i in enumerate(PORD0):
                if idx + 2 < len(PORD0):
                    tx_load(PORD0[idx + 2])
                if i == 7:
                    # layer-1's T7 combos (Q21+Q22) only need L0's C21/C22
                    # quadrants — emit them ahead of i7's Pool relus, split
                    # across Pool and DVE so they emerge 2x faster than
                    # L1's second product consumes them
                    for kk in range(KH):
                        eng = nc.gpsimd if kk % 2 == 0 else nc.vector
                        eng.tensor_add(
                            Tc[7][kk], h0T[8 + kk][:, lo], h0T[8 + kk][:, hi]
                        )
                for j in range(8):
                    st = pool.tile([P, 4 * P], bf16, tag="W0s", bufs=6, name=f"s0_{i}_{j}")
                    nc.sync.dma_start(out=st, in_=w0[i - 1, j])
                    ps = pp.tile([P, NT], f32, tag="pm", bufs=6, name=f"q{i}_{j}")
                    for kk in range(4):
                        nc.tensor.matmul(
                            ps,
                            st[:, ts(kk, P)],
                            TX[i][kk],
                            start=(kk == 0),
                            stop=(kk == 3),
                        )
                    # Pool cannot read PSUM, and one DVE PSUM-read op costs
                    # ~0.76us vs the 0.85us group cadence — so each group
                    # carries exactly one DVE op; ACT copies shared products
                    # (M4, M5) to SBUF where Pool runs the second add.
                    if i == 2:
                        C[21][j] = pool.tile([P, NT], bf16, tag="X", bufs=32, name=f"d21_{j}")
                        C[22][j] = pool.tile([P, NT], bf16, tag="X", bufs=32, name=f"d22_{j}")
                        nc.scalar.copy(C[21][j], ps)
                        nc.vector.tensor_scalar_mul(C[22][j], ps, -1.0)
                    elif i == 1:
                        C[11][j] = pool.tile([P, NT], bf16, tag="X", bufs=32, name=f"d11_{j}")
                        nc.scalar.copy(C[11][j], ps)
                        nc.vector.tensor_add(C[22][j], C[22][j], ps)
                    elif i == 3:
                        C[12][j] = pool.tile([P, NT], bf16, tag="X", bufs=32, name=f"d12_{j}")
                        nc.scalar.copy(C[12][j], ps)
                        nc.vector.tensor_add(C[22][j], C[22][j], ps)
                    elif i == 4:
                        e4 = pool.tile([P, NT], bf16, tag="E", bufs=3, name=f"e4_{j}")
                        nc.scalar.copy(e4, ps)
                        nc.vector.tensor_add(C[11][j], C[11][j], ps)
                        nc.gpsimd.tensor_add(C[21][j], C[21][j], e4)
                        prelu(h0T[8 + j][:, lo], C[21][j], b0_sb[:, ts(8 + j, 1)])
                    elif i == 5:
                        e5 = pool.tile([P, NT], bf16, tag="E", bufs=3, name=f"e5_{j}")
                        nc.scalar.copy(e5, ps)
                        nc.vector.tensor_sub(C[11][j], C[11][j], ps)
                        nc.gpsimd.tensor_add(C[12][j], C[12][j], e5)
                        prelu(h0T[j][:, hi], C[12][j], b0_sb[:, ts(j, 1)])
                    elif i == 6:
                        nc.vector.tensor_add(C[22][j], C[22][j], ps)
                        prelu(h0T[8 + j][:, hi], C[22][j], b0_sb[:, ts(8 + j, 1)])
                    elif i == 7:
                        nc.vector.tensor_add(C[11][j], C[11][j], ps)
                        prelu(h0T[j][:, lo], C[11][j], b0_sb[:, ts(j, 1)])

            # =================== layer 1 (one-level Strassen) ===================
            # C = W1^T @ h0 over 1024x1024 quadrants: 7 half-size products
            # instead of 8 (PE rows 229376 vs 262144). Weight-side combos
            # S1..S7 are free on the host; activation-side combos T run on
            # the otherwise-idle Pool engine; products accumulate into the
            # four C quadrants via DVE reads of each product's PSUM bank.
            #   quadrant views of h0: Q11/Q12 = h0T[kk] cols lo/hi,
            #                         Q21/Q22 = h0T[8+kk] cols lo/hi
            # Product order is chosen by operand availability: L0 finishes
            # quadrants in the order C21(i4), C12(i5), C22(i6), C11(i7), so
            # L1 opens with M5 (pure Q22) and M7 (Q21+Q22) whose inputs are
            # ready a product-width before L0's PE stream even ends (T7's
            # combos were emitted inside the L0 loop for the same reason);
            # remaining combos are product-major so the Pool engine always
            # has a full product window of slack.
            for i, fn, sel in (
                (1, nc.gpsimd.tensor_add, lambda a, b_: (a[:, lo], b_[:, hi])),  # Q11+Q22
                (3, nc.gpsimd.tensor_sub, lambda a, b_: (a[:, hi], b_[:, hi])),  # Q12-Q22
                (4, nc.gpsimd.tensor_sub, lambda a, b_: (b_[:, lo], a[:, lo])),  # Q21-Q11
                (6, nc.gpsimd.tensor_add, lambda a, b_: (a[:, lo], a[:, hi])),   # Q11+Q12
            ):
                for kk in range(KH):
                    x0, x1 = sel(h0T[kk], h0T[8 + kk])
                    fn(Tc[i][kk], x0, x1)

            def mov1(i, kk):
                if i == 2:
                    return h0T[kk][:, lo]       # Q11
                if i == 5:
                    return h0T[8 + kk][:, hi]   # Q22
                return Tc[i][kk]

            h1T = [
                pool.tile([P, B], bf16, tag="H1", bufs=MT1, name=f"h1T{m}")
                for m in range(MT1)
            ]
            # product order by L0-output availability (see combo comment):
            #   C11 = M1 + M4 - M5 + M7   (built as -M5, +M7, +M1, +M4)
            #   C12 = M3 + M5             (copy M5, +M3)
            #   C21 = M2 + M4             (copy M2, +M4)
            #   C22 = M1 - M2 + M3 + M6   (built as -M2, +M1, +M3, +M6)
            PORDER = (5, 7, 2, 1, 3, 4, 6)
            for j in range(8):
                for i in PORDER:
                    st = pool.tile([P, KH * P], bf16, tag="W1", bufs=6, name=f"s{i}_{j}")
                    nc.sync.dma_start(out=st, in_=w1[i - 1, j])
                    ps = pp.tile([P, NT], f32, tag="pm", bufs=6, name=f"p{i}_{j}")
                    for kk in range(KH):
                        nc.tensor.matmul(
                            ps,
                            st[:, ts(kk, P)],
                            mov1(i, kk),
                            start=(kk == 0),
                            stop=(kk == KH - 1),
                        )
                    if i == 5:
                        c11 = pool.tile([P, NT], bf16, tag="X", bufs=32, name=f"c11_{j}")
                        c12 = pool.tile([P, NT], bf16, tag="X", bufs=32, name=f"c12_{j}")
                        nc.vector.tensor_scalar_mul(c11, ps, -1.0)
                        nc.scalar.copy(c12, ps)
                    elif i == 7:
                        nc.vector.tensor_add(c11, c11, ps)
                    elif i == 2:
                        c21 = pool.tile([P, NT], bf16, tag="X", bufs=32, name=f"c21_{j}")
                        c22 = pool.tile([P, NT], bf16, tag="X", bufs=32, name=f"c22_{j}")
                        nc.scalar.copy(c21, ps)
                        nc.vector.tensor_scalar_mul(c22, ps, -1.0)
                    elif i == 1:
                        nc.vector.tensor_add(c11, c11, ps)
                        nc.vector.tensor_add(c22, c22, ps)
                    elif i == 3:
                        nc.vector.tensor_add(c12, c12, ps)
                        nc.vector.tensor_add(c22, c22, ps)
                        nc.scalar.activation(
                            h1T[j][:, hi], c12, AF.Relu, bias=b1_sb[:, ts(j, 1)]
                        )
                    elif i == 4:
                        nc.vector.tensor_add(c11, c11, ps)
                        nc.vector.tensor_add(c21, c21, ps)
                        nc.scalar.activation(
                            h1T[8 + j][:, lo], c21, AF.Relu, bias=b1_sb[:, ts(8 + j, 1)]
                        )
                        nc.scalar.activation(
                            h1T[j][:, lo], c11, AF.Relu, bias=b1_sb[:, ts(j, 1)]
                        )
                    elif i == 6:
                        nc.vector.tensor_add(c22, c22, ps)
                        nc.scalar.activation(
                            h1T[8 + j][:, hi], c22, AF.Relu, bias=b1_sb[:, ts(8 + j, 1)]
                        )

            # =================== layer 2 (one-level Strassen) ===================
            # out = h1 @ W2 over quadrants: the W2-side combos arrive from the
            # host as the 7 moving operands (TW ring, product-outer like L0);
            # the h1-side stationary combos SC run on Pool, emitted in the
            # order products consume them. Finished C quadrants get their
            # (free-dim) bias added on Pool and store immediately.
            nc.gpsimd.dma_start(out=b2_sb, in_=b2[:, :])
            TW = {}

            def tw_load(i):
                TW[i] = [
                    pool.tile([P, NT], bf16, tag="TW", bufs=16, name=f"TW{i}_{kk}")
                    for kk in range(KH)
                ]
                for kk in range(KH):
                    nc.sync.dma_start(out=TW[i][kk], in_=w2[i - 1, kk])

            SC = {
                i: [
                    pool.tile([P, NT], bf16, tag="T", bufs=40, name=f"SC{i}_{kk}")
                    for kk in range(KH)
                ]
                for i in (1, 2, 5, 6, 7)
            }
            for i, fn, sel in (
                (5, nc.gpsimd.tensor_add, lambda a, b_: (a[:, lo], b_[:, lo])),  # P11+P12
                (6, nc.gpsimd.tensor_sub, lambda a, b_: (a[:, hi], a[:, lo])),   # P21-P11
                (2, nc.gpsimd.tensor_add, lambda a, b_: (a[:, hi], b_[:, hi])),  # P21+P22
                (1, nc.gpsimd.tensor_add, lambda a, b_: (a[:, lo], b_[:, hi])),  # P11+P22
                (7, nc.gpsimd.tensor_sub, lambda a, b_: (b_[:, lo], b_[:, hi])),  # P12-P22
            ):
                for kk in range(KH):
                    x0, x1 = sel(h1T[kk], h1T[8 + kk])
                    fn(SC[i][kk], x0, x1)

            def stat2(i, j, kk):
                if i == 3:
                    return h1T[kk][:, ts(j, P)]                    # P11
                if i == 4:
                    return h1T[8 + kk][:, NT + j * P : NT + (j + 1) * P]  # P22
                return SC[i][kk][:, ts(j, P)]

            # quadrant -> (out row block base, out col half)
            QOUT = {11: (0, 0), 12: (0, 1), 21: (1, 0), 22: (1, 1)}

            def store_quad(q, j, ctile, chunks=1):
                rbase, chalf = QOUT[q]
                cw2 = NT // chunks
                for c in range(chunks):
                    o5 = pool.tile([P, cw2], f32, tag="O5", bufs=8, name=f"o{q}_{j}_{c}")
                    nc.gpsimd.tensor_add(
                        o5, ctile[:, ts(c, cw2)],
                        b2_sb[:, chalf * NT + c * cw2 : chalf * NT + (c + 1) * cw2],
                    )
                    nc.scalar.dma_start(
                        out=out[
                            rbase * NT + j * P : rbase * NT + (j + 1) * P,
                            chalf * NT + c * cw2 : chalf * NT + (c + 1) * cw2,
                        ],
                        in_=o5,
                    )

            # C11 = M1+M4-M5+M7, C12 = M3+M5, C21 = M2+M4, C22 = M1-M2+M3+M6
            PORD2 = (5, 3, 2, 6, 1, 4, 7)
            D2 = {q: [None] * 4 for q in (11, 12, 21, 22)}
            tw_load(PORD2[0])
            tw_load(PORD2[1])
            for idx, i in enumerate(PORD2):
                if idx + 2 < len(PORD2):
                    tw_load(PORD2[idx + 2])
                for j in range(4):
                    if i == 7 and j == 3:
                        # very last group: run it as four 128-wide PSUM
                        # sub-groups so each chunk's drain (DVE add -> Pool
                        # bias -> store, alternating Act/sync queues)
                        # pipelines against the PE's remaining sub-groups
                        cw2 = NT // 4
                        for c in range(4):
                            sl = ts(c, cw2)
                            psc = pp.tile([P, cw2], f32, tag="pm", bufs=6, name=f"rf_{c}")
                            for kk in range(KH):
                                nc.tensor.matmul(
                                    psc,
                                    stat2(i, j, kk),
                                    TW[i][kk][:, sl],
                                    start=(kk == 0),
                                    stop=(kk == KH - 1),
                                )
                            # bias was folded at creation; one DVE op
                            # emits the store-ready f32 chunk — no Pool hop
                            o5 = pool.tile([P, cw2], f32, tag="O5", bufs=8, name=f"of_{c}")
                            nc.vector.tensor_add(o5, D2[11][j][:, sl], psc)
                            eng = (nc.scalar, nc.gpsimd, nc.sync, nc.scalar)[c]
                            eng.dma_start(
                                out=out[j * P : (j + 1) * P, c * cw2 : (c + 1) * cw2],
                                in_=o5,
                            )
                        continue
                    ps = pp.tile([P, NT], f32, tag="pm", bufs=6, name=f"r{i}_{j}")
                    for kk in range(KH):
                        nc.tensor.matmul(
                            ps,
                            stat2(i, j, kk),
                            TW[i][kk],
                            start=(kk == 0),
                            stop=(kk == KH - 1),
                        )
                    if i == 5:
                        D2[11][j] = pool.tile([P, NT], bf16, tag="X", bufs=32, name=f"g11_{j}")
                        D2[12][j] = pool.tile([P, NT], bf16, tag="X", bufs=32, name=f"g12_{j}")
                        if j == 3:
                            # bias folded at creation (C11 = b2 - M5 + ...)
                            # so the final drain chain skips the bias hop
                            nc.vector.tensor_sub(D2[11][j], b2_sb[:, lo], ps)
                        else:
                            nc.vector.tensor_scalar_mul(D2[11][j], ps, -1.0)
                        nc.scalar.copy(D2[12][j], ps)
                    elif i == 3:
                        D2[22][j] = pool.tile([P, NT], bf16, tag="X", bufs=32, name=f"g22_{j}")
                        nc.vector.tensor_add(D2[12][j], D2[12][j], ps)
                        nc.scalar.copy(D2[22][j], ps)
                        store_quad(12, j, D2[12][j])
                    elif i == 2:
                        D2[21][j] = pool.tile([P, NT], bf16, tag="X", bufs=32, name=f"g21_{j}")
                        nc.vector.tensor_sub(D2[22][j], D2[22][j], ps)
                        nc.scalar.copy(D2[21][j], ps)
                    elif i == 6:
                        nc.vector.tensor_add(D2[22][j], D2[22][j], ps)
                    elif i == 1:
                        nc.vector.tensor_add(D2[22][j], D2[22][j], ps)
                        nc.vector.tensor_add(D2[11][j], D2[11][j], ps)
                        store_quad(22, j, D2[22][j])
                    elif i == 4:
                        nc.vector.tensor_add(D2[21][j], D2[21][j], ps)
                        nc.vector.tensor_add(D2[11][j], D2[11][j], ps)
                        store_quad(21, j, D2[21][j])
                    elif i == 7:
                        if j < 3:
                            nc.vector.tensor_add(D2[11][j], D2[11][j], ps)
                            store_quad(11, j, D2[11][j])
                        else:
                            # final quadrant: chunk the whole accumulate ->
                            # bias -> store chain so the post-PE tail is short
                            cw2 = NT // 4
                            for c in range(4):
                                sl = ts(c, cw2)
                                nc.vector.tensor_add(
                                    D2[11][j][:, sl], D2[11][j][:, sl], ps[:, sl]
                                )
                                o5 = pool.tile([P, cw2], f32, tag="O5", bufs=8, name=f"of_{c}")
                                nc.gpsimd.tensor_add(
                                    o5, D2[11][j][:, sl], b2_sb[:, c * cw2 : (c + 1) * cw2]
                                )
                                nc.scalar.dma_start(
                                    out=out[j * P : (j + 1) * P, c * cw2 : (c + 1) * cw2],
                                    in_=o5,
                                )

    if not nc.is_finalized():
        nc.finalize()
    return nc


def _get_nc():
    if "nc" not in _CACHE:
        _CACHE["nc"] = _build()
    return _CACHE["nc"]


def _task_in_map(inputs, t, bf16, b0c, b1c, b2c):
    W0 = inputs["k0"] + SCALING * (inputs["d0"][:, :, t] @ inputs["u0"][:, :, t])
    W1 = inputs["k1"] + SCALING * (inputs["d1"][:, :, t] @ inputs["u1"][:, :, t])
    W2 = inputs["k2"] + SCALING * (inputs["d2"][:, :, t] @ inputs["u2"][:, :, t])
    # layer-0 Strassen: both operand sets host-combined.
    # stationary S_i from W0 (512 x 1024) blocks
    blk0 = W0.reshape(2, 512, 2, 1024)
    S0 = np.stack(
        [
            blk0[0, :, 0] + blk0[1, :, 1],
            blk0[0, :, 1] + blk0[1, :, 1],
            blk0[0, :, 0],
            blk0[1, :, 1],
            blk0[0, :, 0] + blk0[1, :, 0],
            blk0[0, :, 1] - blk0[0, :, 0],
            blk0[1, :, 0] - blk0[1, :, 1],
        ]
    )  # [7, K/2, M/2]
    w0r = np.ascontiguousarray(
        S0.reshape(7, 4, 128, 8, 128).transpose(0, 3, 2, 1, 4).reshape(7, 8, 128, 512),
        dtype=bf16,
    )
    # moving combos T_i from x^T quadrants
    xT = inputs["x"][t].T
    Q11, Q12 = xT[:512, :512], xT[:512, 512:]
    Q21, Q22 = xT[512:, :512], xT[512:, 512:]
    xsr = np.ascontiguousarray(
        np.stack(
            [Q11 + Q22, Q11, Q12 - Q22, Q21 - Q11, Q22, Q11 + Q12, Q21 + Q22]
        ),
        dtype=bf16,
    )
    # layer-1 Strassen stationary operands from W1 quadrants blk[r, c]
    blk = W1.reshape(2, 1024, 2, 1024)
    S = np.stack(
        [
            blk[0, :, 0] + blk[1, :, 1],  # (P11+P22)^T
            blk[0, :, 1] + blk[1, :, 1],  # (P21+P22)^T
            blk[0, :, 0],                 # P11^T
            blk[1, :, 1],                 # P22^T
            blk[0, :, 0] + blk[1, :, 0],  # (P11+P12)^T
            blk[0, :, 1] - blk[0, :, 0],  # (P21-P11)^T
            blk[1, :, 0] - blk[1, :, 1],  # (P12-P22)^T
        ]
    )  # [7, K/2, M/2]
    w1r = np.ascontiguousarray(
        S.reshape(7, 8, 128, 8, 128).transpose(0, 3, 2, 1, 4).reshape(7, 8, 128, 1024),
        dtype=bf16,
    )
    # layer-2 Strassen moving operands from W2 (1024 x 512) blocks
    blk2 = W2.reshape(2, 1024, 2, 512)
    S2_ = np.stack(
        [
            blk2[0, :, 0] + blk2[1, :, 1],
            blk2[0, :, 0],
            blk2[0, :, 1] - blk2[1, :, 1],
            blk2[1, :, 0] - blk2[0, :, 0],
            blk2[1, :, 1],
            blk2[0, :, 0] + blk2[0, :, 1],
            blk2[1, :, 0] + blk2[1, :, 1],
        ]
    )  # [7, K/2, M3/2]
    w2r = np.ascontiguousarray(S2_.reshape(7, 8, 128, 512), dtype=bf16)
    return {
        "xt": xsr,
        "w0": w0r,
        "b0": b0c,
        "w1": w1r,
        "b1": b1c,
        "w2": w2r,
        "b2": b2c,
    }


def build_in_maps(inputs):
    import concurrent.futures

    import ml_dtypes

    bf16 = ml_dtypes.bfloat16
    b0c = np.ascontiguousarray(inputs["b0"].reshape(16, 128).T, dtype=np.float32)
    b1c = np.ascontiguousarray(inputs["b1"].reshape(16, 128).T, dtype=np.float32)
    b2c = np.ascontiguousarray(
        np.broadcast_to(inputs["b2"], (P, H3)), dtype=np.float32
    )
    with concurrent.futures.ThreadPoolExecutor(max_workers=T) as ex:
        in_maps = list(
            ex.map(lambda t: _task_in_map(inputs, t, bf16, b0c, b1c, b2c), range(T))
        )
    return in_maps


def kernel(**inputs):
    from concourse import bass_utils

    nc = _get_nc()
    in_maps = build_in_maps(inputs)
    res = bass_utils.run_bass_kernel_spmd(nc, in_maps, core_ids=list(range(T)))
    return np.stack([r["out"] for r in res.results], axis=0)
